# revision 1
# baseline (speedup 1.0000x reference)
"""GAT (2-head) + 3x dense/LayerNorm + pairwise-distance kernel for 8 TRN2 NeuronCores.

Strategy: dst-sharded edge processing (one-hot matmul aggregation), replicated
small dense weights, row-block-sharded NxN cdist output.
"""
import sys
import types

import numpy as np

# Environment bootstrap (harness may run from a bare directory).
for _p in ("/root/.axon_site", "/root/.axon_site/_ro/trn_rl_repo",
           "/root/.axon_site/_ro/pypackages", "/opt/trn_rl_repo"):
    if _p not in sys.path:
        sys.path.append(_p)

import concourse.bass as bass
import concourse.bacc as bacc
import concourse.mybir as mybir
import concourse.tile as tile
from concourse.masks import make_identity
from concourse.bass_utils import run_bass_kernel_spmd

dt = mybir.dt
OP = mybir.AluOpType
AF = mybir.ActivationFunctionType

N = 10000
NPAD = 10112          # 79 * 128
NB = 79               # node blocks (phase A)
FIN = 256
F = 128               # per-head GAT dim
H = 2
HROW = 320            # htable row floats (1280B, multiple of 256B)
ADROW = 64            # adtable row floats (256B)
CORES = 8
SHARD = 1250          # dst nodes per core
RB = 10               # dst blocks per core (9 full + 98)
RPAD = 1280
CCH = 512             # cdist column chunk
NCOL = 10240          # padded output columns
EPS = 1e-5

_BUILD_CACHE = {}
_LAST_RESULTS = None


GC = 6  # tile groups per dma_gather call (768 idxs = 48 descs/engine; 2 calls fit the 128-deep ring)


def _build(TB, phases="ABCDE"):
    """Build the 8-core SPMD program. TB = gather tile groups per dst block (mult of GC)."""
    assert TB % GC == 0
    NCH = TB // GC
    S16 = 8 * TB          # idx columns ([128, S16] wrapped int16)

    nc = bacc.Bacc("TRN2", target_bir_lowering=False, debug=False, num_devices=CORES)

    def din(name, shape, d=dt.float32):
        return nc.dram_tensor(name, shape, d, kind="ExternalInput").ap()

    xt = din("xt", [FIN, NPAD])
    wg = din("wg", [FIN, FIN])
    att_src = din("att_src", [H, F])
    att_dst = din("att_dst", [H, F])
    bgat = din("bgat", [1, FIN])
    wa = din("wa", [128, 256])
    ba = din("ba", [1, 128])
    lnaw = din("lnaw", [1, 128])
    lnab = din("lnab", [1, 128])
    w1 = din("w1", [64, 128])
    b1 = din("b1", [1, 64])
    ln1w = din("ln1w", [1, 64])
    ln1b = din("ln1b", [1, 64])
    w2 = din("w2", [32, 64])
    b2 = din("b2", [1, 32])
    ln2w = din("ln2w", [1, 32])
    ln2b = din("ln2b", [1, 32])
    w3 = din("w3", [3, 32])
    b3 = din("b3", [1, 3])
    hidx = din("hidx", [RB, 128, S16], dt.int16)
    adrows = din("adrows", [RB, 128, 8], dt.int16)
    dstloc = din("dstloc", [RB, 128, TB])
    dstrow = din("dstrow", [RB, 1, TB * 128])
    outD = nc.dram_tensor("outD", [RPAD, NCOL], dt.float32, kind="ExternalOutput").ap()

    class _PhaseDone(Exception):
        pass

    import contextlib
    try:
        _tc_cm = tile.TileContext(nc)
        tc = _tc_cm.__enter__()
        est = contextlib.ExitStack()
        with est:
            top = est.enter_context(tc.tile_pool(name="top", bufs=1))
            dram = est.enter_context(tc.tile_pool(name="dram", bufs=1, space="DRAM"))

            htable = dram.tile([NPAD, HROW], dt.float32, tag="htable")
            cc_in = dram.tile([4, RPAD], dt.float32, tag="cc_in")
            cc_out = dram.tile([CORES, 4, RPAD], dt.float32, tag="cc_out")

            ident = top.tile([128, 128], dt.float32, tag="ident")
            make_identity(nc, ident[:])
            iota_i = top.tile([128, 128], dt.int32, tag="iota_i")
            nc.gpsimd.iota(iota_i[:], pattern=[[1, 128]], base=0, channel_multiplier=0)
            iota_f = top.tile([128, 128], dt.float32, tag="iota_f")
            nc.vector.tensor_copy(out=iota_f[:], in_=iota_i[:])
            ones_row = top.tile([1, 128], dt.float32, tag="ones_row")
            nc.vector.memset(ones_row[:], 1.0)
            iotac_i = top.tile([128, 1], dt.int32, tag="iotac_i")
            nc.gpsimd.iota(iotac_i[:], pattern=[[1, 1]], base=0, channel_multiplier=1)
            iota_c = top.tile([128, 1], dt.float32, tag="iota_c")
            nc.vector.tensor_copy(out=iota_c[:], in_=iotac_i[:])
            eps_col = top.tile([128, 1], dt.float32, tag="eps_col")
            nc.vector.memset(eps_col[:], EPS)

            # ---- weight prep ----
            with tc.tile_pool(name="wprep", bufs=1) as wp, \
                 tc.tile_pool(name="wpsum", bufs=2, space="PSUM") as wps:
                wg0 = wp.tile([128, 256], dt.float32, tag="wg0")
                wg1 = wp.tile([128, 256], dt.float32, tag="wg1")
                nc.sync.dma_start(out=wg0[:], in_=wg[0:128, :])
                nc.sync.dma_start(out=wg1[:], in_=wg[128:256, :])
                # rhs chunks [128, 260]: cols 0:256 = WgT[k,:], cols 256:260 = [ws0,ws1,wd0,wd1]
                rhsA0 = top.tile([128, 260], dt.float32, tag="rhsA0")
                rhsA1 = top.tile([128, 260], dt.float32, tag="rhsA1")
                for (ko, rhs_t) in ((0, rhsA0), (1, rhsA1)):
                    for oo, wgt in ((0, wg0), (1, wg1)):
                        p = wps.tile([128, 128], dt.float32, tag="tp", space="PSUM")
                        nc.tensor.transpose(out=p[:], in_=wgt[:, ko * 128:(ko + 1) * 128],
                                            identity=ident[:])
                        nc.scalar.copy(out=rhs_t[:, oo * 128:(oo + 1) * 128], in_=p[:])
                # attention fold: wtil[c, k] = sum_j attdiag[j, c] * Wg[j, k]
                asb = wp.tile([2, 128], dt.float32, tag="asb")
                adb = wp.tile([2, 128], dt.float32, tag="adb")
                nc.sync.dma_start(out=asb[:], in_=att_src[:])
                nc.sync.dma_start(out=adb[:], in_=att_dst[:])
                asT = wps.tile([128, 2], dt.float32, tag="attp", space="PSUM")
                nc.tensor.transpose(out=asT[:], in_=asb[:], identity=ident[:2, :2])
                adT = wps.tile([128, 2], dt.float32, tag="attp", space="PSUM")
                nc.tensor.transpose(out=adT[:], in_=adb[:], identity=ident[:2, :2])
                attd0 = wp.tile([128, 4], dt.float32, tag="attd0")
                attd1 = wp.tile([128, 4], dt.float32, tag="attd1")
                nc.vector.memset(attd0[:], 0.0)
                nc.vector.memset(attd1[:], 0.0)
                nc.scalar.copy(out=attd0[:, 0:1], in_=asT[:, 0:1])
                nc.scalar.copy(out=attd0[:, 2:3], in_=adT[:, 0:1])
                nc.scalar.copy(out=attd1[:, 1:2], in_=asT[:, 1:2])
                nc.scalar.copy(out=attd1[:, 3:4], in_=adT[:, 1:2])
                wtilp = wps.tile([4, 256], dt.float32, tag="wtilp", space="PSUM")
                nc.tensor.matmul(wtilp[:], attd0[:], wg0[:], start=True, stop=False)
                nc.tensor.matmul(wtilp[:], attd1[:], wg1[:], start=False, stop=True)
                wtil = wp.tile([4, 256], dt.float32, tag="wtil")
                nc.scalar.copy(out=wtil[:], in_=wtilp[:])
                for ko, rhs_t in ((0, rhsA0), (1, rhsA1)):
                    p = wps.tile([128, 4], dt.float32, tag="attp", space="PSUM")
                    nc.tensor.transpose(out=p[:], in_=wtil[:, ko * 128:(ko + 1) * 128],
                                        identity=ident[:4, :4])
                    nc.scalar.copy(out=rhs_t[:, 256:260], in_=p[:])

                # dense weight transposes
                wa_sb = wp.tile([128, 256], dt.float32, tag="wa_sb")
                nc.sync.dma_start(out=wa_sb[:], in_=wa[:])
                waT0 = top.tile([128, 128], dt.float32, tag="waT0")
                waT1 = top.tile([128, 128], dt.float32, tag="waT1")
                for oo, dst_t in ((0, waT0), (1, waT1)):
                    p = wps.tile([128, 128], dt.float32, tag="tp", space="PSUM")
                    nc.tensor.transpose(out=p[:], in_=wa_sb[:, oo * 128:(oo + 1) * 128],
                                        identity=ident[:])
                    nc.scalar.copy(out=dst_t[:], in_=p[:])
                w1_sb = wp.tile([64, 128], dt.float32, tag="w1_sb")
                nc.sync.dma_start(out=w1_sb[:], in_=w1[:])
                w1T = top.tile([128, 64], dt.float32, tag="w1T")
                p = wps.tile([128, 128], dt.float32, tag="tp", space="PSUM")
                nc.tensor.transpose(out=p[:, 0:64], in_=w1_sb[:], identity=ident[:64, :64])
                nc.scalar.copy(out=w1T[:], in_=p[:, 0:64])
                w2_sb = wp.tile([32, 64], dt.float32, tag="w2_sb")
                nc.sync.dma_start(out=w2_sb[:], in_=w2[:])
                w2T = top.tile([64, 32], dt.float32, tag="w2T")
                p = wps.tile([128, 128], dt.float32, tag="tp", space="PSUM")
                nc.tensor.transpose(out=p[:64, 0:32], in_=w2_sb[:], identity=ident[:32, :32])
                nc.scalar.copy(out=w2T[:], in_=p[:64, 0:32])
                w3_sb = wp.tile([3, 32], dt.float32, tag="w3_sb")
                nc.sync.dma_start(out=w3_sb[:], in_=w3[:])
                w3T = top.tile([32, 3], dt.float32, tag="w3T")
                p = wps.tile([128, 128], dt.float32, tag="tp", space="PSUM")
                nc.tensor.transpose(out=p[:32, 0:3], in_=w3_sb[:], identity=ident[:3, :3])
                nc.scalar.copy(out=w3T[:], in_=p[:32, 0:3])

                # broadcast bias / LN tiles
                def bcast(vec_ap, n, tag):
                    t = top.tile([128, n], dt.float32, tag=tag)
                    v = wp.tile([1, n], dt.float32, tag="bvec_" + tag)
                    nc.sync.dma_start(out=v[:], in_=vec_ap)
                    p2 = wps.tile([128, 256], dt.float32, tag="bp", space="PSUM")
                    nc.tensor.matmul(p2[:, 0:n], ones_row[:], v[:], start=True, stop=True)
                    nc.scalar.copy(out=t[:], in_=p2[:, 0:n])
                    return t

                bgat_bc = bcast(bgat[:], 256, "bgat_bc")
                ba_bc = bcast(ba[:], 128, "ba_bc")
                lnaw_bc = bcast(lnaw[:], 128, "lnaw_bc")
                lnab_bc = bcast(lnab[:], 128, "lnab_bc")
                b1_bc = bcast(b1[:], 64, "b1_bc")
                ln1w_bc = bcast(ln1w[:], 64, "ln1w_bc")
                ln1b_bc = bcast(ln1b[:], 64, "ln1b_bc")
                b2_bc = bcast(b2[:], 32, "b2_bc")
                ln2w_bc = bcast(ln2w[:], 32, "ln2w_bc")
                ln2b_bc = bcast(ln2b[:], 32, "ln2b_bc")
                b3_bc = bcast(b3[:], 3, "b3_bc")

            # ---- phase A: htable/adtable = [h | 1 | h | 1 | a_s] per node ----
            with tc.tile_pool(name="pa", bufs=3) as pa, \
                 tc.tile_pool(name="pa_ht", bufs=3) as pa_ht, \
                 tc.tile_pool(name="pa_tp", bufs=4, space="PSUM") as pa_tp, \
                 tc.tile_pool(name="pa_h", bufs=2, space="PSUM") as pa_h:
                for i in range(NB):
                    r0 = i * 128
                    xt0 = pa.tile([128, 128], dt.float32, tag="xt0")
                    xt1 = pa.tile([128, 128], dt.float32, tag="xt1")
                    nc.sync.dma_start(out=xt0[:], in_=xt[0:128, r0:r0 + 128])
                    nc.sync.dma_start(out=xt1[:], in_=xt[128:256, r0:r0 + 128])
                    hp = pa_h.tile([128, 260], dt.float32, tag="hp", space="PSUM")
                    nc.tensor.matmul(hp[:], xt0[:], rhsA0[:], start=True, stop=False)
                    nc.tensor.matmul(hp[:], xt1[:], rhsA1[:], start=False, stop=True)
                    ht = pa_ht.tile([128, HROW], dt.float32, tag="ht")
                    nc.gpsimd.memset(ht[:, 262:HROW], 0.0)
                    nc.scalar.copy(out=ht[:, 0:128], in_=hp[:, 0:128])
                    nc.vector.memset(ht[:, 128:129], 1.0)
                    nc.vector.tensor_copy(out=ht[:, 129:257], in_=hp[:, 128:256])
                    nc.vector.memset(ht[:, 257:258], 1.0)
                    nc.scalar.copy(out=ht[:, 258:262], in_=hp[:, 256:260])
                    nc.scalar.dma_start(out=htable[r0:r0 + 128, :], in_=ht[:])

            if phases == "A":
                dbg = top.tile([128, HROW], dt.float32, tag="dbgA")
                for i in range(RB):
                    nc.sync.dma_start(out=dbg[:], in_=htable[i * 128:(i + 1) * 128, :])
                    nc.sync.dma_start(out=outD[i * 128:(i + 1) * 128, 0:HROW], in_=dbg[:])
                dbg2 = top.tile([128, ADROW], dt.float32, tag="dbgA2")
                for i in range(RB):
                    nc.sync.dma_start(out=dbg2[:], in_=adtable[i * 128:(i + 1) * 128, :])
                    nc.sync.dma_start(out=outD[i * 128:(i + 1) * 128, HROW:HROW + ADROW], in_=dbg2[:])
                raise _PhaseDone()

            # ---- phase B: GAT aggregation per dst block ----
            xg_pool = est.enter_context(tc.tile_pool(name="xg", bufs=1))
            xgs = []
            with tc.tile_pool(name="pb_idx", bufs=3) as pb_idx, \
                 tc.tile_pool(name="pb_g", bufs=4) as pb_g, \
                 tc.tile_pool(name="pb_ad", bufs=2) as pb_ad, \
                 tc.tile_pool(name="pb_ex", bufs=4) as pb_ex, \
                 tc.tile_pool(name="pb_oh", bufs=6) as pb_oh, \
                 tc.tile_pool(name="pb_ep", bufs=2) as pb_ep, \
                 tc.tile_pool(name="pb_ps", bufs=2, space="PSUM") as pb_ps, \
                 tc.tile_pool(name="pb_bc", bufs=2, space="PSUM") as pb_bc:
                for b in range(RB):
                    hix = pb_idx.tile([128, S16], dt.int16, tag="hix")
                    nc.sync.dma_start(out=hix[:], in_=hidx[b])
                    arw = pb_idx.tile([128, 8], dt.int16, tag="arw")
                    nc.sync.dma_start(out=arw[:], in_=adrows[b])
                    dloc = pb_idx.tile([128, TB], dt.float32, tag="dloc")
                    nc.sync.dma_start(out=dloc[:], in_=dstloc[b])
                    drow = pb_idx.tile([1, TB * 128], dt.float32, tag="drow")
                    nc.sync.dma_start(out=drow[:], in_=dstrow[b])

                    # a_d for this block's 128 dsts: [128, 1, 64]; cols 4:6 = a_d
                    adblk = pb_ad.tile([128, 1, ADROW], dt.float32, tag="adblk")
                    nc.gpsimd.dma_gather(
                        out_ap=adblk[:], in_ap=htable[:, 256:HROW],
                        idxs_ap=arw[:], num_idxs=128, num_idxs_reg=128,
                        elem_size=ADROW, elem_step=HROW)

                    ps0 = pb_ps.tile([128, 129], dt.float32, tag="ps0", space="PSUM")
                    ps1 = pb_ps.tile([128, 129], dt.float32, tag="ps1", space="PSUM")

                    for c in range(NCH):
                        g = pb_g.tile([128, GC, HROW], dt.float32, tag="g")
                        nc.gpsimd.dma_gather(
                            out_ap=g[:], in_ap=htable[:],
                            idxs_ap=hix[:, c * 8 * GC:(c + 1) * 8 * GC],
                            num_idxs=128 * GC, num_idxs_reg=128 * GC,
                            elem_size=HROW)
                        # per-edge a_d via transposed one-hot matmul
                        adE = pb_ex.tile([128, GC, 2], dt.float32, tag="adE")
                        for tl in range(GC):
                            t = c * GC + tl
                            bc = pb_bc.tile([128, 128], dt.float32, tag="bc",
                                            space="PSUM")
                            nc.tensor.matmul(bc[:], ones_row[:], drow[0:1, t * 128:(t + 1) * 128],
                                             start=True, stop=True)
                            ohT = pb_oh.tile([128, 128], dt.float32, tag="ohT")
                            nc.vector.tensor_scalar(
                                out=ohT[:], in0=bc[:], scalar1=iota_c[:, 0:1],
                                scalar2=None, op0=OP.is_equal)
                            adp = pb_bc.tile([128, 2], dt.float32, tag="adp",
                                             space="PSUM")
                            nc.tensor.matmul(adp[:], ohT[:], adblk[:, 0, 4:6],
                                             start=True, stop=True)
                            nc.scalar.copy(out=adE[:, tl, :], in_=adp[:])
                        # EX = exp(lrelu02(a_s[src] + a_d[dst])) = max(exp(e), exp(0.2e))
                        exr = pb_ex.tile([128, GC, 2], dt.float32, tag="exr")
                        nc.vector.tensor_tensor(
                            out=exr[:], in0=g[:, :, 258:260], in1=adE[:], op=OP.add)
                        exn = pb_ex.tile([128, GC, 2], dt.float32, tag="exn")
                        nc.scalar.activation(out=exn[:], in_=exr[:], func=AF.Exp,
                                             scale=0.2)
                        exp_ = pb_ex.tile([128, GC, 2], dt.float32, tag="exp_")
                        nc.scalar.activation(out=exp_[:], in_=exr[:], func=AF.Exp)
                        exv = pb_ex.tile([128, GC, 2], dt.float32, tag="exv")
                        nc.vector.tensor_tensor(out=exv[:], in0=exp_[:], in1=exn[:],
                                                op=OP.max)

                        for tl in range(GC):
                            t = c * GC + tl
                            first = (t == 0)
                            last = (t == TB - 1)
                            oh0 = pb_oh.tile([128, 128], dt.float32, tag="oh0")
                            nc.vector.tensor_scalar(
                                out=oh0[:], in0=iota_f[:], scalar1=dloc[:, t:t + 1],
                                scalar2=exv[:, tl, 0:1], op0=OP.is_equal, op1=OP.mult)
                            nc.tensor.matmul(ps0[:], oh0[:], g[:, tl, 0:129],
                                             start=first, stop=last)
                            oh1 = pb_oh.tile([128, 128], dt.float32, tag="oh1")
                            nc.gpsimd.tensor_scalar(
                                out=oh1[:], in0=iota_f[:], scalar1=dloc[:, t:t + 1],
                                scalar2=exv[:, tl, 1:2], op0=OP.is_equal, op1=OP.mult)
                            nc.tensor.matmul(ps1[:], oh1[:], g[:, tl, 129:258],
                                             start=first, stop=last)

                    # epilogue: normalize, +b_gat, lrelu(0.01)
                    rec0 = pb_ep.tile([128, 1], dt.float32, tag="rec0")
                    nc.vector.reciprocal(rec0[:], ps0[:, 128:129])
                    rec1 = pb_ep.tile([128, 1], dt.float32, tag="rec1")
                    nc.vector.reciprocal(rec1[:], ps1[:, 128:129])
                    xg = xg_pool.tile([128, 256], dt.float32, tag=f"xg{b}")
                    nc.scalar.activation(out=xg[:, 0:128], in_=ps0[:, 0:128],
                                         func=AF.Copy, scale=rec0[:])
                    nc.scalar.activation(out=xg[:, 128:256], in_=ps1[:, 0:128],
                                         func=AF.Copy, scale=rec1[:])
                    nc.vector.tensor_tensor(out=xg[:], in0=xg[:], in1=bgat_bc[:], op=OP.add)
                    ng = pb_ep.tile([128, 256], dt.float32, tag="ng")
                    nc.vector.tensor_scalar(out=ng[:], in0=xg[:], scalar1=0.0,
                                            scalar2=0.01, op0=OP.min, op1=OP.mult)
                    nc.vector.scalar_tensor_tensor(out=xg[:], in0=xg[:], scalar=0.0,
                                                   in1=ng[:], op0=OP.max, op1=OP.add)
                    xgs.append(xg)

            if phases == "AB":
                for b in range(RB):
                    nc.sync.dma_start(out=outD[b * 128:(b + 1) * 128, 0:256], in_=xgs[b][:])
                raise _PhaseDone()

            # ---- phase C: dense + LN on own shard; build cc_in and cdist lhsT ----
            cc_sb = top.tile([4, RPAD], dt.float32, tag="cc_sb")
            lhsT_pool = est.enter_context(tc.tile_pool(name="lhsT", bufs=1))
            lhsTs = []
            with tc.tile_pool(name="pc", bufs=3) as pc, \
                 tc.tile_pool(name="pc_ps", bufs=2, space="PSUM") as pc_ps:

                def layer_norm_lrelu(xin, fdim, bias_bc, w_bc, b_bc):
                    # y = xin + bias; u = LN(y)*w + b; return lrelu001(u)
                    y = pc.tile([128, fdim], dt.float32, tag=f"y{fdim}")
                    nc.vector.tensor_tensor(out=y[:], in0=xin, in1=bias_bc[:], op=OP.add)
                    scr = pc.tile([128, fdim], dt.float32, tag=f"scr{fdim}")
                    msum = pc.tile([128, 1], dt.float32, tag="msum")
                    nc.scalar.activation(out=scr[:], in_=y[:], func=AF.Copy,
                                         accum_out=msum[:])
                    sqs = pc.tile([128, 1], dt.float32, tag="sqs")
                    nc.scalar.activation(out=scr[:], in_=y[:], func=AF.Square,
                                         accum_out=sqs[:])
                    mean = pc.tile([128, 1], dt.float32, tag="mean")
                    nc.vector.tensor_scalar(out=mean[:], in0=msum[:], scalar1=1.0 / fdim,
                                            scalar2=None, op0=OP.mult)
                    var = pc.tile([128, 1], dt.float32, tag="var")
                    # var = sqs/f - mean^2
                    nc.vector.tensor_scalar(out=var[:], in0=sqs[:], scalar1=1.0 / fdim,
                                            scalar2=None, op0=OP.mult)
                    m2 = pc.tile([128, 1], dt.float32, tag="m2")
                    nc.vector.tensor_scalar(out=m2[:], in0=mean[:], scalar1=mean[:, 0:1],
                                            scalar2=None, op0=OP.mult)
                    nc.vector.tensor_tensor(out=var[:], in0=var[:], in1=m2[:], op=OP.subtract)
                    sd = pc.tile([128, 1], dt.float32, tag="sd")
                    nc.scalar.activation(out=sd[:], in_=var[:], func=AF.Sqrt, bias=eps_col[:, 0:1])
                    rstd = pc.tile([128, 1], dt.float32, tag="rstd")
                    nc.vector.reciprocal(rstd[:], sd[:])
                    u = pc.tile([128, fdim], dt.float32, tag=f"u{fdim}")
                    nc.vector.scalar_tensor_tensor(out=u[:], in0=y[:], scalar=mean[:, 0:1],
                                                   in1=w_bc[:], op0=OP.subtract, op1=OP.mult)
                    nc.vector.scalar_tensor_tensor(out=u[:], in0=u[:], scalar=rstd[:, 0:1],
                                                   in1=b_bc[:], op0=OP.mult, op1=OP.add)
                    ngt = pc.tile([128, fdim], dt.float32, tag=f"ng{fdim}")
                    nc.vector.tensor_scalar(out=ngt[:], in0=u[:], scalar1=0.0,
                                            scalar2=0.01, op0=OP.min, op1=OP.mult)
                    nc.vector.scalar_tensor_tensor(out=u[:], in0=u[:], scalar=0.0,
                                                   in1=ngt[:], op0=OP.max, op1=OP.add)
                    return u

                def transpose_to(xin, pdim, fdim):
                    # xin [pdim, fdim] -> sbuf [fdim, pdim]
                    p = pc_ps.tile([128, 128], dt.float32, tag="tpp", space="PSUM")
                    nc.tensor.transpose(out=p[:fdim, 0:pdim], in_=xin,
                                        identity=ident[:pdim, :pdim])
                    s = pc.tile([fdim, pdim], dt.float32, tag=f"tt{fdim}_{pdim}")
                    nc.scalar.copy(out=s[:], in_=p[:fdim, 0:pdim])
                    return s

                for b in range(RB):
                    x0 = xgs[b]
                    xt0 = transpose_to(x0[:, 0:128], 128, 128)
                    xt1 = transpose_to(x0[:, 128:256], 128, 128)
                    pA = pc_ps.tile([128, 128], dt.float32, tag="mm", space="PSUM")
                    nc.tensor.matmul(pA[:], xt0[:], waT0[:], start=True, stop=False)
                    nc.tensor.matmul(pA[:], xt1[:], waT1[:], start=False, stop=True)
                    x1 = layer_norm_lrelu(pA[:], 128, ba_bc, lnaw_bc, lnab_bc)

                    x1t = transpose_to(x1[:], 128, 128)
                    p1 = pc_ps.tile([128, 64], dt.float32, tag="mm", space="PSUM")
                    nc.tensor.matmul(p1[:], x1t[:], w1T[:], start=True, stop=True)
                    x2 = layer_norm_lrelu(p1[:], 64, b1_bc, ln1w_bc, ln1b_bc)

                    x2t = transpose_to(x2[:], 128, 64)
                    p2 = pc_ps.tile([128, 32], dt.float32, tag="mm", space="PSUM")
                    nc.tensor.matmul(p2[:], x2t[:], w2T[:], start=True, stop=True)
                    x3 = layer_norm_lrelu(p2[:], 32, b2_bc, ln2w_bc, ln2b_bc)

                    x3t = transpose_to(x3[:], 128, 32)
                    p3 = pc_ps.tile([128, 3], dt.float32, tag="mm", space="PSUM")
                    nc.tensor.matmul(p3[:], x3t[:], w3T[:], start=True, stop=True)
                    y3 = pc.tile([128, 4], dt.float32, tag="y3")
                    nc.vector.tensor_tensor(out=y3[:, 0:3], in0=p3[:], in1=b3_bc[:], op=OP.add)
                    scr3 = pc.tile([128, 3], dt.float32, tag="scr3")
                    nc.scalar.activation(out=scr3[:], in_=y3[:, 0:3], func=AF.Square,
                                         accum_out=y3[:, 3:4])
                    # y3e = [-2*y3 | sq | 1 | 0...] then transpose -> lhsT rows
                    y3e = pc.tile([128, 8], dt.float32, tag="y3e")
                    nc.scalar.activation(out=y3e[:, 0:3], in_=y3[:, 0:3],
                                         func=AF.Copy, scale=-2.0)
                    nc.scalar.copy(out=y3e[:, 3:4], in_=y3[:, 3:4])
                    nc.vector.memset(y3e[:, 4:5], 1.0)
                    nc.vector.memset(y3e[:, 5:8], 0.0)
                    # transpose [128,4] -> [4,128] for cc, and [128,8] -> [8,128] for lhsT
                    h3p = pc_ps.tile([128, 128], dt.float32, tag="tpp", space="PSUM")
                    nc.tensor.transpose(out=h3p[:4, 0:128], in_=y3[:], identity=ident[:])
                    nc.scalar.copy(out=cc_sb[:, b * 128:(b + 1) * 128], in_=h3p[:4, 0:128])
                    h3q = pc_ps.tile([128, 128], dt.float32, tag="tpp", space="PSUM")
                    nc.tensor.transpose(out=h3q[:8, 0:128], in_=y3e[:], identity=ident[:])
                    lt = lhsT_pool.tile([5, 128], dt.float32, tag=f"lt{b}")
                    nc.scalar.copy(out=lt[:], in_=h3q[:5, 0:128])
                    lhsTs.append(lt)

            if phases == "ABC":
                nc.sync.dma_start(out=outD[0:4, 0:RPAD], in_=cc_sb[:])
                for b in range(RB):
                    nc.sync.dma_start(out=outD[8 + b, 0:128], in_=lhsTs[b][0:1,:].rearrange("a b -> (a b)")[None, :])
                raise _PhaseDone()

            # ---- phase D: allgather h3/sq ----
            nc.sync.dma_start(out=cc_in[:], in_=cc_sb[:])
            nc.gpsimd.collective_compute(
                "AllGather", OP.bypass, replica_groups=[list(range(CORES))],
                ins=[cc_in[:].opt()], outs=[cc_out[:].opt()])
            rhs_all = top.tile([5, NCOL], dt.float32, tag="rhs_all")
            nc.vector.memset(rhs_all[:], 0.0)
            ones_ncol = top.tile([1, NCOL], dt.float32, tag="ones_ncol")
            nc.vector.memset(ones_ncol[:], 1.0)
            nc.sync.dma_start(out=rhs_all[3:4, :], in_=ones_ncol[:])
            for s in range(CORES):
                c0 = s * SHARD
                nc.sync.dma_start(out=rhs_all[0:3, c0:c0 + SHARD],
                                  in_=cc_out[:][s, 0:3, 0:SHARD])
                nc.sync.dma_start(out=rhs_all[4:5, c0:c0 + SHARD],
                                  in_=cc_out[:][s, 3:4, 0:SHARD])

            if phases == "ABCD":
                nc.sync.dma_start(out=outD[0:5, 0:NCOL], in_=rhs_all[:])
                raise _PhaseDone()

            # ---- phase E: cdist row-block x col-chunk ----
            MRG = 4   # psum chunks merged into one output tile/DMA
            with tc.tile_pool(name="pe_d", bufs=8) as pe_d, \
                 tc.tile_pool(name="pe_d2", bufs=3) as pe_d2, \
                 tc.tile_pool(name="pe_ps", bufs=6, space="PSUM") as pe_ps:
                for rb in range(RB):
                    for mg in range(NCOL // (CCH * MRG)):
                        d2t = pe_d2.tile([128, CCH * MRG], dt.float32, tag="d2t")
                        for k in range(MRG):
                            ch = mg * MRG + k
                            dp = pe_ps.tile([128, CCH], dt.float32, tag="dp", space="PSUM")
                            nc.tensor.matmul(dp[:], lhsTs[rb][:],
                                             rhs_all[:, ch * CCH:(ch + 1) * CCH],
                                             start=True, stop=True)
                            dtl = pe_d.tile([128, CCH], dt.float32, tag="dtl")
                            nc.vector.tensor_scalar(out=dtl[:], in0=dp[:], scalar1=0.0,
                                                    scalar2=None, op0=OP.max)
                            nc.scalar.activation(out=d2t[:, k * CCH:(k + 1) * CCH],
                                                 in_=dtl[:], func=AF.Sqrt)
                        nc.sync.dma_start(
                            out=outD[rb * 128:(rb + 1) * 128,
                                     mg * CCH * MRG:(mg + 1) * CCH * MRG],
                            in_=d2t[:])

    except _PhaseDone:
        pass
    _tc_cm.__exit__(None, None, None)
    nc.compile()
    return nc


def _prep_host(x, edge_index):
    global GC
    xp = np.zeros((NPAD, FIN), np.float32)
    xp[:N] = np.asarray(x, np.float32)
    xp = np.ascontiguousarray(xp.T)  # [256, NPAD]

    ei = np.asarray(edge_index)
    src = np.concatenate([ei[0], np.arange(N, dtype=np.int64)]).astype(np.int64)
    dst = np.concatenate([ei[1], np.arange(N, dtype=np.int64)]).astype(np.int64)

    core = dst // SHARD
    per_core = []
    max_tiles = 0
    for c in range(CORES):
        sel = core == c
        s_c = src[sel]
        d_c = dst[sel]
        loc = d_c - c * SHARD
        blk = loc // 128
        dl = loc - blk * 128
        blocks = []
        for b in range(RB):
            m = blk == b
            blocks.append((s_c[m], d_c[m], dl[m]))
            max_tiles = max(max_tiles, (len(blocks[-1][0]) + 127) // 128)
        per_core.append(blocks)

    TB = GC * ((max_tiles + GC - 1) // GC)
    S16 = 8 * TB
    E_pad = 128 * TB

    hidx = np.zeros((CORES, RB, 16, S16), np.int16)
    adrw = np.zeros((CORES, RB, 16, 8), np.int16)
    dstl = np.full((CORES, RB, 128, TB), 255.0, np.float32)
    for c in range(CORES):
        for b in range(RB):
            s_b, d_b, dl_b = per_core[c][b]
            n = len(s_b)
            js = np.arange(n)
            hidx[c, b, js % 16, js // 16] = s_b.astype(np.int16)
            dstl[c, b, js % 128, js // 128] = dl_b.astype(np.float32)
            # block's 128 dst rows (clamped pad)
            rows = np.minimum(c * SHARD + b * 128 + np.arange(128), N - 1)
            jr = np.arange(128)
            adrw[c, b, jr % 16, jr // 16] = rows.astype(np.int16)
    hidx = np.tile(hidx, (1, 1, 8, 1))
    adrw = np.tile(adrw, (1, 1, 8, 1))
    dstrow = np.ascontiguousarray(dstl.transpose(0, 1, 3, 2)).reshape(CORES, RB, 1, TB * 128)
    return xp, hidx, adrw, dstl, dstrow, TB


def build_in_maps(inputs):
    xp, hidx, adrw, dstl, dstrow, TB = _prep_host(inputs["x"], inputs["edge_index"])
    f32 = lambda a: np.ascontiguousarray(np.asarray(a, np.float32))
    row = lambda a: f32(a).reshape(1, -1)
    shared = {
        "xt": xp, "wg": f32(inputs["W_gat"]),
        "att_src": f32(inputs["att_src"]), "att_dst": f32(inputs["att_dst"]),
        "bgat": row(inputs["b_gat"]), "wa": f32(inputs["Wa"]), "ba": row(inputs["ba"]),
        "lnaw": row(inputs["lna_w"]), "lnab": row(inputs["lna_b"]),
        "w1": f32(inputs["W1"]), "b1": row(inputs["b1"]),
        "ln1w": row(inputs["ln1_w"]), "ln1b": row(inputs["ln1_b"]),
        "w2": f32(inputs["W2"]), "b2": row(inputs["b2"]),
        "ln2w": row(inputs["ln2_w"]), "ln2b": row(inputs["ln2_b"]),
        "w3": f32(inputs["W3"]), "b3": row(inputs["b3"]),
    }
    in_maps = [
        {**shared, "hidx": np.ascontiguousarray(hidx[c]),
         "adrows": np.ascontiguousarray(adrw[c]),
         "dstloc": np.ascontiguousarray(dstl[c]),
         "dstrow": np.ascontiguousarray(dstrow[c])}
        for c in range(CORES)
    ]
    return in_maps, TB


def kernel(**inputs):
    in_maps, TB = build_in_maps(inputs)

    import os
    phases = os.environ.get("K_PHASES", "ABCDE")
    key = (TB, phases)
    if key not in _BUILD_CACHE:
        _BUILD_CACHE[key] = _build(TB, phases)
    nc = _BUILD_CACHE[key]
    res = run_bass_kernel_spmd(nc, in_maps, core_ids=list(range(CORES)))
    global _LAST_RESULTS
    _LAST_RESULTS = res.results
    out = np.empty((N, N), np.float32)
    for c in range(CORES):
        out[c * SHARD:(c + 1) * SHARD, :] = res.results[c]["outD"][:SHARD, :N]
    return out



# revision 9
# speedup vs baseline: 1.6021x; 1.6021x over previous
"""GAT (2-head) + 3x dense/LayerNorm + pairwise-distance kernel for 8 TRN2 NeuronCores.

Strategy: dst-sharded edge processing (one-hot matmul aggregation), replicated
small dense weights, row-block-sharded NxN cdist output.

v2: fp16 htable/gather rows, single shared one-hot per edge tile (fp16 matmul),
batched a_d fetch, host-side weight prep, stage-parallel dense chain, fp32r
cdist matmuls, fp16 output.
"""
import sys

import numpy as np

# Environment bootstrap (harness may run from a bare directory).
for _p in ("/root/.axon_site", "/root/.axon_site/_ro/trn_rl_repo",
           "/root/.axon_site/_ro/pypackages", "/opt/trn_rl_repo"):
    if _p not in sys.path:
        sys.path.append(_p)

import concourse.bass as bass
import concourse.bacc as bacc
import concourse.mybir as mybir
import concourse.tile as tile
from concourse.masks import make_identity
from concourse.bass_utils import run_bass_kernel_spmd

dt = mybir.dt
OP = mybir.AluOpType
AF = mybir.ActivationFunctionType

N = 10000
NPAD = 10112          # 79 * 128
NB = 79               # node blocks (phase A)
FIN = 256
F = 128               # per-head GAT dim
H = 2
HROW = 384            # htable row fp16 elems (768B, multiple of 256B)
CORES = 8
SHARD = 1250          # dst nodes per core
RB = 10               # dst blocks per core
RPAD = 1280
CCH = 512             # cdist column chunk
NCOL = 10240          # padded output columns
EPS = 1e-5

_BUILD_CACHE = {}
_LAST_RESULTS = None


GC = 6   # tile groups per dma_gather call (768 idxs = 48 descs/engine)
SUB = 3  # tiles per a_d subchunk (384 psum cols <= 512)


def _build(TB, phases="ABCDE"):
    """Build the 8-core SPMD program. TB = gather tile groups per dst block (mult of GC)."""
    assert TB % GC == 0
    NCH = TB // GC
    S16 = 8 * TB          # idx columns ([128, S16] wrapped int16)

    nc = bacc.Bacc("TRN2", target_bir_lowering=False, debug=False, num_devices=CORES)

    def din(name, shape, d=dt.float32):
        return nc.dram_tensor(name, shape, d, kind="ExternalInput").ap()

    xt16 = din("xt16", [FIN, NPAD], dt.float16)
    rhsA = din("rhsA", [2, 128, 262], dt.float16)
    waT = din("waT", [2, 128, 128], dt.float16)
    w1T = din("w1T", [128, 64], dt.float16)
    w2T = din("w2T", [64, 32], dt.float16)
    w3T = din("w3T", [32, 3], dt.float16)
    bgat_bc = din("bgat_bc", [128, 256], dt.float16)
    ba_bc = din("ba_bc", [128, 128])
    lnaw_bc = din("lnaw_bc", [128, 128])
    lnab_bc = din("lnab_bc", [128, 128])
    b1_bc = din("b1_bc", [128, 64])
    ln1w_bc = din("ln1w_bc", [128, 64])
    ln1b_bc = din("ln1b_bc", [128, 64])
    b2_bc = din("b2_bc", [128, 32])
    ln2w_bc = din("ln2w_bc", [128, 32])
    ln2b_bc = din("ln2b_bc", [128, 32])
    b3_bc = din("b3_bc", [128, 3])
    hidx = din("hidx", [RB, 128, S16], dt.int16)
    adrows = din("adrows", [RB, 128, 8], dt.int16)
    dstloc = din("dstloc", [RB, 128, TB])
    dstrow = din("dstrow", [RB, 1, TB * 128], dt.float16)
    outD = nc.dram_tensor("outD", [RPAD, NCOL], dt.float16, kind="ExternalOutput").ap()

    class _PhaseDone(Exception):
        pass

    import contextlib
    try:
        _tc_cm = tile.TileContext(nc)
        tc = _tc_cm.__enter__()
        est = contextlib.ExitStack()
        with est:
            top = est.enter_context(tc.tile_pool(name="top", bufs=1))
            dram = est.enter_context(tc.tile_pool(name="dram", bufs=1, space="DRAM"))

            htable = dram.tile([NPAD, HROW], dt.float16, tag="htable")
            cc_in = dram.tile([4, RPAD], dt.float32, tag="cc_in")
            cc_out = dram.tile([CORES, 4, RPAD], dt.float32, tag="cc_out")

            ident = top.tile([128, 128], dt.float32, tag="ident")
            make_identity(nc, ident[:])
            ident16 = top.tile([128, 128], dt.float16, tag="ident16")
            nc.vector.tensor_copy(out=ident16[:], in_=ident[:])
            iota_i = top.tile([128, 128], dt.int32, tag="iota_i")
            nc.gpsimd.iota(iota_i[:], pattern=[[1, 128]], base=0, channel_multiplier=0)
            iota_f = top.tile([128, 128], dt.float32, tag="iota_f")
            nc.vector.tensor_copy(out=iota_f[:], in_=iota_i[:])
            ones16 = top.tile([1, 128], dt.float16, tag="ones16")
            nc.vector.memset(ones16[:], 1.0)
            iotac_i = top.tile([128, 1], dt.int32, tag="iotac_i")
            nc.gpsimd.iota(iotac_i[:], pattern=[[1, 1]], base=0, channel_multiplier=1)
            iota_c = top.tile([128, 1], dt.float32, tag="iota_c")
            nc.vector.tensor_copy(out=iota_c[:], in_=iotac_i[:])
            eps_col = top.tile([128, 1], dt.float32, tag="eps_col")
            nc.vector.memset(eps_col[:], EPS)

            # ---- load replicated weights / biases into SBUF ----
            def ldw(name, ap, shape, d=dt.float32):
                t = top.tile(shape, d, tag=name)
                nc.sync.dma_start(out=t[:], in_=ap)
                return t

            rhsA0 = ldw("rhsA0", rhsA[0], [128, 262], dt.float16)
            rhsA1 = ldw("rhsA1", rhsA[1], [128, 262], dt.float16)
            waT0 = ldw("waT0", waT[0], [128, 128], dt.float16)
            waT1 = ldw("waT1", waT[1], [128, 128], dt.float16)
            w1T_sb = ldw("w1T_sb", w1T[:], [128, 64], dt.float16)
            w2T_sb = ldw("w2T_sb", w2T[:], [64, 32], dt.float16)
            w3T_sb = ldw("w3T_sb", w3T[:], [32, 3], dt.float16)
            bgat_sb = ldw("bgat_sb", bgat_bc[:], [128, 256], dt.float16)
            ba_sb = ldw("ba_sb", ba_bc[:], [128, 128])
            lnaw_sb = ldw("lnaw_sb", lnaw_bc[:], [128, 128])
            lnab_sb = ldw("lnab_sb", lnab_bc[:], [128, 128])
            b1_sb = ldw("b1_sb", b1_bc[:], [128, 64])
            ln1w_sb = ldw("ln1w_sb", ln1w_bc[:], [128, 64])
            ln1b_sb = ldw("ln1b_sb", ln1b_bc[:], [128, 64])
            b2_sb = ldw("b2_sb", b2_bc[:], [128, 32])
            ln2w_sb = ldw("ln2w_sb", ln2w_bc[:], [128, 32])
            ln2b_sb = ldw("ln2b_sb", ln2b_bc[:], [128, 32])
            b3_sb = ldw("b3_sb", b3_bc[:], [128, 3])

            # ---- phase A: htable rows [h0 | 1 | h1 | 1 | a_s(2) a_d(2)] fp16 ----
            with tc.tile_pool(name="pa", bufs=3) as pa, \
                 tc.tile_pool(name="pa_ht", bufs=3) as pa_ht, \
                 tc.tile_pool(name="pa_h", bufs=3, space="PSUM") as pa_h:
                for i in range(NB):
                    r0 = i * 128
                    xt0 = pa.tile([128, 128], dt.float16, tag="xt0")
                    xt1 = pa.tile([128, 128], dt.float16, tag="xt1")
                    nc.sync.dma_start(out=xt0[:], in_=xt16[0:128, r0:r0 + 128])
                    nc.sync.dma_start(out=xt1[:], in_=xt16[128:256, r0:r0 + 128])
                    hp = pa_h.tile([128, 262], dt.float32, tag="hp", space="PSUM")
                    nc.tensor.matmul(hp[:], xt0[:], rhsA0[:], start=True, stop=False)
                    nc.tensor.matmul(hp[:], xt1[:], rhsA1[:], start=False, stop=True)
                    ht = pa_ht.tile([128, 262], dt.float16, tag="ht")
                    nc.scalar.copy(out=ht[:], in_=hp[:])
                    nc.vector.memset(ht[:, 128:129], 1.0)
                    nc.vector.memset(ht[:, 257:258], 1.0)
                    nc.scalar.dma_start(out=htable[r0:r0 + 128, 0:262], in_=ht[:])

            if phases == "A":
                dbg = top.tile([128, 262], dt.float16, tag="dbgA")
                for i in range(RB):
                    nc.sync.dma_start(out=dbg[:], in_=htable[i * 128:(i + 1) * 128, 0:262])
                    nc.sync.dma_start(out=outD[i * 128:(i + 1) * 128, 0:262], in_=dbg[:])
                raise _PhaseDone()

            # ---- phase B: GAT aggregation per dst block ----
            xg_pool = est.enter_context(tc.tile_pool(name="xg", bufs=1))
            xgs = []
            with tc.tile_pool(name="pb_idx", bufs=3) as pb_idx, \
                 tc.tile_pool(name="pb_g", bufs=4) as pb_g, \
                 tc.tile_pool(name="pb_ad", bufs=2) as pb_ad, \
                 tc.tile_pool(name="pb_ex", bufs=4) as pb_ex, \
                 tc.tile_pool(name="pb_oh", bufs=6) as pb_oh, \
                 tc.tile_pool(name="pb_rhs", bufs=6) as pb_rhs, \
                 tc.tile_pool(name="pb_ep", bufs=2) as pb_ep, \
                 tc.tile_pool(name="pb_ps", bufs=2, space="PSUM") as pb_ps, \
                 tc.tile_pool(name="pb_bc", bufs=2, space="PSUM") as pb_bc, \
                 tc.tile_pool(name="pb_adp", bufs=2, space="PSUM") as pb_adp:
                for b in range(RB):
                    hix = pb_idx.tile([128, S16], dt.int16, tag="hix")
                    nc.sync.dma_start(out=hix[:], in_=hidx[b])
                    arw = pb_idx.tile([128, 8], dt.int16, tag="arw")
                    nc.sync.dma_start(out=arw[:], in_=adrows[b])
                    dloc = pb_idx.tile([128, TB], dt.float32, tag="dloc")
                    nc.sync.dma_start(out=dloc[:], in_=dstloc[b])
                    drow = pb_idx.tile([1, TB * 128], dt.float16, tag="drow")
                    nc.sync.dma_start(out=drow[:], in_=dstrow[b])

                    # a_d for this block's 128 dsts
                    adrow_g = pb_ad.tile([128, 1, HROW], dt.float16, tag="adrow_g")
                    nc.gpsimd.dma_gather(
                        out_ap=adrow_g[:], in_ap=htable[:],
                        idxs_ap=arw[:], num_idxs=128, num_idxs_reg=128,
                        elem_size=HROW)
                    adblk = pb_ad.tile([128, 2], dt.float16, tag="adblk")
                    nc.vector.tensor_copy(out=adblk[:], in_=adrow_g[:, 0, 260:262])

                    ps = pb_ps.tile([128, 258], dt.float32, tag="ps", space="PSUM")

                    for c in range(NCH):
                        g = pb_g.tile([128, GC, HROW], dt.float16, tag="g")
                        nc.gpsimd.dma_gather(
                            out_ap=g[:], in_ap=htable[:],
                            idxs_ap=hix[:, c * 8 * GC:(c + 1) * 8 * GC],
                            num_idxs=128 * GC, num_idxs_reg=128 * GC,
                            elem_size=HROW)
                        for hs in range(GC // SUB):
                            t0 = c * GC + hs * SUB
                            # broadcast dst-slot row to all partitions
                            bcp = pb_bc.tile([128, SUB * 128], dt.float32, tag="bcp",
                                             space="PSUM")
                            nc.tensor.matmul(bcp[:], ones16[:],
                                             drow[0:1, t0 * 128:(t0 + SUB) * 128],
                                             start=True, stop=True)
                            ohT = pb_oh.tile([128, SUB * 128], dt.float16, tag="ohT")
                            nc.vector.tensor_scalar(
                                out=ohT[:], in0=bcp[:], scalar1=iota_c[:, 0:1],
                                scalar2=None, op0=OP.is_equal)
                            adps = pb_adp.tile([128, SUB, 2], dt.float32, tag="adps",
                                               space="PSUM")
                            for k in range(SUB):
                                nc.tensor.matmul(adps[:, k, :],
                                                 ohT[:, k * 128:(k + 1) * 128],
                                                 adblk[:], start=True, stop=True)
                            # e = a_s[src] + a_d[dst]; exv = max(exp(e), exp(0.2e))
                            exr = pb_ex.tile([128, SUB, 2], dt.float32, tag="exr")
                            nc.vector.tensor_tensor(
                                out=exr[:], in0=adps[:],
                                in1=g[:, hs * SUB:(hs + 1) * SUB, 258:260], op=OP.add)
                            exn = pb_ex.tile([128, SUB, 2], dt.float32, tag="exn")
                            nc.scalar.activation(out=exn[:], in_=exr[:], func=AF.Exp,
                                                 scale=0.2)
                            exp_ = pb_ex.tile([128, SUB, 2], dt.float32, tag="exp_")
                            nc.scalar.activation(out=exp_[:], in_=exr[:], func=AF.Exp)
                            exv = pb_ex.tile([128, SUB, 2], dt.float32, tag="exv")
                            nc.vector.tensor_tensor(out=exv[:], in0=exp_[:], in1=exn[:],
                                                    op=OP.max)
                            for k in range(SUB):
                                t = t0 + k
                                tl = hs * SUB + k
                                oh = pb_oh.tile([128, 128], dt.float16, tag="oh")
                                nc.vector.tensor_scalar(
                                    out=oh[:], in0=iota_f[:], scalar1=dloc[:, t:t + 1],
                                    scalar2=None, op0=OP.is_equal)
                                rhs = pb_rhs.tile([128, 258], dt.float16, tag="rhs")
                                nc.scalar.activation(out=rhs[:, 0:129],
                                                     in_=g[:, tl, 0:129],
                                                     func=AF.Copy,
                                                     scale=exv[:, k, 0:1])
                                nc.scalar.activation(out=rhs[:, 129:258],
                                                     in_=g[:, tl, 129:258],
                                                     func=AF.Copy,
                                                     scale=exv[:, k, 1:2])
                                nc.tensor.matmul(ps[:], oh[:], rhs[:],
                                                 start=(t == 0), stop=(t == TB - 1))

                    # epilogue: normalize, +b_gat, lrelu(0.01) -> fp16 xg
                    rec0 = pb_ep.tile([128, 1], dt.float32, tag="rec0")
                    nc.vector.reciprocal(rec0[:], ps[:, 128:129])
                    rec1 = pb_ep.tile([128, 1], dt.float32, tag="rec1")
                    nc.vector.reciprocal(rec1[:], ps[:, 257:258])
                    xg = xg_pool.tile([128, 256], dt.float16, tag=f"xg{b}")
                    nc.scalar.activation(out=xg[:, 0:128], in_=ps[:, 0:128],
                                         func=AF.Copy, scale=rec0[:])
                    nc.scalar.activation(out=xg[:, 128:256], in_=ps[:, 129:257],
                                         func=AF.Copy, scale=rec1[:])
                    nc.vector.tensor_tensor(out=xg[:], in0=xg[:], in1=bgat_sb[:], op=OP.add)
                    ng = pb_ep.tile([128, 256], dt.float16, tag="ng")
                    nc.vector.tensor_scalar(out=ng[:], in0=xg[:], scalar1=0.0,
                                            scalar2=0.01, op0=OP.min, op1=OP.mult)
                    nc.vector.scalar_tensor_tensor(out=xg[:], in0=xg[:], scalar=0.0,
                                                   in1=ng[:], op0=OP.max, op1=OP.add)
                    xgs.append(xg)

            if phases == "AB":
                for b in range(RB):
                    nc.sync.dma_start(out=outD[b * 128:(b + 1) * 128, 0:256], in_=xgs[b][:])
                raise _PhaseDone()

            # ---- phase C: dense + LN on own shard (stage-parallel across blocks) ----
            cc_sb = top.tile([4, RPAD], dt.float32, tag="cc_sb")
            lhsT_pool = est.enter_context(tc.tile_pool(name="lhsT", bufs=1))
            lhsTs = [None] * RB
            with tc.tile_pool(name="pc", bufs=12) as pc, \
                 tc.tile_pool(name="pc_ps", bufs=2, space="PSUM") as pc_ps, \
                 tc.tile_pool(name="pc_mm", bufs=4, space="PSUM") as pc_mm:

                def transpose16(xin, pdim, fdim):
                    # xin fp16 [pdim, fdim] -> sbuf fp16 [fdim, pdim]
                    p = pc_ps.tile([128, 128], dt.float16, tag="tpp16", space="PSUM")
                    nc.tensor.transpose(out=p[:fdim, 0:pdim], in_=xin,
                                        identity=ident16[:pdim, :pdim])
                    s = pc.tile([fdim, pdim], dt.float16, tag=f"tt{fdim}_{pdim}")
                    nc.scalar.copy(out=s[:], in_=p[:fdim, 0:pdim])
                    return s

                def c_chain(b):
                    x0 = xgs[b]
                    xt0 = transpose16(x0[:, 0:128], 128, 128)
                    yield
                    xt1 = transpose16(x0[:, 128:256], 128, 128)
                    yield
                    pA = pc_mm.tile([128, 128], dt.float32, tag="mm", space="PSUM")
                    nc.tensor.matmul(pA[:], xt0[:], waT0[:], start=True, stop=False)
                    nc.tensor.matmul(pA[:], xt1[:], waT1[:], start=False, stop=True)
                    yield
                    x1 = yield from ln_lrelu(b, pA[:], 128, ba_sb, lnaw_sb, lnab_sb)
                    x1t = transpose16(x1[:], 128, 128)
                    yield
                    p1 = pc_mm.tile([128, 64], dt.float32, tag="mm", space="PSUM")
                    nc.tensor.matmul(p1[:], x1t[:], w1T_sb[:], start=True, stop=True)
                    yield
                    x2 = yield from ln_lrelu(b, p1[:], 64, b1_sb, ln1w_sb, ln1b_sb)
                    x2t = transpose16(x2[:], 128, 64)
                    yield
                    p2 = pc_mm.tile([128, 32], dt.float32, tag="mm", space="PSUM")
                    nc.tensor.matmul(p2[:], x2t[:], w2T_sb[:], start=True, stop=True)
                    yield
                    x3 = yield from ln_lrelu(b, p2[:], 32, b2_sb, ln2w_sb, ln2b_sb)
                    x3t = transpose16(x3[:], 128, 32)
                    yield
                    p3 = pc_mm.tile([128, 3], dt.float32, tag="mm", space="PSUM")
                    nc.tensor.matmul(p3[:], x3t[:], w3T_sb[:], start=True, stop=True)
                    yield
                    y3 = pc.tile([128, 4], dt.float32, tag="y3")
                    nc.vector.tensor_tensor(out=y3[:, 0:3], in0=p3[:], in1=b3_sb[:], op=OP.add)
                    scr3 = pc.tile([128, 3], dt.float32, tag="scr3")
                    nc.scalar.activation(out=scr3[:], in_=y3[:, 0:3], func=AF.Square,
                                         accum_out=y3[:, 3:4])
                    yield
                    # y3e = [-2*y3 | sq | 1 | 0...] then transpose -> lhsT rows
                    y3e = pc.tile([128, 8], dt.float32, tag="y3e")
                    nc.scalar.activation(out=y3e[:, 0:3], in_=y3[:, 0:3],
                                         func=AF.Copy, scale=-2.0)
                    nc.scalar.copy(out=y3e[:, 3:4], in_=y3[:, 3:4])
                    nc.vector.memset(y3e[:, 4:5], 1.0)
                    nc.vector.memset(y3e[:, 5:8], 0.0)
                    yield
                    h3p = pc_ps.tile([128, 128], dt.float32, tag="tpp", space="PSUM")
                    nc.tensor.transpose(out=h3p[:4, 0:128], in_=y3[:], identity=ident[:])
                    nc.scalar.copy(out=cc_sb[:, b * 128:(b + 1) * 128], in_=h3p[:4, 0:128])
                    yield
                    h3q = pc_ps.tile([128, 128], dt.float32, tag="tpp", space="PSUM")
                    nc.tensor.transpose(out=h3q[:8, 0:128], in_=y3e[:], identity=ident[:])
                    lt = lhsT_pool.tile([5, 128], dt.float32, tag=f"lt{b}")
                    nc.scalar.copy(out=lt[:], in_=h3q[:5, 0:128])
                    lhsTs[b] = lt

                def ln_lrelu(b, xin, fdim, bias_bc, w_bc, b_bc):
                    # y = xin + bias; u = LN(y)*w + b; return lrelu001(u) fp16
                    y = pc.tile([128, fdim], dt.float32, tag=f"y{fdim}")
                    nc.vector.tensor_tensor(out=y[:], in0=xin, in1=bias_bc[:], op=OP.add)
                    yield
                    scr = pc.tile([128, fdim], dt.float32, tag=f"scr{fdim}")
                    msum = pc.tile([128, 1], dt.float32, tag="msum")
                    nc.scalar.activation(out=scr[:], in_=y[:], func=AF.Copy,
                                         accum_out=msum[:])
                    sqs = pc.tile([128, 1], dt.float32, tag="sqs")
                    nc.scalar.activation(out=scr[:], in_=y[:], func=AF.Square,
                                         accum_out=sqs[:])
                    yield
                    mean = pc.tile([128, 1], dt.float32, tag="mean")
                    nc.vector.tensor_scalar(out=mean[:], in0=msum[:], scalar1=1.0 / fdim,
                                            scalar2=None, op0=OP.mult)
                    var = pc.tile([128, 1], dt.float32, tag="var")
                    nc.vector.tensor_scalar(out=var[:], in0=sqs[:], scalar1=1.0 / fdim,
                                            scalar2=None, op0=OP.mult)
                    m2 = pc.tile([128, 1], dt.float32, tag="m2")
                    nc.vector.tensor_scalar(out=m2[:], in0=mean[:], scalar1=mean[:, 0:1],
                                            scalar2=None, op0=OP.mult)
                    nc.vector.tensor_tensor(out=var[:], in0=var[:], in1=m2[:], op=OP.subtract)
                    sd = pc.tile([128, 1], dt.float32, tag="sd")
                    nc.scalar.activation(out=sd[:], in_=var[:], func=AF.Sqrt,
                                         bias=eps_col[:, 0:1])
                    rstd = pc.tile([128, 1], dt.float32, tag="rstd")
                    nc.vector.reciprocal(rstd[:], sd[:])
                    yield
                    u = pc.tile([128, fdim], dt.float32, tag=f"u{fdim}")
                    nc.vector.scalar_tensor_tensor(out=u[:], in0=y[:], scalar=mean[:, 0:1],
                                                   in1=w_bc[:], op0=OP.subtract, op1=OP.mult)
                    nc.vector.scalar_tensor_tensor(out=u[:], in0=u[:], scalar=rstd[:, 0:1],
                                                   in1=b_bc[:], op0=OP.mult, op1=OP.add)
                    yield
                    ngt = pc.tile([128, fdim], dt.float32, tag=f"ng{fdim}")
                    nc.vector.tensor_scalar(out=ngt[:], in0=u[:], scalar1=0.0,
                                            scalar2=0.01, op0=OP.min, op1=OP.mult)
                    u16 = pc.tile([128, fdim], dt.float16, tag=f"u16_{fdim}")
                    nc.vector.scalar_tensor_tensor(out=u16[:], in0=u[:], scalar=0.0,
                                                   in1=ngt[:], op0=OP.max, op1=OP.add)
                    yield
                    return u16

                gens = [c_chain(b) for b in range(RB)]
                done = [False] * RB
                while not all(done):
                    for b in range(RB):
                        if not done[b]:
                            try:
                                next(gens[b])
                            except StopIteration:
                                done[b] = True

            if phases == "ABC":
                dbg16 = top.tile([4, RPAD], dt.float16, tag="dbgc")
                nc.vector.tensor_copy(out=dbg16[:], in_=cc_sb[:])
                nc.sync.dma_start(out=outD[0:4, 0:RPAD], in_=dbg16[:])
                raise _PhaseDone()

            # ---- phase D: allgather h3/sq ----
            nc.sync.dma_start(out=cc_in[:], in_=cc_sb[:])
            nc.gpsimd.collective_compute(
                "AllGather", OP.bypass, replica_groups=[list(range(CORES))],
                ins=[cc_in[:].opt()], outs=[cc_out[:].opt()])
            rhs_all = top.tile([5, NCOL], dt.float32, tag="rhs_all")
            nc.vector.memset(rhs_all[:], 0.0)
            ones_ncol = top.tile([1, NCOL], dt.float32, tag="ones_ncol")
            nc.vector.memset(ones_ncol[:], 1.0)
            nc.sync.dma_start(out=rhs_all[3:4, :], in_=ones_ncol[:])
            for s in range(CORES):
                c0 = s * SHARD
                nc.sync.dma_start(out=rhs_all[0:3, c0:c0 + SHARD],
                                  in_=cc_out[:][s, 0:3, 0:SHARD])
                nc.sync.dma_start(out=rhs_all[4:5, c0:c0 + SHARD],
                                  in_=cc_out[:][s, 3:4, 0:SHARD])

            if phases == "ABCD":
                dbg5 = top.tile([5, NCOL], dt.float16, tag="dbgd")
                nc.vector.tensor_copy(out=dbg5[:], in_=rhs_all[:])
                nc.sync.dma_start(out=outD[0:5, 0:NCOL], in_=dbg5[:])
                raise _PhaseDone()

            # ---- phase E: cdist row-block x col-chunk (fp32r matmul, fp16 out) ----
            MRG = 4   # psum chunks merged into one output tile/DMA
            with tc.tile_pool(name="pe_d", bufs=8) as pe_d, \
                 tc.tile_pool(name="pe_d2", bufs=3) as pe_d2, \
                 tc.tile_pool(name="pe_ps", bufs=6, space="PSUM") as pe_ps:
                for rb in range(RB):
                    lhsr = lhsTs[rb][:]
                    for mg in range(NCOL // (CCH * MRG)):
                        d2t = pe_d2.tile([128, CCH * MRG], dt.float16, tag="d2t")
                        for k in range(MRG):
                            ch = mg * MRG + k
                            dp = pe_ps.tile([128, CCH], dt.float32, tag="dp", space="PSUM")
                            nc.tensor.matmul(
                                dp[:], lhsr,
                                rhs_all[:, ch * CCH:(ch + 1) * CCH],
                                start=True, stop=True)
                            dtl = pe_d.tile([128, CCH], dt.float32, tag="dtl")
                            nc.vector.tensor_scalar(out=dtl[:], in0=dp[:], scalar1=0.0,
                                                    scalar2=None, op0=OP.max)
                            nc.scalar.activation(out=d2t[:, k * CCH:(k + 1) * CCH],
                                                 in_=dtl[:], func=AF.Sqrt)
                        nc.sync.dma_start(
                            out=outD[rb * 128:(rb + 1) * 128,
                                     mg * CCH * MRG:(mg + 1) * CCH * MRG],
                            in_=d2t[:])

    except _PhaseDone:
        pass
    _tc_cm.__exit__(None, None, None)
    nc.compile()
    return nc


def _prep_host(x, edge_index):
    xp = np.zeros((NPAD, FIN), np.float32)
    xp[:N] = np.asarray(x, np.float32)
    xp16 = np.ascontiguousarray(xp.T.astype(np.float16))  # [256, NPAD]

    ei = np.asarray(edge_index)
    src = np.concatenate([ei[0], np.arange(N, dtype=np.int64)]).astype(np.int64)
    dst = np.concatenate([ei[1], np.arange(N, dtype=np.int64)]).astype(np.int64)

    core = dst // SHARD
    per_core = []
    max_tiles = 0
    for c in range(CORES):
        sel = core == c
        s_c = src[sel]
        d_c = dst[sel]
        loc = d_c - c * SHARD
        blk = loc // 128
        dl = loc - blk * 128
        blocks = []
        for b in range(RB):
            m = blk == b
            blocks.append((s_c[m], d_c[m], dl[m]))
            max_tiles = max(max_tiles, (len(blocks[-1][0]) + 127) // 128)
        per_core.append(blocks)

    TB = GC * ((max_tiles + GC - 1) // GC)
    S16 = 8 * TB

    hidx = np.zeros((CORES, RB, 16, S16), np.int16)
    adrw = np.zeros((CORES, RB, 16, 8), np.int16)
    dstl = np.full((CORES, RB, 128, TB), 255.0, np.float32)
    for c in range(CORES):
        for b in range(RB):
            s_b, d_b, dl_b = per_core[c][b]
            n = len(s_b)
            js = np.arange(n)
            hidx[c, b, js % 16, js // 16] = s_b.astype(np.int16)
            dstl[c, b, js % 128, js // 128] = dl_b.astype(np.float32)
            rows = np.minimum(c * SHARD + b * 128 + np.arange(128), N - 1)
            jr = np.arange(128)
            adrw[c, b, jr % 16, jr // 16] = rows.astype(np.int16)
    hidx = np.tile(hidx, (1, 1, 8, 1))
    adrw = np.tile(adrw, (1, 1, 8, 1))
    dstrow = np.ascontiguousarray(
        dstl.transpose(0, 1, 3, 2)).reshape(CORES, RB, 1, TB * 128).astype(np.float16)
    return xp16, hidx, adrw, dstl, dstrow, TB


def build_in_maps(inputs):
    xp16, hidx, adrw, dstl, dstrow, TB = _prep_host(inputs["x"], inputs["edge_index"])
    f32 = lambda a: np.ascontiguousarray(np.asarray(a, np.float32))
    f16 = lambda a: np.ascontiguousarray(np.asarray(a, np.float32).astype(np.float16))

    def bc(vec, n):
        v = np.asarray(vec, np.float32).reshape(1, n)
        return np.ascontiguousarray(np.broadcast_to(v, (128, n)).copy())

    # rhsA: [256 (xfeat, 2 chunks of 128), 262] fp16
    # cols: 0:128 WgT head0 | 128 zero | 129:257 WgT head1 | 257 zero | 258:262 wtil
    Wg = np.asarray(inputs["W_gat"], np.float32)       # [256, 256] rows = H*F out
    att_src = np.asarray(inputs["att_src"], np.float32)  # [2, 128]
    att_dst = np.asarray(inputs["att_dst"], np.float32)
    rhsA = np.zeros((256, 262), np.float32)
    rhsA[:, 0:128] = Wg[0:128, :].T
    rhsA[:, 129:257] = Wg[128:256, :].T
    # wtil[:, c] for c in [as0, as1, ad0, ad1]
    rhsA[:, 258] = Wg[0:128, :].T @ att_src[0]
    rhsA[:, 259] = Wg[128:256, :].T @ att_src[1]
    rhsA[:, 260] = Wg[0:128, :].T @ att_dst[0]
    rhsA[:, 261] = Wg[128:256, :].T @ att_dst[1]
    rhsA16 = rhsA.astype(np.float16).reshape(2, 128, 262)

    Wa = np.asarray(inputs["Wa"], np.float32)  # [128, 256]
    waT16 = np.ascontiguousarray(Wa.T.astype(np.float16)).reshape(2, 128, 128)
    w1T16 = np.ascontiguousarray(np.asarray(inputs["W1"], np.float32).T.astype(np.float16))
    w2T16 = np.ascontiguousarray(np.asarray(inputs["W2"], np.float32).T.astype(np.float16))
    w3T16 = np.ascontiguousarray(np.asarray(inputs["W3"], np.float32).T.astype(np.float16))

    shared = {
        "xt16": xp16,
        "rhsA": np.ascontiguousarray(rhsA16),
        "waT": waT16, "w1T": w1T16, "w2T": w2T16, "w3T": w3T16,
        "bgat_bc": bc(inputs["b_gat"], 256).astype(np.float16),
        "ba_bc": bc(inputs["ba"], 128),
        "lnaw_bc": bc(inputs["lna_w"], 128), "lnab_bc": bc(inputs["lna_b"], 128),
        "b1_bc": bc(inputs["b1"], 64),
        "ln1w_bc": bc(inputs["ln1_w"], 64), "ln1b_bc": bc(inputs["ln1_b"], 64),
        "b2_bc": bc(inputs["b2"], 32),
        "ln2w_bc": bc(inputs["ln2_w"], 32), "ln2b_bc": bc(inputs["ln2_b"], 32),
        "b3_bc": bc(inputs["b3"], 3),
    }
    in_maps = [
        {**shared, "hidx": np.ascontiguousarray(hidx[c]),
         "adrows": np.ascontiguousarray(adrw[c]),
         "dstloc": np.ascontiguousarray(dstl[c]),
         "dstrow": np.ascontiguousarray(dstrow[c])}
        for c in range(CORES)
    ]
    return in_maps, TB


def kernel(**inputs):
    in_maps, TB = build_in_maps(inputs)

    import os
    phases = os.environ.get("K_PHASES", "ABCDE")
    key = (TB, phases)
    if key not in _BUILD_CACHE:
        _BUILD_CACHE[key] = _build(TB, phases)
    nc = _BUILD_CACHE[key]
    res = run_bass_kernel_spmd(nc, in_maps, core_ids=list(range(CORES)))
    global _LAST_RESULTS
    _LAST_RESULTS = res.results
    out = np.empty((N, N), np.float32)
    for c in range(CORES):
        out[c * SHARD:(c + 1) * SHARD, :] = \
            res.results[c]["outD"][:SHARD, :N].astype(np.float32)
    return out


# revision 13
# speedup vs baseline: 2.1249x; 1.3264x over previous
"""GAT (2-head) + 3x dense/LayerNorm + pairwise-distance kernel for 8 TRN2 NeuronCores.

Strategy: dst-sharded edge processing (one-hot matmul aggregation), replicated
small dense weights, row-block-sharded NxN cdist output.

v3: fp16 htable/gather rows, host-precomputed one-hot tables (no on-device
is_eq), 2-queue GC=12 gathers, whole-x preload, split-fp16 cdist matmuls,
fp16 output.
"""
import sys

import numpy as np

# Environment bootstrap (harness may run from a bare directory).
for _p in ("/root/.axon_site", "/root/.axon_site/_ro/trn_rl_repo",
           "/root/.axon_site/_ro/pypackages", "/opt/trn_rl_repo"):
    if _p not in sys.path:
        sys.path.append(_p)

import concourse.bass as bass
import concourse.bacc as bacc
import concourse.mybir as mybir
import concourse.tile as tile
from concourse.masks import make_identity
from concourse.bass_utils import run_bass_kernel_spmd

dt = mybir.dt
OP = mybir.AluOpType
AF = mybir.ActivationFunctionType

N = 10000
NPAD = 10112          # 79 * 128
NB = 79               # node blocks (phase A)
FIN = 256
F = 128               # per-head GAT dim
H = 2
HROW = 384            # htable row fp16 elems (768B, multiple of 256B)
CORES = 8
SHARD = 1250          # dst nodes per core
RB = 10               # dst blocks per core
RPAD = 1280
CCH = 512             # cdist column chunk
NCOL = 10240          # padded output columns
EPS = 1e-5

_BUILD_CACHE = {}
_LAST_RESULTS = None


GC = 6   # tile groups per dma_gather call (768 idxs = 48 descs/engine)
SUB = 3  # tiles per a_d/exp subchunk


def _build(TB, phases="ABCDE"):
    """Build the 8-core SPMD program. TB = gather tile groups per dst block (mult of GC)."""
    assert TB % GC == 0
    NCH = TB // GC
    S16 = 8 * TB          # idx columns ([128, S16] wrapped int16)

    nc = bacc.Bacc("TRN2", target_bir_lowering=False, debug=False,
                   num_devices=CORES, num_swdge_queues=2)

    def din(name, shape, d=dt.float32):
        return nc.dram_tensor(name, shape, d, kind="ExternalInput").ap()

    xt16 = din("xt16", [FIN, NPAD], dt.float16)
    rhsA = din("rhsA", [2, 128, 262], dt.float16)
    waT = din("waT", [2, 128, 128], dt.float16)
    w1T = din("w1T", [128, 64], dt.float16)
    w2T = din("w2T", [64, 32], dt.float16)
    w3T = din("w3T", [32, 3], dt.float16)
    bgat_bc = din("bgat_bc", [128, 256], dt.float16)
    ba_bc = din("ba_bc", [128, 128])
    lnaw_bc = din("lnaw_bc", [128, 128])
    lnab_bc = din("lnab_bc", [128, 128])
    b1_bc = din("b1_bc", [128, 64])
    ln1w_bc = din("ln1w_bc", [128, 64])
    ln1b_bc = din("ln1b_bc", [128, 64])
    b2_bc = din("b2_bc", [128, 32])
    ln2w_bc = din("ln2w_bc", [128, 32])
    ln2b_bc = din("ln2b_bc", [128, 32])
    b3_bc = din("b3_bc", [128, 3])
    hidx = din("hidx", [RB, 128, S16], dt.int16)
    adrows = din("adrows", [RB, 128, 8], dt.int16)
    ohtab = din("ohtab", [RB, 128, TB * 128], dt.float16)    # [edge_p, t*128+slot]
    ohTtab = din("ohTtab", [RB, 128, TB * 128], dt.float16)  # [slot_p, t*128+edge]
    outD = nc.dram_tensor("outD", [RPAD, NCOL], dt.float16, kind="ExternalOutput").ap()

    class _PhaseDone(Exception):
        pass

    import contextlib
    try:
        _tc_cm = tile.TileContext(nc)
        tc = _tc_cm.__enter__()
        est = contextlib.ExitStack()
        with est:
            top = est.enter_context(tc.tile_pool(name="top", bufs=1))
            dram = est.enter_context(tc.tile_pool(name="dram", bufs=1, space="DRAM"))

            htable = dram.tile([NPAD, HROW], dt.float16, tag="htable")
            cc_in = dram.tile([4, RPAD], dt.float32, tag="cc_in")
            cc_out = dram.tile([CORES, 4, RPAD], dt.float32, tag="cc_out")

            ident = top.tile([128, 128], dt.float32, tag="ident")
            make_identity(nc, ident[:])
            ident16 = top.tile([128, 128], dt.float16, tag="ident16")
            nc.vector.tensor_copy(out=ident16[:], in_=ident[:])
            eps_col = top.tile([128, 1], dt.float32, tag="eps_col")
            nc.vector.memset(eps_col[:], EPS)

            # ---- load replicated weights / biases into SBUF ----
            def ldw(name, ap, shape, d=dt.float32):
                t = top.tile(shape, d, tag=name)
                nc.sync.dma_start(out=t[:], in_=ap)
                return t

            rhsA0 = ldw("rhsA0", rhsA[0], [128, 262], dt.float16)
            rhsA1 = ldw("rhsA1", rhsA[1], [128, 262], dt.float16)
            waT0 = ldw("waT0", waT[0], [128, 128], dt.float16)
            waT1 = ldw("waT1", waT[1], [128, 128], dt.float16)
            w1T_sb = ldw("w1T_sb", w1T[:], [128, 64], dt.float16)
            w2T_sb = ldw("w2T_sb", w2T[:], [64, 32], dt.float16)
            w3T_sb = ldw("w3T_sb", w3T[:], [32, 3], dt.float16)
            bgat_sb = ldw("bgat_sb", bgat_bc[:], [128, 256], dt.float16)
            ba_sb = ldw("ba_sb", ba_bc[:], [128, 128])
            lnaw_sb = ldw("lnaw_sb", lnaw_bc[:], [128, 128])
            lnab_sb = ldw("lnab_sb", lnab_bc[:], [128, 128])
            b1_sb = ldw("b1_sb", b1_bc[:], [128, 64])
            ln1w_sb = ldw("ln1w_sb", ln1w_bc[:], [128, 64])
            ln1b_sb = ldw("ln1b_sb", ln1b_bc[:], [128, 64])
            b2_sb = ldw("b2_sb", b2_bc[:], [128, 32])
            ln2w_sb = ldw("ln2w_sb", ln2w_bc[:], [128, 32])
            ln2b_sb = ldw("ln2b_sb", ln2b_bc[:], [128, 32])
            b3_sb = ldw("b3_sb", b3_bc[:], [128, 3])

            # ---- phase A: htable rows [h0 | 1 | h1 | 1 | a_s(2) a_d(2)] fp16 ----
            with tc.tile_pool(name="pa", bufs=1) as pa, \
                 tc.tile_pool(name="pa_ht", bufs=3) as pa_ht, \
                 tc.tile_pool(name="pa_h", bufs=3, space="PSUM") as pa_h:
                xta = pa.tile([128, NPAD], dt.float16, tag="xta")
                xtb = pa.tile([128, NPAD], dt.float16, tag="xtb")
                nc.sync.dma_start(out=xta[:], in_=xt16[0:128, :])
                nc.sync.dma_start(out=xtb[:], in_=xt16[128:256, :])
                for i in range(NB):
                    r0 = i * 128
                    hp = pa_h.tile([128, 262], dt.float32, tag="hp", space="PSUM")
                    nc.tensor.matmul(hp[:], xta[:, r0:r0 + 128], rhsA0[:],
                                     start=True, stop=False)
                    nc.tensor.matmul(hp[:], xtb[:, r0:r0 + 128], rhsA1[:],
                                     start=False, stop=True)
                    ht = pa_ht.tile([128, 262], dt.float16, tag="ht")
                    nc.scalar.copy(out=ht[:], in_=hp[:])
                    nc.vector.memset(ht[:, 128:129], 1.0)
                    nc.vector.memset(ht[:, 257:258], 1.0)
                    nc.scalar.dma_start(out=htable[r0:r0 + 128, 0:262], in_=ht[:])

            if phases == "A":
                dbg = top.tile([128, 262], dt.float16, tag="dbgA")
                for i in range(RB):
                    nc.sync.dma_start(out=dbg[:], in_=htable[i * 128:(i + 1) * 128, 0:262])
                    nc.sync.dma_start(out=outD[i * 128:(i + 1) * 128, 0:262], in_=dbg[:])
                raise _PhaseDone()

            # ---- phase B: GAT aggregation per dst block ----
            xg_pool = est.enter_context(tc.tile_pool(name="xg", bufs=1))
            xgs = []
            with tc.tile_pool(name="pb_idx", bufs=2) as pb_idx, \
                 tc.tile_pool(name="pb_oht", bufs=2) as pb_oht, \
                 tc.tile_pool(name="pb_g", bufs=3) as pb_g, \
                 tc.tile_pool(name="pb_ad", bufs=2) as pb_ad, \
                 tc.tile_pool(name="pb_ex", bufs=4) as pb_ex, \
                 tc.tile_pool(name="pb_rhs", bufs=6) as pb_rhs, \
                 tc.tile_pool(name="pb_ep", bufs=2) as pb_ep, \
                 tc.tile_pool(name="pb_ps", bufs=2, space="PSUM") as pb_ps, \
                 tc.tile_pool(name="pb_adp", bufs=4, space="PSUM") as pb_adp:
                for b in range(RB):
                    hix = pb_idx.tile([128, S16], dt.int16, tag="hix")
                    nc.sync.dma_start(out=hix[:], in_=hidx[b])
                    arw = pb_idx.tile([128, 8], dt.int16, tag="arw")
                    nc.sync.dma_start(out=arw[:], in_=adrows[b])
                    oh_sb = pb_oht.tile([128, TB * 128], dt.float16, tag="oh_sb")
                    nc.sync.dma_start(out=oh_sb[:], in_=ohtab[b])
                    ohT_sb = pb_oht.tile([128, TB * 128], dt.float16, tag="ohT_sb")
                    nc.sync.dma_start(out=ohT_sb[:], in_=ohTtab[b])

                    # a_d for this block's 128 dsts
                    adrow_g = pb_ad.tile([128, 1, HROW], dt.float16, tag="adrow_g")
                    nc.gpsimd.dma_gather(
                        out_ap=adrow_g[:], in_ap=htable[:],
                        idxs_ap=arw[:], num_idxs=128, num_idxs_reg=128,
                        elem_size=HROW)
                    adblk = pb_ad.tile([128, 2], dt.float16, tag="adblk")
                    nc.vector.tensor_copy(out=adblk[:], in_=adrow_g[:, 0, 260:262])

                    ps = pb_ps.tile([128, 258], dt.float32, tag="ps", space="PSUM")

                    for c in range(NCH):
                        g = pb_g.tile([128, GC, HROW], dt.float16, tag="g")
                        nc.gpsimd.dma_gather(
                            out_ap=g[:], in_ap=htable[:],
                            idxs_ap=hix[:, c * 8 * GC:(c + 1) * 8 * GC],
                            num_idxs=128 * GC, num_idxs_reg=128 * GC,
                            elem_size=HROW, queue_num=c % 2)
                        for hs in range(GC // SUB):
                            t0 = c * GC + hs * SUB
                            adps = pb_adp.tile([128, SUB, 2], dt.float32, tag="adps",
                                               space="PSUM")
                            for k in range(SUB):
                                t = t0 + k
                                nc.tensor.matmul(adps[:, k, :],
                                                 ohT_sb[:, t * 128:(t + 1) * 128],
                                                 adblk[:], start=True, stop=True)
                            # e = a_s[src] + a_d[dst]; exv = max(exp(e), exp(0.2e))
                            exr = pb_ex.tile([128, SUB, 2], dt.float32, tag="exr")
                            nc.vector.tensor_tensor(
                                out=exr[:], in0=adps[:],
                                in1=g[:, hs * SUB:(hs + 1) * SUB, 258:260], op=OP.add)
                            exn = pb_ex.tile([128, SUB, 2], dt.float32, tag="exn")
                            nc.scalar.activation(out=exn[:], in_=exr[:], func=AF.Exp,
                                                 scale=0.2)
                            exp_ = pb_ex.tile([128, SUB, 2], dt.float32, tag="exp_")
                            nc.scalar.activation(out=exp_[:], in_=exr[:], func=AF.Exp)
                            exv = pb_ex.tile([128, SUB, 2], dt.float32, tag="exv")
                            nc.vector.tensor_tensor(out=exv[:], in0=exp_[:], in1=exn[:],
                                                    op=OP.max)
                            for k in range(SUB):
                                t = t0 + k
                                tl = hs * SUB + k
                                rhs = pb_rhs.tile([128, 258], dt.float16, tag="rhs")
                                nc.scalar.activation(out=rhs[:, 0:129],
                                                     in_=g[:, tl, 0:129],
                                                     func=AF.Copy,
                                                     scale=exv[:, k, 0:1])
                                nc.vector.tensor_scalar(
                                    out=rhs[:, 129:258], in0=g[:, tl, 129:258],
                                    scalar1=exv[:, k, 1:2], scalar2=None, op0=OP.mult)
                                nc.tensor.matmul(ps[:],
                                                 oh_sb[:, t * 128:(t + 1) * 128],
                                                 rhs[:],
                                                 start=(t == 0), stop=(t == TB - 1))

                    # epilogue: normalize, +b_gat, lrelu(0.01) -> fp16 xg
                    rec0 = pb_ep.tile([128, 1], dt.float32, tag="rec0")
                    nc.vector.reciprocal(rec0[:], ps[:, 128:129])
                    rec1 = pb_ep.tile([128, 1], dt.float32, tag="rec1")
                    nc.vector.reciprocal(rec1[:], ps[:, 257:258])
                    xg = xg_pool.tile([128, 256], dt.float16, tag=f"xg{b}")
                    nc.scalar.activation(out=xg[:, 0:128], in_=ps[:, 0:128],
                                         func=AF.Copy, scale=rec0[:])
                    nc.scalar.activation(out=xg[:, 128:256], in_=ps[:, 129:257],
                                         func=AF.Copy, scale=rec1[:])
                    nc.vector.tensor_tensor(out=xg[:], in0=xg[:], in1=bgat_sb[:], op=OP.add)
                    ng = pb_ep.tile([128, 256], dt.float16, tag="ng")
                    nc.vector.tensor_scalar(out=ng[:], in0=xg[:], scalar1=0.0,
                                            scalar2=0.01, op0=OP.min, op1=OP.mult)
                    nc.vector.scalar_tensor_tensor(out=xg[:], in0=xg[:], scalar=0.0,
                                                   in1=ng[:], op0=OP.max, op1=OP.add)
                    xgs.append(xg)

            if phases == "AB":
                for b in range(RB):
                    nc.sync.dma_start(out=outD[b * 128:(b + 1) * 128, 0:256], in_=xgs[b][:])
                raise _PhaseDone()

            # ---- phase C: dense + LN on own shard (stage-parallel across blocks) ----
            cc_sb = top.tile([4, RPAD], dt.float32, tag="cc_sb")
            with tc.tile_pool(name="pc", bufs=12) as pc, \
                 tc.tile_pool(name="pc_ps", bufs=2, space="PSUM") as pc_ps, \
                 tc.tile_pool(name="pc_mm", bufs=4, space="PSUM") as pc_mm:

                def transpose16(xin, pdim, fdim):
                    # xin fp16 [pdim, fdim] -> sbuf fp16 [fdim, pdim]
                    p = pc_ps.tile([128, 128], dt.float16, tag="tpp16", space="PSUM")
                    nc.tensor.transpose(out=p[:fdim, 0:pdim], in_=xin,
                                        identity=ident16[:pdim, :pdim])
                    s = pc.tile([fdim, pdim], dt.float16, tag=f"tt{fdim}_{pdim}")
                    nc.scalar.copy(out=s[:], in_=p[:fdim, 0:pdim])
                    return s

                def c_chain(b):
                    x0 = xgs[b]
                    xt0 = transpose16(x0[:, 0:128], 128, 128)
                    yield
                    xt1 = transpose16(x0[:, 128:256], 128, 128)
                    yield
                    pA = pc_mm.tile([128, 128], dt.float32, tag="mm", space="PSUM")
                    nc.tensor.matmul(pA[:], xt0[:], waT0[:], start=True, stop=False)
                    nc.tensor.matmul(pA[:], xt1[:], waT1[:], start=False, stop=True)
                    yield
                    x1 = yield from ln_lrelu(b, pA[:], 128, ba_sb, lnaw_sb, lnab_sb)
                    x1t = transpose16(x1[:], 128, 128)
                    yield
                    p1 = pc_mm.tile([128, 64], dt.float32, tag="mm", space="PSUM")
                    nc.tensor.matmul(p1[:], x1t[:], w1T_sb[:], start=True, stop=True)
                    yield
                    x2 = yield from ln_lrelu(b, p1[:], 64, b1_sb, ln1w_sb, ln1b_sb)
                    x2t = transpose16(x2[:], 128, 64)
                    yield
                    p2 = pc_mm.tile([128, 32], dt.float32, tag="mm", space="PSUM")
                    nc.tensor.matmul(p2[:], x2t[:], w2T_sb[:], start=True, stop=True)
                    yield
                    x3 = yield from ln_lrelu(b, p2[:], 32, b2_sb, ln2w_sb, ln2b_sb)
                    x3t = transpose16(x3[:], 128, 32)
                    yield
                    p3 = pc_mm.tile([128, 3], dt.float32, tag="mm", space="PSUM")
                    nc.tensor.matmul(p3[:], x3t[:], w3T_sb[:], start=True, stop=True)
                    yield
                    y3 = pc.tile([128, 4], dt.float32, tag="y3")
                    nc.vector.tensor_tensor(out=y3[:, 0:3], in0=p3[:], in1=b3_sb[:], op=OP.add)
                    scr3 = pc.tile([128, 3], dt.float32, tag="scr3")
                    nc.scalar.activation(out=scr3[:], in_=y3[:, 0:3], func=AF.Square,
                                         accum_out=y3[:, 3:4])
                    yield
                    h3p = pc_ps.tile([128, 128], dt.float32, tag="tpp", space="PSUM")
                    nc.tensor.transpose(out=h3p[:4, 0:128], in_=y3[:], identity=ident[:])
                    nc.scalar.copy(out=cc_sb[:, b * 128:(b + 1) * 128], in_=h3p[:4, 0:128])

                def ln_lrelu(b, xin, fdim, bias_bc, w_bc, b_bc):
                    # y = xin + bias; u = LN(y)*w + b; return lrelu001(u) fp16
                    y = pc.tile([128, fdim], dt.float32, tag=f"y{fdim}")
                    nc.vector.tensor_tensor(out=y[:], in0=xin, in1=bias_bc[:], op=OP.add)
                    yield
                    scr = pc.tile([128, fdim], dt.float32, tag=f"scr{fdim}")
                    msum = pc.tile([128, 1], dt.float32, tag="msum")
                    nc.scalar.activation(out=scr[:], in_=y[:], func=AF.Copy,
                                         accum_out=msum[:])
                    sqs = pc.tile([128, 1], dt.float32, tag="sqs")
                    nc.scalar.activation(out=scr[:], in_=y[:], func=AF.Square,
                                         accum_out=sqs[:])
                    yield
                    mean = pc.tile([128, 1], dt.float32, tag="mean")
                    nc.vector.tensor_scalar(out=mean[:], in0=msum[:], scalar1=1.0 / fdim,
                                            scalar2=None, op0=OP.mult)
                    var = pc.tile([128, 1], dt.float32, tag="var")
                    nc.vector.tensor_scalar(out=var[:], in0=sqs[:], scalar1=1.0 / fdim,
                                            scalar2=None, op0=OP.mult)
                    m2 = pc.tile([128, 1], dt.float32, tag="m2")
                    nc.vector.tensor_scalar(out=m2[:], in0=mean[:], scalar1=mean[:, 0:1],
                                            scalar2=None, op0=OP.mult)
                    nc.vector.tensor_tensor(out=var[:], in0=var[:], in1=m2[:], op=OP.subtract)
                    sd = pc.tile([128, 1], dt.float32, tag="sd")
                    nc.scalar.activation(out=sd[:], in_=var[:], func=AF.Sqrt,
                                         bias=eps_col[:, 0:1])
                    rstd = pc.tile([128, 1], dt.float32, tag="rstd")
                    nc.vector.reciprocal(rstd[:], sd[:])
                    yield
                    u = pc.tile([128, fdim], dt.float32, tag=f"u{fdim}")
                    nc.vector.scalar_tensor_tensor(out=u[:], in0=y[:], scalar=mean[:, 0:1],
                                                   in1=w_bc[:], op0=OP.subtract, op1=OP.mult)
                    nc.vector.scalar_tensor_tensor(out=u[:], in0=u[:], scalar=rstd[:, 0:1],
                                                   in1=b_bc[:], op0=OP.mult, op1=OP.add)
                    yield
                    ngt = pc.tile([128, fdim], dt.float32, tag=f"ng{fdim}")
                    nc.vector.tensor_scalar(out=ngt[:], in0=u[:], scalar1=0.0,
                                            scalar2=0.01, op0=OP.min, op1=OP.mult)
                    u16 = pc.tile([128, fdim], dt.float16, tag=f"u16_{fdim}")
                    nc.vector.scalar_tensor_tensor(out=u16[:], in0=u[:], scalar=0.0,
                                                   in1=ngt[:], op0=OP.max, op1=OP.add)
                    yield
                    return u16

                gens = [c_chain(b) for b in range(RB)]
                done = [False] * RB
                while not all(done):
                    for b in range(RB):
                        if not done[b]:
                            try:
                                next(gens[b])
                            except StopIteration:
                                done[b] = True

            if phases == "ABC":
                dbg16 = top.tile([4, RPAD], dt.float16, tag="dbgc")
                nc.vector.tensor_copy(out=dbg16[:], in_=cc_sb[:])
                nc.sync.dma_start(out=outD[0:4, 0:RPAD], in_=dbg16[:])
                raise _PhaseDone()

            # ---- phase D: allgather h3/sq; build split-fp16 cdist operands ----
            nc.sync.dma_start(out=cc_in[:], in_=cc_sb[:])
            # lhsT16 rows: [-2a(3) | -2a(3) | -2b(3) | sqhi | sqlo | 1 | 1]
            # (compute in partition-0 tiles, assemble via sbuf-to-sbuf DMA)
            lhsT16 = top.tile([13, RPAD], dt.float16, tag="lhsT16")
            pd = est.enter_context(tc.tile_pool(name="pd", bufs=1))
            a_own = pd.tile([4, RPAD], dt.float16, tag="a_own")
            nc.vector.tensor_copy(out=a_own[:], in_=cc_sb[:])
            b_own = pd.tile([4, RPAD], dt.float16, tag="b_own")
            nc.vector.tensor_tensor(out=b_own[:], in0=cc_sb[:], in1=a_own[:],
                                    op=OP.subtract)
            na4 = pd.tile([4, RPAD], dt.float16, tag="na4")
            nc.scalar.activation(out=na4[:], in_=a_own[:], func=AF.Copy, scale=-2.0)
            nb4 = pd.tile([4, RPAD], dt.float16, tag="nb4")
            nc.scalar.activation(out=nb4[:], in_=b_own[:], func=AF.Copy, scale=-2.0)
            ones_r = pd.tile([2, RPAD], dt.float16, tag="ones_r")
            nc.vector.memset(ones_r[:], 1.0)
            nc.sync.dma_start(out=lhsT16[0:3, :], in_=na4[0:3, :])
            nc.sync.dma_start(out=lhsT16[3:6, :], in_=na4[0:3, :])
            nc.sync.dma_start(out=lhsT16[6:9, :], in_=nb4[0:3, :])
            nc.sync.dma_start(out=lhsT16[9:10, :], in_=a_own[3:4, :])
            nc.sync.dma_start(out=lhsT16[10:11, :], in_=b_own[3:4, :])
            nc.sync.dma_start(out=lhsT16[11:13, :], in_=ones_r[:])

            nc.gpsimd.collective_compute(
                "AllGather", OP.bypass, replica_groups=[list(range(CORES))],
                ins=[cc_in[:].opt()], outs=[cc_out[:].opt()])
            # rhs16 rows: [a(3) | b(3) | a(3) | 1 | 1 | sqhi | sqlo]
            rhs_f = pd.tile([4, NCOL], dt.float32, tag="rhs_f")
            nc.vector.memset(rhs_f[:], 0.0)
            for s in range(CORES):
                c0 = s * SHARD
                nc.sync.dma_start(out=rhs_f[0:4, c0:c0 + SHARD],
                                  in_=cc_out[:][s, 0:4, 0:SHARD])
            a16 = pd.tile([4, NCOL], dt.float16, tag="a16")
            nc.vector.tensor_copy(out=a16[:], in_=rhs_f[:])                  # a | sqhi
            b16 = pd.tile([4, NCOL], dt.float16, tag="b16")
            nc.vector.tensor_tensor(out=b16[:], in0=rhs_f[:], in1=a16[:],
                                    op=OP.subtract)                          # b | sqlo
            ones_n = pd.tile([2, NCOL], dt.float16, tag="ones_n")
            nc.vector.memset(ones_n[:], 1.0)
            rhs16 = top.tile([13, NCOL], dt.float16, tag="rhs16")
            nc.sync.dma_start(out=rhs16[0:3, :], in_=a16[0:3, :])
            nc.sync.dma_start(out=rhs16[3:6, :], in_=b16[0:3, :])
            nc.sync.dma_start(out=rhs16[6:9, :], in_=a16[0:3, :])
            nc.sync.dma_start(out=rhs16[9:11, :], in_=ones_n[:])
            nc.sync.dma_start(out=rhs16[11:12, :], in_=a16[3:4, :])
            nc.sync.dma_start(out=rhs16[12:13, :], in_=b16[3:4, :])

            if phases == "ABCD":
                nc.sync.dma_start(out=outD[0:13, 0:NCOL], in_=rhs16[:])
                raise _PhaseDone()

            # ---- phase E: cdist row-block x col-chunk (split-fp16 matmul, fp16 out) ----
            MRG = 4   # psum chunks merged into one output tile/DMA
            with tc.tile_pool(name="pe_d", bufs=8) as pe_d, \
                 tc.tile_pool(name="pe_d2", bufs=3) as pe_d2, \
                 tc.tile_pool(name="pe_ps", bufs=6, space="PSUM") as pe_ps:
                for rb in range(RB):
                    for mg in range(NCOL // (CCH * MRG)):
                        d2t = pe_d2.tile([128, CCH * MRG], dt.float16, tag="d2t")
                        for k in range(MRG):
                            ch = mg * MRG + k
                            dp = pe_ps.tile([128, CCH], dt.float32, tag="dp", space="PSUM")
                            nc.tensor.matmul(
                                dp[:], lhsT16[:, rb * 128:(rb + 1) * 128],
                                rhs16[:, ch * CCH:(ch + 1) * CCH],
                                start=True, stop=True)
                            dtl = pe_d.tile([128, CCH], dt.float32, tag="dtl")
                            nc.vector.tensor_scalar(out=dtl[:], in0=dp[:], scalar1=0.0,
                                                    scalar2=None, op0=OP.max)
                            nc.scalar.activation(out=d2t[:, k * CCH:(k + 1) * CCH],
                                                 in_=dtl[:], func=AF.Sqrt)
                        nc.sync.dma_start(
                            out=outD[rb * 128:(rb + 1) * 128,
                                     mg * CCH * MRG:(mg + 1) * CCH * MRG],
                            in_=d2t[:])

    except _PhaseDone:
        pass
    _tc_cm.__exit__(None, None, None)
    nc.compile()
    return nc


def _prep_host(x, edge_index):
    xp = np.zeros((NPAD, FIN), np.float32)
    xp[:N] = np.asarray(x, np.float32)
    xp16 = np.ascontiguousarray(xp.T.astype(np.float16))  # [256, NPAD]

    ei = np.asarray(edge_index)
    src = np.concatenate([ei[0], np.arange(N, dtype=np.int64)]).astype(np.int64)
    dst = np.concatenate([ei[1], np.arange(N, dtype=np.int64)]).astype(np.int64)

    core = dst // SHARD
    per_core = []
    max_tiles = 0
    for c in range(CORES):
        sel = core == c
        s_c = src[sel]
        d_c = dst[sel]
        loc = d_c - c * SHARD
        blk = loc // 128
        dl = loc - blk * 128
        blocks = []
        for b in range(RB):
            m = blk == b
            blocks.append((s_c[m], dl[m]))
            max_tiles = max(max_tiles, (len(blocks[-1][0]) + 127) // 128)
        per_core.append(blocks)

    TB = GC * ((max_tiles + GC - 1) // GC)
    S16 = 8 * TB
    NE = TB * 128

    hidx = np.zeros((CORES, RB, 16, S16), np.int16)
    adrw = np.zeros((CORES, RB, 16, 8), np.int16)
    ohtab = np.zeros((CORES, RB, 128, NE), np.float16)
    ohTtab = np.zeros((CORES, RB, 128, NE), np.float16)
    for c in range(CORES):
        for b in range(RB):
            s_b, dl_b = per_core[c][b]
            n = len(s_b)
            js = np.arange(n)
            hidx[c, b, js % 16, js // 16] = s_b.astype(np.int16)
            p = js % 128          # edge partition
            t = js // 128         # edge tile
            sl = dl_b.astype(np.int64)
            ohtab[c, b, p, t * 128 + sl] = 1.0
            ohTtab[c, b, sl, t * 128 + p] = 1.0
            rows = np.minimum(c * SHARD + b * 128 + np.arange(128), N - 1)
            jr = np.arange(128)
            adrw[c, b, jr % 16, jr // 16] = rows.astype(np.int16)
    hidx = np.tile(hidx, (1, 1, 8, 1))
    adrw = np.tile(adrw, (1, 1, 8, 1))
    return xp16, hidx, adrw, ohtab, ohTtab, TB


def build_in_maps(inputs):
    xp16, hidx, adrw, ohtab, ohTtab, TB = _prep_host(inputs["x"], inputs["edge_index"])

    def bc(vec, n, f16=False):
        v = np.asarray(vec, np.float32).reshape(1, n)
        out = np.ascontiguousarray(np.broadcast_to(v, (128, n)).copy())
        return out.astype(np.float16) if f16 else out

    # rhsA: [256 (xfeat, 2 chunks of 128), 262] fp16
    # cols: 0:128 WgT head0 | 128 zero | 129:257 WgT head1 | 257 zero | 258:262 wtil
    Wg = np.asarray(inputs["W_gat"], np.float32)       # [256, 256] rows = H*F out
    att_src = np.asarray(inputs["att_src"], np.float32)  # [2, 128]
    att_dst = np.asarray(inputs["att_dst"], np.float32)
    rhsA = np.zeros((256, 262), np.float32)
    rhsA[:, 0:128] = Wg[0:128, :].T
    rhsA[:, 129:257] = Wg[128:256, :].T
    rhsA[:, 258] = Wg[0:128, :].T @ att_src[0]
    rhsA[:, 259] = Wg[128:256, :].T @ att_src[1]
    rhsA[:, 260] = Wg[0:128, :].T @ att_dst[0]
    rhsA[:, 261] = Wg[128:256, :].T @ att_dst[1]
    rhsA16 = rhsA.astype(np.float16).reshape(2, 128, 262)

    Wa = np.asarray(inputs["Wa"], np.float32)  # [128, 256]
    waT16 = np.ascontiguousarray(Wa.T.astype(np.float16)).reshape(2, 128, 128)
    w1T16 = np.ascontiguousarray(np.asarray(inputs["W1"], np.float32).T.astype(np.float16))
    w2T16 = np.ascontiguousarray(np.asarray(inputs["W2"], np.float32).T.astype(np.float16))
    w3T16 = np.ascontiguousarray(np.asarray(inputs["W3"], np.float32).T.astype(np.float16))

    shared = {
        "xt16": xp16,
        "rhsA": np.ascontiguousarray(rhsA16),
        "waT": waT16, "w1T": w1T16, "w2T": w2T16, "w3T": w3T16,
        "bgat_bc": bc(inputs["b_gat"], 256, f16=True),
        "ba_bc": bc(inputs["ba"], 128),
        "lnaw_bc": bc(inputs["lna_w"], 128), "lnab_bc": bc(inputs["lna_b"], 128),
        "b1_bc": bc(inputs["b1"], 64),
        "ln1w_bc": bc(inputs["ln1_w"], 64), "ln1b_bc": bc(inputs["ln1_b"], 64),
        "b2_bc": bc(inputs["b2"], 32),
        "ln2w_bc": bc(inputs["ln2_w"], 32), "ln2b_bc": bc(inputs["ln2_b"], 32),
        "b3_bc": bc(inputs["b3"], 3),
    }
    in_maps = [
        {**shared, "hidx": np.ascontiguousarray(hidx[c]),
         "adrows": np.ascontiguousarray(adrw[c]),
         "ohtab": np.ascontiguousarray(ohtab[c]),
         "ohTtab": np.ascontiguousarray(ohTtab[c])}
        for c in range(CORES)
    ]
    return in_maps, TB


def kernel(**inputs):
    in_maps, TB = build_in_maps(inputs)

    import os
    phases = os.environ.get("K_PHASES", "ABCDE")
    key = (TB, phases)
    if key not in _BUILD_CACHE:
        _BUILD_CACHE[key] = _build(TB, phases)
    nc = _BUILD_CACHE[key]
    res = run_bass_kernel_spmd(nc, in_maps, core_ids=list(range(CORES)))
    global _LAST_RESULTS
    _LAST_RESULTS = res.results
    out = np.empty((N, N), np.float32)
    for c in range(CORES):
        out[c * SHARD:(c + 1) * SHARD, :] = \
            res.results[c]["outD"][:SHARD, :N].astype(np.float32)
    return out


# revision 16
# speedup vs baseline: 2.2639x; 1.0654x over previous
"""GAT (2-head) + 3x dense/LayerNorm + pairwise-distance kernel for 8 TRN2 NeuronCores.

Strategy: dst-sharded edge processing (one-hot matmul aggregation), replicated
small dense weights, row-block-sharded NxN cdist output.

v3: fp16 htable/gather rows, host-precomputed one-hot tables (no on-device
is_eq), 2-queue GC=12 gathers, whole-x preload, split-fp16 cdist matmuls,
fp16 output.
"""
import sys

import numpy as np

# Environment bootstrap (harness may run from a bare directory).
for _p in ("/root/.axon_site", "/root/.axon_site/_ro/trn_rl_repo",
           "/root/.axon_site/_ro/pypackages", "/opt/trn_rl_repo"):
    if _p not in sys.path:
        sys.path.append(_p)

import concourse.bass as bass
import concourse.bacc as bacc
import concourse.mybir as mybir
import concourse.tile as tile
from concourse.masks import make_identity
from concourse.bass_utils import run_bass_kernel_spmd

dt = mybir.dt
OP = mybir.AluOpType
AF = mybir.ActivationFunctionType

N = 10000
NPAD = 10112          # 79 * 128
NB = 79               # node blocks (phase A)
FIN = 256
F = 128               # per-head GAT dim
H = 2
HROW = 384            # htable row fp16 elems (768B, multiple of 256B)
CORES = 8
SHARD = 1250          # dst nodes per core
RB = 10               # dst blocks per core
RPAD = 1280
CCH = 512             # cdist column chunk
NCOL = 10240          # padded output columns
EPS = 1e-5

_BUILD_CACHE = {}
_LAST_RESULTS = None


GC = 6   # tile groups per dma_gather call (768 idxs = 48 descs/engine)
SUB = 3  # tiles per a_d/exp subchunk


def _build(TB, phases="ABCDE"):
    """Build the 8-core SPMD program. TB = gather tile groups per dst block (mult of GC)."""
    assert TB % GC == 0
    NCH = TB // GC
    S16 = 8 * TB          # idx columns ([128, S16] wrapped int16)

    nc = bacc.Bacc("TRN2", target_bir_lowering=False, debug=False,
                   num_devices=CORES, num_swdge_queues=2)

    def din(name, shape, d=dt.float32):
        return nc.dram_tensor(name, shape, d, kind="ExternalInput").ap()

    xt16 = din("xt16", [FIN, NPAD], dt.float16)
    rhsA = din("rhsA", [2, 128, 262], dt.float16)
    waT = din("waT", [2, 128, 128], dt.float16)
    w1T = din("w1T", [128, 64], dt.float16)
    w2T = din("w2T", [64, 32], dt.float16)
    w3T = din("w3T", [32, 3], dt.float16)
    bgat_bc = din("bgat_bc", [128, 256], dt.float16)
    ba_bc = din("ba_bc", [128, 128])
    lnaw_bc = din("lnaw_bc", [128, 128])
    lnab_bc = din("lnab_bc", [128, 128])
    b1_bc = din("b1_bc", [128, 64])
    ln1w_bc = din("ln1w_bc", [128, 64])
    ln1b_bc = din("ln1b_bc", [128, 64])
    b2_bc = din("b2_bc", [128, 32])
    ln2w_bc = din("ln2w_bc", [128, 32])
    ln2b_bc = din("ln2b_bc", [128, 32])
    b3_bc = din("b3_bc", [128, 3])
    hidx = din("hidx", [RB, 128, S16], dt.int16)
    ohtab = din("ohtab", [RB, 128, TB * 128], dt.float16)    # [edge_p, t*128+slot]
    ohTtab = din("ohTtab", [RB, 128, TB * 128], dt.float16)  # [slot_p, t*128+edge]
    outD = nc.dram_tensor("outD", [RPAD, NCOL], dt.float16, kind="ExternalOutput").ap()

    class _PhaseDone(Exception):
        pass

    import contextlib
    try:
        _tc_cm = tile.TileContext(nc)
        tc = _tc_cm.__enter__()
        est = contextlib.ExitStack()
        with est:
            top = est.enter_context(tc.tile_pool(name="top", bufs=1))
            dram = est.enter_context(tc.tile_pool(name="dram", bufs=1, space="DRAM"))

            htable = dram.tile([NPAD, HROW], dt.float16, tag="htable")
            HALF = RPAD // 2
            cc_inA = dram.tile([4, HALF], dt.float32, tag="cc_inA")
            cc_outA = dram.tile([CORES, 4, HALF], dt.float32, tag="cc_outA")
            cc_inB = dram.tile([4, HALF], dt.float32, tag="cc_inB")
            cc_outB = dram.tile([CORES, 4, HALF], dt.float32, tag="cc_outB")

            ident = top.tile([128, 128], dt.float32, tag="ident")
            make_identity(nc, ident[:])
            ident16 = top.tile([128, 128], dt.float16, tag="ident16")
            nc.vector.tensor_copy(out=ident16[:], in_=ident[:])
            eps_col = top.tile([128, 1], dt.float32, tag="eps_col")
            nc.vector.memset(eps_col[:], EPS)

            # ---- load replicated weights / biases into SBUF ----
            def ldw(name, ap, shape, d=dt.float32):
                t = top.tile(shape, d, tag=name)
                nc.sync.dma_start(out=t[:], in_=ap)
                return t

            rhsA0 = ldw("rhsA0", rhsA[0], [128, 262], dt.float16)
            rhsA1 = ldw("rhsA1", rhsA[1], [128, 262], dt.float16)
            waT0 = ldw("waT0", waT[0], [128, 128], dt.float16)
            waT1 = ldw("waT1", waT[1], [128, 128], dt.float16)
            w1T_sb = ldw("w1T_sb", w1T[:], [128, 64], dt.float16)
            w2T_sb = ldw("w2T_sb", w2T[:], [64, 32], dt.float16)
            w3T_sb = ldw("w3T_sb", w3T[:], [32, 3], dt.float16)
            bgat_sb = ldw("bgat_sb", bgat_bc[:], [128, 256], dt.float16)
            ba_sb = ldw("ba_sb", ba_bc[:], [128, 128])
            lnaw_sb = ldw("lnaw_sb", lnaw_bc[:], [128, 128])
            lnab_sb = ldw("lnab_sb", lnab_bc[:], [128, 128])
            b1_sb = ldw("b1_sb", b1_bc[:], [128, 64])
            ln1w_sb = ldw("ln1w_sb", ln1w_bc[:], [128, 64])
            ln1b_sb = ldw("ln1b_sb", ln1b_bc[:], [128, 64])
            b2_sb = ldw("b2_sb", b2_bc[:], [128, 32])
            ln2w_sb = ldw("ln2w_sb", ln2w_bc[:], [128, 32])
            ln2b_sb = ldw("ln2b_sb", ln2b_bc[:], [128, 32])
            b3_sb = ldw("b3_sb", b3_bc[:], [128, 3])

            # ---- phase A: htable rows [h0 | 1 | h1 | 1 | a_s(2) a_d(2)] fp16 ----
            with tc.tile_pool(name="pa", bufs=1) as pa, \
                 tc.tile_pool(name="pa_ht", bufs=3) as pa_ht, \
                 tc.tile_pool(name="pa_h", bufs=3, space="PSUM") as pa_h:
                xta = pa.tile([128, NPAD], dt.float16, tag="xta")
                xtb = pa.tile([128, NPAD], dt.float16, tag="xtb")
                nc.sync.dma_start(out=xta[:], in_=xt16[0:128, :])
                nc.sync.dma_start(out=xtb[:], in_=xt16[128:256, :])
                for i in range(NB):
                    r0 = i * 128
                    hp = pa_h.tile([128, 262], dt.float32, tag="hp", space="PSUM")
                    nc.tensor.matmul(hp[:], xta[:, r0:r0 + 128], rhsA0[:],
                                     start=True, stop=False)
                    nc.tensor.matmul(hp[:], xtb[:, r0:r0 + 128], rhsA1[:],
                                     start=False, stop=True)
                    ht = pa_ht.tile([128, 262], dt.float16, tag="ht")
                    if i % 2 == 0:
                        nc.scalar.copy(out=ht[:], in_=hp[:])
                    else:
                        nc.vector.tensor_copy(out=ht[:], in_=hp[:])
                    nc.gpsimd.memset(ht[:, 128:129], 1.0)
                    nc.gpsimd.memset(ht[:, 257:258], 1.0)
                    nc.sync.dma_start(out=htable[r0:r0 + 128, 0:262], in_=ht[:])

            if phases == "A":
                dbg = top.tile([128, 262], dt.float16, tag="dbgA")
                for i in range(RB):
                    nc.sync.dma_start(out=dbg[:], in_=htable[i * 128:(i + 1) * 128, 0:262])
                    nc.sync.dma_start(out=outD[i * 128:(i + 1) * 128, 0:262], in_=dbg[:])
                raise _PhaseDone()

            # ---- phase B: GAT aggregation per dst block ----
            xg_pool = est.enter_context(tc.tile_pool(name="xg", bufs=1))
            xgs = []
            with tc.tile_pool(name="pb_idx", bufs=2) as pb_idx, \
                 tc.tile_pool(name="pb_oht", bufs=2) as pb_oht, \
                 tc.tile_pool(name="pb_g", bufs=4) as pb_g, \
                 tc.tile_pool(name="pb_ad", bufs=2) as pb_ad, \
                 tc.tile_pool(name="pb_ex", bufs=4) as pb_ex, \
                 tc.tile_pool(name="pb_rhs", bufs=6) as pb_rhs, \
                 tc.tile_pool(name="pb_ep", bufs=2) as pb_ep, \
                 tc.tile_pool(name="pb_ps", bufs=2, space="PSUM") as pb_ps, \
                 tc.tile_pool(name="pb_adp", bufs=4, space="PSUM") as pb_adp:
                for b in range(RB):
                    hix = pb_idx.tile([128, S16], dt.int16, tag="hix")
                    nc.sync.dma_start(out=hix[:], in_=hidx[b])
                    oh_sb = pb_oht.tile([128, TB * 128], dt.float16, tag="oh_sb")
                    nc.sync.dma_start(out=oh_sb[:], in_=ohtab[b])
                    ohT_sb = pb_oht.tile([128, TB * 128], dt.float16, tag="ohT_sb")
                    nc.sync.dma_start(out=ohT_sb[:], in_=ohTtab[b])

                    adblk = pb_ad.tile([128, 2], dt.float16, tag="adblk")
                    ps = pb_ps.tile([128, 258], dt.float32, tag="ps", space="PSUM")

                    for c in range(NCH):
                        g = pb_g.tile([128, GC, HROW], dt.float16, tag="g")
                        nc.gpsimd.dma_gather(
                            out_ap=g[:], in_ap=htable[:],
                            idxs_ap=hix[:, c * 8 * GC:(c + 1) * 8 * GC],
                            num_idxs=128 * GC, num_idxs_reg=128 * GC,
                            elem_size=HROW, queue_num=c % 2)
                        if c == 0:
                            # tile 0 rows are this block's own dst rows (self loops)
                            nc.vector.tensor_copy(out=adblk[:], in_=g[:, 0, 260:262])
                        for hs in range(GC // SUB):
                            t0 = c * GC + hs * SUB
                            adps = pb_adp.tile([128, SUB, 2], dt.float32, tag="adps",
                                               space="PSUM")
                            for k in range(SUB):
                                t = t0 + k
                                nc.tensor.matmul(adps[:, k, :],
                                                 ohT_sb[:, t * 128:(t + 1) * 128],
                                                 adblk[:], start=True, stop=True)
                            # e = a_s[src] + a_d[dst]; exv = max(exp(e), exp(0.2e))
                            exr = pb_ex.tile([128, SUB, 2], dt.float32, tag="exr")
                            nc.vector.tensor_tensor(
                                out=exr[:], in0=adps[:],
                                in1=g[:, hs * SUB:(hs + 1) * SUB, 258:260], op=OP.add)
                            exn = pb_ex.tile([128, SUB, 2], dt.float32, tag="exn")
                            nc.scalar.activation(out=exn[:], in_=exr[:], func=AF.Exp,
                                                 scale=0.2)
                            exp_ = pb_ex.tile([128, SUB, 2], dt.float32, tag="exp_")
                            nc.scalar.activation(out=exp_[:], in_=exr[:], func=AF.Exp)
                            exv = pb_ex.tile([128, SUB, 2], dt.float32, tag="exv")
                            nc.vector.tensor_tensor(out=exv[:], in0=exp_[:], in1=exn[:],
                                                    op=OP.max)
                            for k in range(SUB):
                                t = t0 + k
                                tl = hs * SUB + k
                                rhs = pb_rhs.tile([128, 258], dt.float16, tag="rhs")
                                if t % 2 == 0:
                                    nc.scalar.activation(out=rhs[:, 0:129],
                                                         in_=g[:, tl, 0:129],
                                                         func=AF.Copy,
                                                         scale=exv[:, k, 0:1])
                                    nc.vector.tensor_scalar(
                                        out=rhs[:, 129:258], in0=g[:, tl, 129:258],
                                        scalar1=exv[:, k, 1:2], scalar2=None, op0=OP.mult)
                                else:
                                    nc.vector.tensor_scalar(
                                        out=rhs[:, 0:129], in0=g[:, tl, 0:129],
                                        scalar1=exv[:, k, 0:1], scalar2=None, op0=OP.mult)
                                    nc.scalar.activation(out=rhs[:, 129:258],
                                                         in_=g[:, tl, 129:258],
                                                         func=AF.Copy,
                                                         scale=exv[:, k, 1:2])
                                nc.tensor.matmul(ps[:],
                                                 oh_sb[:, t * 128:(t + 1) * 128],
                                                 rhs[:],
                                                 start=(t == 0), stop=(t == TB - 1))

                    # epilogue: normalize, +b_gat, lrelu(0.01) -> fp16 xg
                    rec0 = pb_ep.tile([128, 1], dt.float32, tag="rec0")
                    nc.vector.reciprocal(rec0[:], ps[:, 128:129])
                    rec1 = pb_ep.tile([128, 1], dt.float32, tag="rec1")
                    nc.vector.reciprocal(rec1[:], ps[:, 257:258])
                    xg = xg_pool.tile([128, 256], dt.float16, tag=f"xg{b}")
                    nc.scalar.activation(out=xg[:, 0:128], in_=ps[:, 0:128],
                                         func=AF.Copy, scale=rec0[:])
                    nc.scalar.activation(out=xg[:, 128:256], in_=ps[:, 129:257],
                                         func=AF.Copy, scale=rec1[:])
                    nc.vector.tensor_tensor(out=xg[:], in0=xg[:], in1=bgat_sb[:], op=OP.add)
                    ng = pb_ep.tile([128, 256], dt.float16, tag="ng")
                    nc.vector.tensor_scalar(out=ng[:], in0=xg[:], scalar1=0.0,
                                            scalar2=0.01, op0=OP.min, op1=OP.mult)
                    nc.vector.scalar_tensor_tensor(out=xg[:], in0=xg[:], scalar=0.0,
                                                   in1=ng[:], op0=OP.max, op1=OP.add)
                    xgs.append(xg)

            if phases == "AB":
                for b in range(RB):
                    nc.sync.dma_start(out=outD[b * 128:(b + 1) * 128, 0:256], in_=xgs[b][:])
                raise _PhaseDone()

            # ---- phase C: dense + LN on own shard (stage-parallel across blocks) ----
            cc_sb = top.tile([4, RPAD], dt.float32, tag="cc_sb")
            with tc.tile_pool(name="pc", bufs=12) as pc, \
                 tc.tile_pool(name="pc_ps", bufs=2, space="PSUM") as pc_ps, \
                 tc.tile_pool(name="pc_mm", bufs=4, space="PSUM") as pc_mm:

                def transpose16(xin, pdim, fdim):
                    # xin fp16 [pdim, fdim] -> sbuf fp16 [fdim, pdim]
                    p = pc_ps.tile([128, 128], dt.float16, tag="tpp16", space="PSUM")
                    nc.tensor.transpose(out=p[:fdim, 0:pdim], in_=xin,
                                        identity=ident16[:pdim, :pdim])
                    s = pc.tile([fdim, pdim], dt.float16, tag=f"tt{fdim}_{pdim}")
                    nc.scalar.copy(out=s[:], in_=p[:fdim, 0:pdim])
                    return s

                def c_chain(b):
                    x0 = xgs[b]
                    xt0 = transpose16(x0[:, 0:128], 128, 128)
                    yield
                    xt1 = transpose16(x0[:, 128:256], 128, 128)
                    yield
                    pA = pc_mm.tile([128, 128], dt.float32, tag="mm", space="PSUM")
                    nc.tensor.matmul(pA[:], xt0[:], waT0[:], start=True, stop=False)
                    nc.tensor.matmul(pA[:], xt1[:], waT1[:], start=False, stop=True)
                    yield
                    x1 = yield from ln_lrelu(b, pA[:], 128, ba_sb, lnaw_sb, lnab_sb)
                    x1t = transpose16(x1[:], 128, 128)
                    yield
                    p1 = pc_mm.tile([128, 64], dt.float32, tag="mm", space="PSUM")
                    nc.tensor.matmul(p1[:], x1t[:], w1T_sb[:], start=True, stop=True)
                    yield
                    x2 = yield from ln_lrelu(b, p1[:], 64, b1_sb, ln1w_sb, ln1b_sb)
                    x2t = transpose16(x2[:], 128, 64)
                    yield
                    p2 = pc_mm.tile([128, 32], dt.float32, tag="mm", space="PSUM")
                    nc.tensor.matmul(p2[:], x2t[:], w2T_sb[:], start=True, stop=True)
                    yield
                    x3 = yield from ln_lrelu(b, p2[:], 32, b2_sb, ln2w_sb, ln2b_sb)
                    x3t = transpose16(x3[:], 128, 32)
                    yield
                    p3 = pc_mm.tile([128, 3], dt.float32, tag="mm", space="PSUM")
                    nc.tensor.matmul(p3[:], x3t[:], w3T_sb[:], start=True, stop=True)
                    yield
                    y3 = pc.tile([128, 4], dt.float32, tag="y3")
                    nc.vector.tensor_tensor(out=y3[:, 0:3], in0=p3[:], in1=b3_sb[:], op=OP.add)
                    scr3 = pc.tile([128, 3], dt.float32, tag="scr3")
                    nc.scalar.activation(out=scr3[:], in_=y3[:, 0:3], func=AF.Square,
                                         accum_out=y3[:, 3:4])
                    yield
                    h3p = pc_ps.tile([128, 128], dt.float32, tag="tpp", space="PSUM")
                    nc.tensor.transpose(out=h3p[:4, 0:128], in_=y3[:], identity=ident[:])
                    nc.scalar.copy(out=cc_sb[:, b * 128:(b + 1) * 128], in_=h3p[:4, 0:128])

                def ln_lrelu(b, xin, fdim, bias_bc, w_bc, b_bc):
                    # y = xin + bias; u = LN(y)*w + b; return lrelu001(u) fp16
                    y = pc.tile([128, fdim], dt.float32, tag=f"y{fdim}")
                    nc.vector.tensor_tensor(out=y[:], in0=xin, in1=bias_bc[:], op=OP.add)
                    yield
                    scr = pc.tile([128, fdim], dt.float32, tag=f"scr{fdim}")
                    msum = pc.tile([128, 1], dt.float32, tag="msum")
                    nc.scalar.activation(out=scr[:], in_=y[:], func=AF.Copy,
                                         accum_out=msum[:])
                    sqs = pc.tile([128, 1], dt.float32, tag="sqs")
                    nc.scalar.activation(out=scr[:], in_=y[:], func=AF.Square,
                                         accum_out=sqs[:])
                    yield
                    mean = pc.tile([128, 1], dt.float32, tag="mean")
                    nc.vector.tensor_scalar(out=mean[:], in0=msum[:], scalar1=1.0 / fdim,
                                            scalar2=None, op0=OP.mult)
                    var = pc.tile([128, 1], dt.float32, tag="var")
                    nc.vector.tensor_scalar(out=var[:], in0=sqs[:], scalar1=1.0 / fdim,
                                            scalar2=None, op0=OP.mult)
                    m2 = pc.tile([128, 1], dt.float32, tag="m2")
                    nc.vector.tensor_scalar(out=m2[:], in0=mean[:], scalar1=mean[:, 0:1],
                                            scalar2=None, op0=OP.mult)
                    nc.vector.tensor_tensor(out=var[:], in0=var[:], in1=m2[:], op=OP.subtract)
                    sd = pc.tile([128, 1], dt.float32, tag="sd")
                    nc.scalar.activation(out=sd[:], in_=var[:], func=AF.Sqrt,
                                         bias=eps_col[:, 0:1])
                    rstd = pc.tile([128, 1], dt.float32, tag="rstd")
                    nc.vector.reciprocal(rstd[:], sd[:])
                    yield
                    u = pc.tile([128, fdim], dt.float32, tag=f"u{fdim}")
                    nc.vector.scalar_tensor_tensor(out=u[:], in0=y[:], scalar=mean[:, 0:1],
                                                   in1=w_bc[:], op0=OP.subtract, op1=OP.mult)
                    nc.vector.scalar_tensor_tensor(out=u[:], in0=u[:], scalar=rstd[:, 0:1],
                                                   in1=b_bc[:], op0=OP.mult, op1=OP.add)
                    yield
                    ngt = pc.tile([128, fdim], dt.float32, tag=f"ng{fdim}")
                    nc.vector.tensor_scalar(out=ngt[:], in0=u[:], scalar1=0.0,
                                            scalar2=0.01, op0=OP.min, op1=OP.mult)
                    u16 = pc.tile([128, fdim], dt.float16, tag=f"u16_{fdim}")
                    nc.vector.scalar_tensor_tensor(out=u16[:], in0=u[:], scalar=0.0,
                                                   in1=ngt[:], op0=OP.max, op1=OP.add)
                    yield
                    return u16

                gens = [c_chain(b) for b in range(RB)]

                def run_gens(idxs):
                    done = {b: False for b in idxs}
                    while not all(done.values()):
                        for b in idxs:
                            if not done[b]:
                                try:
                                    next(gens[b])
                                except StopIteration:
                                    done[b] = True

                run_gens(range(RB // 2))
                nc.sync.dma_start(out=cc_inA[:], in_=cc_sb[:, 0:HALF])
                nc.gpsimd.collective_compute(
                    "AllGather", OP.bypass, replica_groups=[list(range(CORES))],
                    ins=[cc_inA[:].opt()], outs=[cc_outA[:].opt()])
                run_gens(range(RB // 2, RB))
                nc.sync.dma_start(out=cc_inB[:], in_=cc_sb[:, HALF:RPAD])
                nc.gpsimd.collective_compute(
                    "AllGather", OP.bypass, replica_groups=[list(range(CORES))],
                    ins=[cc_inB[:].opt()], outs=[cc_outB[:].opt()])

            if phases == "ABC":
                dbg16 = top.tile([4, RPAD], dt.float16, tag="dbgc")
                nc.vector.tensor_copy(out=dbg16[:], in_=cc_sb[:])
                nc.sync.dma_start(out=outD[0:4, 0:RPAD], in_=dbg16[:])
                raise _PhaseDone()

            # ---- phase D: build split-fp16 cdist operands ----
            # lhsT16 rows: [-2a(3) | -2a(3) | -2b(3) | sqhi | sqlo | 1 | 1]
            # (compute in partition-0 tiles, assemble via sbuf-to-sbuf DMA)
            lhsT16 = top.tile([13, RPAD], dt.float16, tag="lhsT16")
            pd = est.enter_context(tc.tile_pool(name="pd", bufs=1))
            a_own = pd.tile([4, RPAD], dt.float16, tag="a_own")
            nc.vector.tensor_copy(out=a_own[:], in_=cc_sb[:])
            b_own = pd.tile([4, RPAD], dt.float16, tag="b_own")
            nc.vector.tensor_tensor(out=b_own[:], in0=cc_sb[:], in1=a_own[:],
                                    op=OP.subtract)
            na4 = pd.tile([4, RPAD], dt.float16, tag="na4")
            nc.scalar.activation(out=na4[:], in_=a_own[:], func=AF.Copy, scale=-2.0)
            nb4 = pd.tile([4, RPAD], dt.float16, tag="nb4")
            nc.scalar.activation(out=nb4[:], in_=b_own[:], func=AF.Copy, scale=-2.0)
            ones_r = pd.tile([2, RPAD], dt.float16, tag="ones_r")
            nc.vector.memset(ones_r[:], 1.0)
            nc.sync.dma_start(out=lhsT16[0:3, :], in_=na4[0:3, :])
            nc.sync.dma_start(out=lhsT16[3:6, :], in_=na4[0:3, :])
            nc.sync.dma_start(out=lhsT16[6:9, :], in_=nb4[0:3, :])
            nc.sync.dma_start(out=lhsT16[9:10, :], in_=a_own[3:4, :])
            nc.sync.dma_start(out=lhsT16[10:11, :], in_=b_own[3:4, :])
            nc.sync.dma_start(out=lhsT16[11:13, :], in_=ones_r[:])

            # rhs16 rows: [a(3) | b(3) | a(3) | 1 | 1 | sqhi | sqlo]
            rhs_f = pd.tile([4, NCOL], dt.float32, tag="rhs_f")
            nc.vector.memset(rhs_f[:, N:NCOL], 0.0)
            for s in range(CORES):
                c0 = s * SHARD
                nc.sync.dma_start(out=rhs_f[0:4, c0:c0 + HALF],
                                  in_=cc_outA[:][s])
                nc.sync.dma_start(out=rhs_f[0:4, c0 + HALF:c0 + SHARD],
                                  in_=cc_outB[:][s, 0:4, 0:SHARD - HALF])
            a16 = pd.tile([4, NCOL], dt.float16, tag="a16")
            nc.vector.tensor_copy(out=a16[:], in_=rhs_f[:])                  # a | sqhi
            b16 = pd.tile([4, NCOL], dt.float16, tag="b16")
            nc.vector.tensor_tensor(out=b16[:], in0=rhs_f[:], in1=a16[:],
                                    op=OP.subtract)                          # b | sqlo
            ones_n = pd.tile([2, NCOL], dt.float16, tag="ones_n")
            nc.vector.memset(ones_n[:], 1.0)
            rhs16 = top.tile([13, NCOL], dt.float16, tag="rhs16")
            nc.sync.dma_start(out=rhs16[0:3, :], in_=a16[0:3, :])
            nc.sync.dma_start(out=rhs16[3:6, :], in_=b16[0:3, :])
            nc.sync.dma_start(out=rhs16[6:9, :], in_=a16[0:3, :])
            nc.sync.dma_start(out=rhs16[9:11, :], in_=ones_n[:])
            nc.sync.dma_start(out=rhs16[11:12, :], in_=a16[3:4, :])
            nc.sync.dma_start(out=rhs16[12:13, :], in_=b16[3:4, :])

            if phases == "ABCD":
                nc.sync.dma_start(out=outD[0:13, 0:NCOL], in_=rhs16[:])
                raise _PhaseDone()

            # ---- phase E: cdist row-block x col-chunk (split-fp16 matmul, fp16 out) ----
            MRG = 4   # psum chunks merged into one output tile/DMA
            with tc.tile_pool(name="pe_d", bufs=3) as pe_d, \
                 tc.tile_pool(name="pe_d2", bufs=3) as pe_d2, \
                 tc.tile_pool(name="pe_ps", bufs=6, space="PSUM") as pe_ps:
                for rb in range(RB):
                    for mg in range(NCOL // (CCH * MRG)):
                        d2t = pe_d2.tile([128, CCH * MRG], dt.float16, tag="d2t")
                        dtl = pe_d.tile([128, CCH * MRG], dt.float32, tag="dtl")
                        for k in range(MRG):
                            ch = mg * MRG + k
                            dp = pe_ps.tile([128, CCH], dt.float32, tag="dp", space="PSUM")
                            nc.tensor.matmul(
                                dp[:], lhsT16[:, rb * 128:(rb + 1) * 128],
                                rhs16[:, ch * CCH:(ch + 1) * CCH],
                                start=True, stop=True)
                            nc.vector.tensor_scalar(out=dtl[:, k * CCH:(k + 1) * CCH],
                                                    in0=dp[:], scalar1=0.0,
                                                    scalar2=None, op0=OP.max)
                        nc.scalar.activation(out=d2t[:], in_=dtl[:], func=AF.Sqrt)
                        nc.sync.dma_start(
                            out=outD[rb * 128:(rb + 1) * 128,
                                     mg * CCH * MRG:(mg + 1) * CCH * MRG],
                            in_=d2t[:])

    except _PhaseDone:
        pass
    _tc_cm.__exit__(None, None, None)
    nc.compile()
    return nc


def _prep_host(x, edge_index):
    xp = np.zeros((NPAD, FIN), np.float32)
    xp[:N] = np.asarray(x, np.float32)
    xp16 = np.ascontiguousarray(xp.T.astype(np.float16))  # [256, NPAD]

    ei = np.asarray(edge_index)
    src = ei[0].astype(np.int64)
    dst = ei[1].astype(np.int64)

    core = dst // SHARD
    per_core = []
    max_tiles = 0
    for c in range(CORES):
        sel = core == c
        s_c = src[sel]
        d_c = dst[sel]
        loc = d_c - c * SHARD
        blk = loc // 128
        dl = loc - blk * 128
        blocks = []
        for b in range(RB):
            m = blk == b
            blocks.append((s_c[m], dl[m]))
            # tile 0 holds the block's self-loop edges; rest start at tile 1
            max_tiles = max(max_tiles, 1 + (len(blocks[-1][0]) + 127) // 128)
        per_core.append(blocks)

    TB = GC * ((max_tiles + GC - 1) // GC)
    S16 = 8 * TB
    NE = TB * 128

    hidx = np.zeros((CORES, RB, 16, S16), np.int16)
    ohtab = np.zeros((CORES, RB, 128, NE), np.float16)
    ohTtab = np.zeros((CORES, RB, 128, NE), np.float16)
    for c in range(CORES):
        for b in range(RB):
            # tile 0: self loops (edge at partition p has src=dst=block row p)
            rows = c * SHARD + b * 128 + np.arange(128)
            real = rows < N
            crows = np.minimum(rows, N - 1)
            jr = np.arange(128)
            hidx[c, b, jr % 16, jr // 16] = crows.astype(np.int16)
            pr = jr[real]
            ohtab[c, b, pr, pr] = 1.0
            ohTtab[c, b, pr, pr] = 1.0
            # remaining edges from tile 1 on
            s_b, dl_b = per_core[c][b]
            n = len(s_b)
            js = 128 + np.arange(n)
            hidx[c, b, js % 16, js // 16] = s_b.astype(np.int16)
            p = js % 128          # edge partition
            t = js // 128         # edge tile (>= 1)
            sl = dl_b.astype(np.int64)
            ohtab[c, b, p, t * 128 + sl] = 1.0
            ohTtab[c, b, sl, t * 128 + p] = 1.0
    hidx = np.tile(hidx, (1, 1, 8, 1))
    return xp16, hidx, ohtab, ohTtab, TB


def build_in_maps(inputs):
    xp16, hidx, ohtab, ohTtab, TB = _prep_host(inputs["x"], inputs["edge_index"])

    def bc(vec, n, f16=False):
        v = np.asarray(vec, np.float32).reshape(1, n)
        out = np.ascontiguousarray(np.broadcast_to(v, (128, n)).copy())
        return out.astype(np.float16) if f16 else out

    # rhsA: [256 (xfeat, 2 chunks of 128), 262] fp16
    # cols: 0:128 WgT head0 | 128 zero | 129:257 WgT head1 | 257 zero | 258:262 wtil
    Wg = np.asarray(inputs["W_gat"], np.float32)       # [256, 256] rows = H*F out
    att_src = np.asarray(inputs["att_src"], np.float32)  # [2, 128]
    att_dst = np.asarray(inputs["att_dst"], np.float32)
    rhsA = np.zeros((256, 262), np.float32)
    rhsA[:, 0:128] = Wg[0:128, :].T
    rhsA[:, 129:257] = Wg[128:256, :].T
    rhsA[:, 258] = Wg[0:128, :].T @ att_src[0]
    rhsA[:, 259] = Wg[128:256, :].T @ att_src[1]
    rhsA[:, 260] = Wg[0:128, :].T @ att_dst[0]
    rhsA[:, 261] = Wg[128:256, :].T @ att_dst[1]
    rhsA16 = rhsA.astype(np.float16).reshape(2, 128, 262)

    Wa = np.asarray(inputs["Wa"], np.float32)  # [128, 256]
    waT16 = np.ascontiguousarray(Wa.T.astype(np.float16)).reshape(2, 128, 128)
    w1T16 = np.ascontiguousarray(np.asarray(inputs["W1"], np.float32).T.astype(np.float16))
    w2T16 = np.ascontiguousarray(np.asarray(inputs["W2"], np.float32).T.astype(np.float16))
    w3T16 = np.ascontiguousarray(np.asarray(inputs["W3"], np.float32).T.astype(np.float16))

    shared = {
        "xt16": xp16,
        "rhsA": np.ascontiguousarray(rhsA16),
        "waT": waT16, "w1T": w1T16, "w2T": w2T16, "w3T": w3T16,
        "bgat_bc": bc(inputs["b_gat"], 256, f16=True),
        "ba_bc": bc(inputs["ba"], 128),
        "lnaw_bc": bc(inputs["lna_w"], 128), "lnab_bc": bc(inputs["lna_b"], 128),
        "b1_bc": bc(inputs["b1"], 64),
        "ln1w_bc": bc(inputs["ln1_w"], 64), "ln1b_bc": bc(inputs["ln1_b"], 64),
        "b2_bc": bc(inputs["b2"], 32),
        "ln2w_bc": bc(inputs["ln2_w"], 32), "ln2b_bc": bc(inputs["ln2_b"], 32),
        "b3_bc": bc(inputs["b3"], 3),
    }
    in_maps = [
        {**shared, "hidx": np.ascontiguousarray(hidx[c]),
         "ohtab": np.ascontiguousarray(ohtab[c]),
         "ohTtab": np.ascontiguousarray(ohTtab[c])}
        for c in range(CORES)
    ]
    return in_maps, TB


def kernel(**inputs):
    in_maps, TB = build_in_maps(inputs)

    import os
    phases = os.environ.get("K_PHASES", "ABCDE")
    key = (TB, phases)
    if key not in _BUILD_CACHE:
        _BUILD_CACHE[key] = _build(TB, phases)
    nc = _BUILD_CACHE[key]
    res = run_bass_kernel_spmd(nc, in_maps, core_ids=list(range(CORES)))
    global _LAST_RESULTS
    _LAST_RESULTS = res.results
    out = np.empty((N, N), np.float32)
    for c in range(CORES):
        out[c * SHARD:(c + 1) * SHARD, :] = \
            res.results[c]["outD"][:SHARD, :N].astype(np.float32)
    return out


# revision 18
# speedup vs baseline: 2.2822x; 1.0081x over previous
"""GAT (2-head) + 3x dense/LayerNorm + pairwise-distance kernel for 8 TRN2 NeuronCores.

Strategy: dst-sharded edge processing (one-hot matmul aggregation), replicated
small dense weights, row-block-sharded NxN cdist output.

v3: fp16 htable/gather rows, host-precomputed one-hot tables (no on-device
is_eq), 2-queue GC=12 gathers, whole-x preload, split-fp16 cdist matmuls,
fp16 output.
"""
import sys

import numpy as np

# Environment bootstrap (harness may run from a bare directory).
for _p in ("/root/.axon_site", "/root/.axon_site/_ro/trn_rl_repo",
           "/root/.axon_site/_ro/pypackages", "/opt/trn_rl_repo"):
    if _p not in sys.path:
        sys.path.append(_p)

import concourse.bass as bass
import concourse.bacc as bacc
import concourse.mybir as mybir
import concourse.tile as tile
from concourse.masks import make_identity
from concourse.bass_utils import run_bass_kernel_spmd

dt = mybir.dt
OP = mybir.AluOpType
AF = mybir.ActivationFunctionType

N = 10000
NPAD = 10112          # 79 * 128
NB = 79               # node blocks (phase A)
FIN = 256
F = 128               # per-head GAT dim
H = 2
HROW = 384            # htable row fp16 elems (768B, multiple of 256B)
CORES = 8
SHARD = 1250          # dst nodes per core
RB = 10               # dst blocks per core
RPAD = 1280
CCH = 512             # cdist column chunk
NCOL = 10240          # padded output columns
EPS = 1e-5

_BUILD_CACHE = {}
_LAST_RESULTS = None


GC = 6   # tile groups per dma_gather call (768 idxs = 48 descs/engine)
SUB = 3  # tiles per a_d/exp subchunk


def _build(TB, phases="ABCDE"):
    """Build the 8-core SPMD program. TB = gather tile groups per dst block (mult of GC)."""
    assert TB % GC == 0
    NCH = TB // GC
    S16 = 8 * TB          # idx columns ([128, S16] wrapped int16)

    nc = bacc.Bacc("TRN2", target_bir_lowering=False, debug=False,
                   num_devices=CORES, num_swdge_queues=2)

    def din(name, shape, d=dt.float32):
        return nc.dram_tensor(name, shape, d, kind="ExternalInput").ap()

    xt16 = din("xt16", [FIN, NPAD], dt.float16)
    rhsA = din("rhsA", [2, 128, 262], dt.float16)
    waT = din("waT", [2, 128, 128], dt.float16)
    w1T = din("w1T", [128, 64], dt.float16)
    w2T = din("w2T", [64, 32], dt.float16)
    w3T = din("w3T", [32, 3], dt.float16)
    bgat_bc = din("bgat_bc", [128, 256], dt.float16)
    ba_bc = din("ba_bc", [128, 128])
    lnaw_bc = din("lnaw_bc", [128, 128])
    lnab_bc = din("lnab_bc", [128, 128])
    b1_bc = din("b1_bc", [128, 64])
    ln1w_bc = din("ln1w_bc", [128, 64])
    ln1b_bc = din("ln1b_bc", [128, 64])
    b2_bc = din("b2_bc", [128, 32])
    ln2w_bc = din("ln2w_bc", [128, 32])
    ln2b_bc = din("ln2b_bc", [128, 32])
    b3_bc = din("b3_bc", [128, 3])
    hidx = din("hidx", [RB, 128, S16], dt.int16)
    ohtab = din("ohtab", [RB, 128, TB * 128], dt.float16)    # [edge_p, t*128+slot]
    ohTtab = din("ohTtab", [RB, 128, TB * 128], dt.float16)  # [slot_p, t*128+edge]
    outD = nc.dram_tensor("outD", [RPAD, NCOL], dt.float16, kind="ExternalOutput").ap()

    class _PhaseDone(Exception):
        pass

    import contextlib
    try:
        _tc_cm = tile.TileContext(nc)
        tc = _tc_cm.__enter__()
        est = contextlib.ExitStack()
        with est:
            top = est.enter_context(tc.tile_pool(name="top", bufs=1))
            dram = est.enter_context(tc.tile_pool(name="dram", bufs=1, space="DRAM"))

            htable = dram.tile([NPAD, HROW], dt.float16, tag="htable")
            HALF = RPAD // 2
            cc_inA = dram.tile([4, HALF], dt.float32, tag="cc_inA")
            cc_outA = dram.tile([CORES, 4, HALF], dt.float32, tag="cc_outA")
            cc_inB = dram.tile([4, HALF], dt.float32, tag="cc_inB")
            cc_outB = dram.tile([CORES, 4, HALF], dt.float32, tag="cc_outB")

            ident = top.tile([128, 128], dt.float32, tag="ident")
            make_identity(nc, ident[:])
            ident16 = top.tile([128, 128], dt.float16, tag="ident16")
            nc.vector.tensor_copy(out=ident16[:], in_=ident[:])
            eps_col = top.tile([128, 1], dt.float32, tag="eps_col")
            nc.vector.memset(eps_col[:], EPS)
            eps4_col = top.tile([128, 1], dt.float32, tag="eps4_col")
            nc.vector.memset(eps4_col[:], 1e-4)

            # ---- load replicated weights / biases into SBUF ----
            def ldw(name, ap, shape, d=dt.float32):
                t = top.tile(shape, d, tag=name)
                nc.sync.dma_start(out=t[:], in_=ap)
                return t

            rhsA0 = ldw("rhsA0", rhsA[0], [128, 262], dt.float16)
            rhsA1 = ldw("rhsA1", rhsA[1], [128, 262], dt.float16)
            waT0 = ldw("waT0", waT[0], [128, 128], dt.float16)
            waT1 = ldw("waT1", waT[1], [128, 128], dt.float16)
            w1T_sb = ldw("w1T_sb", w1T[:], [128, 64], dt.float16)
            w2T_sb = ldw("w2T_sb", w2T[:], [64, 32], dt.float16)
            w3T_sb = ldw("w3T_sb", w3T[:], [32, 3], dt.float16)
            bgat_sb = ldw("bgat_sb", bgat_bc[:], [128, 256], dt.float16)
            ba_sb = ldw("ba_sb", ba_bc[:], [128, 128])
            lnaw_sb = ldw("lnaw_sb", lnaw_bc[:], [128, 128])
            lnab_sb = ldw("lnab_sb", lnab_bc[:], [128, 128])
            b1_sb = ldw("b1_sb", b1_bc[:], [128, 64])
            ln1w_sb = ldw("ln1w_sb", ln1w_bc[:], [128, 64])
            ln1b_sb = ldw("ln1b_sb", ln1b_bc[:], [128, 64])
            b2_sb = ldw("b2_sb", b2_bc[:], [128, 32])
            ln2w_sb = ldw("ln2w_sb", ln2w_bc[:], [128, 32])
            ln2b_sb = ldw("ln2b_sb", ln2b_bc[:], [128, 32])
            b3_sb = ldw("b3_sb", b3_bc[:], [128, 3])

            # ---- phase A: htable rows [h0 | 1 | h1 | 1 | a_s(2) a_d(2)] fp16 ----
            with tc.tile_pool(name="pa", bufs=1) as pa, \
                 tc.tile_pool(name="pa_ht", bufs=4) as pa_ht, \
                 tc.tile_pool(name="pa_h", bufs=4, space="PSUM") as pa_h:
                xta = pa.tile([128, NPAD], dt.float16, tag="xta")
                xtb = pa.tile([128, NPAD], dt.float16, tag="xtb")
                nc.sync.dma_start(out=xta[:], in_=xt16[0:128, :])
                nc.sync.dma_start(out=xtb[:], in_=xt16[128:256, :])
                for i in range(NB):
                    r0 = i * 128
                    hp = pa_h.tile([128, 262], dt.float32, tag="hp", space="PSUM")
                    nc.tensor.matmul(hp[:], xta[:, r0:r0 + 128], rhsA0[:],
                                     start=True, stop=False)
                    nc.tensor.matmul(hp[:], xtb[:, r0:r0 + 128], rhsA1[:],
                                     start=False, stop=True)
                    ht = pa_ht.tile([128, 262], dt.float16, tag="ht")
                    if i % 2 == 0:
                        nc.scalar.copy(out=ht[:], in_=hp[:])
                    else:
                        nc.vector.tensor_copy(out=ht[:], in_=hp[:])
                    nc.gpsimd.memset(ht[:, 128:129], 1.0)
                    nc.gpsimd.memset(ht[:, 257:258], 1.0)
                    nc.sync.dma_start(out=htable[r0:r0 + 128, 0:262], in_=ht[:])

            if phases == "A":
                dbg = top.tile([128, 262], dt.float16, tag="dbgA")
                for i in range(RB):
                    nc.sync.dma_start(out=dbg[:], in_=htable[i * 128:(i + 1) * 128, 0:262])
                    nc.sync.dma_start(out=outD[i * 128:(i + 1) * 128, 0:262], in_=dbg[:])
                raise _PhaseDone()

            # ---- phase B: GAT aggregation per dst block ----
            xg_pool = est.enter_context(tc.tile_pool(name="xg", bufs=1))
            xgs = []
            with tc.tile_pool(name="pb_idx", bufs=2) as pb_idx, \
                 tc.tile_pool(name="pb_oht", bufs=3) as pb_oht, \
                 tc.tile_pool(name="pb_g", bufs=4) as pb_g, \
                 tc.tile_pool(name="pb_ad", bufs=2) as pb_ad, \
                 tc.tile_pool(name="pb_ex", bufs=4) as pb_ex, \
                 tc.tile_pool(name="pb_rhs", bufs=6) as pb_rhs, \
                 tc.tile_pool(name="pb_ep", bufs=2) as pb_ep, \
                 tc.tile_pool(name="pb_ps", bufs=2, space="PSUM") as pb_ps, \
                 tc.tile_pool(name="pb_adp", bufs=4, space="PSUM") as pb_adp:
                for _slot in range(4):
                    gz = pb_g.tile([128, GC, HROW], dt.float16, tag="g")
                    nc.vector.memset(gz[:], 0.0)
                for b in range(RB):
                    hix = pb_idx.tile([128, S16], dt.int16, tag="hix")
                    nc.sync.dma_start(out=hix[:], in_=hidx[b])
                    oh_sb = pb_oht.tile([128, TB * 128], dt.float16, tag="oh_sb")
                    nc.sync.dma_start(out=oh_sb[:], in_=ohtab[b])
                    ohT_sb = pb_oht.tile([128, TB * 128], dt.float16, tag="ohT_sb")
                    nc.sync.dma_start(out=ohT_sb[:], in_=ohTtab[b])

                    adblk = pb_ad.tile([128, 2], dt.float16, tag="adblk")
                    ps = pb_ps.tile([128, 258], dt.float32, tag="ps", space="PSUM")

                    for c in range(NCH):
                        g = pb_g.tile([128, GC, HROW], dt.float16, tag="g")
                        nc.gpsimd.dma_gather(
                            out_ap=g[:], in_ap=htable[:],
                            idxs_ap=hix[:, c * 8 * GC:(c + 1) * 8 * GC],
                            num_idxs=128 * GC, num_idxs_reg=128 * GC,
                            elem_size=HROW, queue_num=c % 2)
                        if c == 0:
                            # tile 0 rows are this block's own dst rows (self loops)
                            nc.vector.tensor_copy(out=adblk[:], in_=g[:, 0, 260:262])
                        for hs in range(GC // SUB):
                            t0 = c * GC + hs * SUB
                            adps = pb_adp.tile([128, SUB, 2], dt.float32, tag="adps",
                                               space="PSUM")
                            for k in range(SUB):
                                t = t0 + k
                                nc.tensor.matmul(adps[:, k, :],
                                                 ohT_sb[:, t * 128:(t + 1) * 128],
                                                 adblk[:], start=True, stop=True)
                            # e = a_s[src] + a_d[dst]; exv = max(exp(e), exp(0.2e))
                            exr = pb_ex.tile([128, SUB, 2], dt.float32, tag="exr")
                            nc.vector.tensor_tensor(
                                out=exr[:], in0=adps[:],
                                in1=g[:, hs * SUB:(hs + 1) * SUB, 258:260], op=OP.add)
                            exn = pb_ex.tile([128, SUB, 2], dt.float32, tag="exn")
                            nc.scalar.activation(out=exn[:], in_=exr[:], func=AF.Exp,
                                                 scale=0.2)
                            exp_ = pb_ex.tile([128, SUB, 2], dt.float32, tag="exp_")
                            nc.scalar.activation(out=exp_[:], in_=exr[:], func=AF.Exp)
                            exv = pb_ex.tile([128, SUB, 2], dt.float32, tag="exv")
                            nc.vector.tensor_tensor(out=exv[:], in0=exp_[:], in1=exn[:],
                                                    op=OP.max)
                            for k in range(SUB):
                                t = t0 + k
                                tl = hs * SUB + k
                                rhs = pb_rhs.tile([128, 258], dt.float16, tag="rhs")
                                if t % 2 == 0:
                                    nc.scalar.activation(out=rhs[:, 0:129],
                                                         in_=g[:, tl, 0:129],
                                                         func=AF.Copy,
                                                         scale=exv[:, k, 0:1])
                                    nc.vector.tensor_scalar(
                                        out=rhs[:, 129:258], in0=g[:, tl, 129:258],
                                        scalar1=exv[:, k, 1:2], scalar2=None, op0=OP.mult)
                                else:
                                    nc.vector.tensor_scalar(
                                        out=rhs[:, 0:129], in0=g[:, tl, 0:129],
                                        scalar1=exv[:, k, 0:1], scalar2=None, op0=OP.mult)
                                    nc.scalar.activation(out=rhs[:, 129:258],
                                                         in_=g[:, tl, 129:258],
                                                         func=AF.Copy,
                                                         scale=exv[:, k, 1:2])
                                nc.tensor.matmul(ps[:],
                                                 oh_sb[:, t * 128:(t + 1) * 128],
                                                 rhs[:],
                                                 start=(t == 0), stop=(t == TB - 1))

                    # epilogue: normalize, +b_gat, lrelu(0.01) -> fp16 xg
                    rec0 = pb_ep.tile([128, 1], dt.float32, tag="rec0")
                    nc.vector.reciprocal(rec0[:], ps[:, 128:129])
                    rec1 = pb_ep.tile([128, 1], dt.float32, tag="rec1")
                    nc.vector.reciprocal(rec1[:], ps[:, 257:258])
                    xg = xg_pool.tile([128, 256], dt.float16, tag=f"xg{b}")
                    nc.scalar.activation(out=xg[:, 0:128], in_=ps[:, 0:128],
                                         func=AF.Copy, scale=rec0[:])
                    nc.scalar.activation(out=xg[:, 128:256], in_=ps[:, 129:257],
                                         func=AF.Copy, scale=rec1[:])
                    nc.vector.tensor_tensor(out=xg[:], in0=xg[:], in1=bgat_sb[:], op=OP.add)
                    ng = pb_ep.tile([128, 256], dt.float16, tag="ng")
                    nc.vector.tensor_scalar(out=ng[:], in0=xg[:], scalar1=0.0,
                                            scalar2=0.01, op0=OP.min, op1=OP.mult)
                    nc.vector.scalar_tensor_tensor(out=xg[:], in0=xg[:], scalar=0.0,
                                                   in1=ng[:], op0=OP.max, op1=OP.add)
                    xgs.append(xg)

            if phases == "AB":
                for b in range(RB):
                    nc.sync.dma_start(out=outD[b * 128:(b + 1) * 128, 0:256], in_=xgs[b][:])
                raise _PhaseDone()

            # ---- phase C: dense + LN on own shard (stage-parallel across blocks) ----
            cc_sb = top.tile([4, RPAD], dt.float32, tag="cc_sb")
            with tc.tile_pool(name="pc", bufs=12) as pc, \
                 tc.tile_pool(name="pc_ps", bufs=2, space="PSUM") as pc_ps, \
                 tc.tile_pool(name="pc_mm", bufs=4, space="PSUM") as pc_mm:

                def transpose16(xin, pdim, fdim):
                    # xin fp16 [pdim, fdim] -> sbuf fp16 [fdim, pdim]
                    p = pc_ps.tile([128, 128], dt.float16, tag="tpp16", space="PSUM")
                    nc.tensor.transpose(out=p[:fdim, 0:pdim], in_=xin,
                                        identity=ident16[:pdim, :pdim])
                    s = pc.tile([fdim, pdim], dt.float16, tag=f"tt{fdim}_{pdim}")
                    nc.scalar.copy(out=s[:], in_=p[:fdim, 0:pdim])
                    return s

                def c_chain(b):
                    x0 = xgs[b]
                    xt0 = transpose16(x0[:, 0:128], 128, 128)
                    yield
                    xt1 = transpose16(x0[:, 128:256], 128, 128)
                    yield
                    pA = pc_mm.tile([128, 128], dt.float32, tag="mm", space="PSUM")
                    nc.tensor.matmul(pA[:], xt0[:], waT0[:], start=True, stop=False)
                    nc.tensor.matmul(pA[:], xt1[:], waT1[:], start=False, stop=True)
                    yield
                    x1 = yield from ln_lrelu(b, pA[:], 128, ba_sb, lnaw_sb, lnab_sb)
                    x1t = transpose16(x1[:], 128, 128)
                    yield
                    p1 = pc_mm.tile([128, 64], dt.float32, tag="mm", space="PSUM")
                    nc.tensor.matmul(p1[:], x1t[:], w1T_sb[:], start=True, stop=True)
                    yield
                    x2 = yield from ln_lrelu(b, p1[:], 64, b1_sb, ln1w_sb, ln1b_sb)
                    x2t = transpose16(x2[:], 128, 64)
                    yield
                    p2 = pc_mm.tile([128, 32], dt.float32, tag="mm", space="PSUM")
                    nc.tensor.matmul(p2[:], x2t[:], w2T_sb[:], start=True, stop=True)
                    yield
                    x3 = yield from ln_lrelu(b, p2[:], 32, b2_sb, ln2w_sb, ln2b_sb)
                    x3t = transpose16(x3[:], 128, 32)
                    yield
                    p3 = pc_mm.tile([128, 3], dt.float32, tag="mm", space="PSUM")
                    nc.tensor.matmul(p3[:], x3t[:], w3T_sb[:], start=True, stop=True)
                    yield
                    y3 = pc.tile([128, 4], dt.float32, tag="y3")
                    nc.vector.tensor_tensor(out=y3[:, 0:3], in0=p3[:], in1=b3_sb[:], op=OP.add)
                    scr3 = pc.tile([128, 3], dt.float32, tag="scr3")
                    nc.scalar.activation(out=scr3[:], in_=y3[:, 0:3], func=AF.Square,
                                         accum_out=y3[:, 3:4])
                    yield
                    h3p = pc_ps.tile([128, 128], dt.float32, tag="tpp", space="PSUM")
                    nc.tensor.transpose(out=h3p[:4, 0:128], in_=y3[:], identity=ident[:])
                    nc.scalar.copy(out=cc_sb[:, b * 128:(b + 1) * 128], in_=h3p[:4, 0:128])

                def ln_lrelu(b, xin, fdim, bias_bc, w_bc, b_bc):
                    # y = xin + bias; u = LN(y)*w + b; return lrelu001(u) fp16
                    y = pc.tile([128, fdim], dt.float32, tag=f"y{fdim}")
                    nc.vector.tensor_tensor(out=y[:], in0=xin, in1=bias_bc[:], op=OP.add)
                    yield
                    scr = pc.tile([128, fdim], dt.float32, tag=f"scr{fdim}")
                    msum = pc.tile([128, 1], dt.float32, tag="msum")
                    nc.scalar.activation(out=scr[:], in_=y[:], func=AF.Copy,
                                         accum_out=msum[:])
                    sqs = pc.tile([128, 1], dt.float32, tag="sqs")
                    nc.scalar.activation(out=scr[:], in_=y[:], func=AF.Square,
                                         accum_out=sqs[:])
                    yield
                    mean = pc.tile([128, 1], dt.float32, tag="mean")
                    nc.vector.tensor_scalar(out=mean[:], in0=msum[:], scalar1=1.0 / fdim,
                                            scalar2=None, op0=OP.mult)
                    var = pc.tile([128, 1], dt.float32, tag="var")
                    nc.vector.tensor_scalar(out=var[:], in0=sqs[:], scalar1=1.0 / fdim,
                                            scalar2=None, op0=OP.mult)
                    m2 = pc.tile([128, 1], dt.float32, tag="m2")
                    nc.vector.tensor_scalar(out=m2[:], in0=mean[:], scalar1=mean[:, 0:1],
                                            scalar2=None, op0=OP.mult)
                    nc.vector.tensor_tensor(out=var[:], in0=var[:], in1=m2[:], op=OP.subtract)
                    sd = pc.tile([128, 1], dt.float32, tag="sd")
                    nc.scalar.activation(out=sd[:], in_=var[:], func=AF.Sqrt,
                                         bias=eps_col[:, 0:1])
                    rstd = pc.tile([128, 1], dt.float32, tag="rstd")
                    nc.vector.reciprocal(rstd[:], sd[:])
                    yield
                    u = pc.tile([128, fdim], dt.float32, tag=f"u{fdim}")
                    nc.vector.scalar_tensor_tensor(out=u[:], in0=y[:], scalar=mean[:, 0:1],
                                                   in1=w_bc[:], op0=OP.subtract, op1=OP.mult)
                    nc.vector.scalar_tensor_tensor(out=u[:], in0=u[:], scalar=rstd[:, 0:1],
                                                   in1=b_bc[:], op0=OP.mult, op1=OP.add)
                    yield
                    ngt = pc.tile([128, fdim], dt.float32, tag=f"ng{fdim}")
                    nc.vector.tensor_scalar(out=ngt[:], in0=u[:], scalar1=0.0,
                                            scalar2=0.01, op0=OP.min, op1=OP.mult)
                    u16 = pc.tile([128, fdim], dt.float16, tag=f"u16_{fdim}")
                    nc.vector.scalar_tensor_tensor(out=u16[:], in0=u[:], scalar=0.0,
                                                   in1=ngt[:], op0=OP.max, op1=OP.add)
                    yield
                    return u16

                gens = [c_chain(b) for b in range(RB)]

                def run_gens(idxs):
                    done = {b: False for b in idxs}
                    while not all(done.values()):
                        for b in idxs:
                            if not done[b]:
                                try:
                                    next(gens[b])
                                except StopIteration:
                                    done[b] = True

                run_gens(range(RB // 2))
                nc.sync.dma_start(out=cc_inA[:], in_=cc_sb[:, 0:HALF])
                nc.gpsimd.collective_compute(
                    "AllGather", OP.bypass, replica_groups=[list(range(CORES))],
                    ins=[cc_inA[:].opt()], outs=[cc_outA[:].opt()])
                run_gens(range(RB // 2, RB))
                nc.sync.dma_start(out=cc_inB[:], in_=cc_sb[:, HALF:RPAD])
                nc.gpsimd.collective_compute(
                    "AllGather", OP.bypass, replica_groups=[list(range(CORES))],
                    ins=[cc_inB[:].opt()], outs=[cc_outB[:].opt()])

            if phases == "ABC":
                dbg16 = top.tile([4, RPAD], dt.float16, tag="dbgc")
                nc.vector.tensor_copy(out=dbg16[:], in_=cc_sb[:])
                nc.sync.dma_start(out=outD[0:4, 0:RPAD], in_=dbg16[:])
                raise _PhaseDone()

            # ---- phase D: build split-fp16 cdist operands ----
            # lhsT16 rows: [-2a(3) | -2a(3) | -2b(3) | sqhi | sqlo | 1 | 1]
            # (compute in partition-0 tiles, assemble via sbuf-to-sbuf DMA)
            lhsT16 = top.tile([13, RPAD], dt.float16, tag="lhsT16")
            pd = est.enter_context(tc.tile_pool(name="pd", bufs=1))
            a_own = pd.tile([4, RPAD], dt.float16, tag="a_own")
            nc.vector.tensor_copy(out=a_own[:], in_=cc_sb[:])
            b_own = pd.tile([4, RPAD], dt.float16, tag="b_own")
            nc.vector.tensor_tensor(out=b_own[:], in0=cc_sb[:], in1=a_own[:],
                                    op=OP.subtract)
            na4 = pd.tile([4, RPAD], dt.float16, tag="na4")
            nc.scalar.activation(out=na4[:], in_=a_own[:], func=AF.Copy, scale=-2.0)
            nb4 = pd.tile([4, RPAD], dt.float16, tag="nb4")
            nc.scalar.activation(out=nb4[:], in_=b_own[:], func=AF.Copy, scale=-2.0)
            ones_r = pd.tile([2, RPAD], dt.float16, tag="ones_r")
            nc.vector.memset(ones_r[:], 1.0)
            nc.sync.dma_start(out=lhsT16[0:3, :], in_=na4[0:3, :])
            nc.sync.dma_start(out=lhsT16[3:6, :], in_=na4[0:3, :])
            nc.sync.dma_start(out=lhsT16[6:9, :], in_=nb4[0:3, :])
            nc.sync.dma_start(out=lhsT16[9:10, :], in_=a_own[3:4, :])
            nc.sync.dma_start(out=lhsT16[10:11, :], in_=b_own[3:4, :])
            nc.sync.dma_start(out=lhsT16[11:13, :], in_=ones_r[:])

            # rhs16 rows: [a(3) | b(3) | a(3) | 1 | 1 | sqhi | sqlo]
            rhs_f = pd.tile([4, NCOL], dt.float32, tag="rhs_f")
            nc.vector.memset(rhs_f[:, N:NCOL], 0.0)
            for s in range(CORES):
                c0 = s * SHARD
                nc.sync.dma_start(out=rhs_f[0:4, c0:c0 + HALF],
                                  in_=cc_outA[:][s])
                nc.sync.dma_start(out=rhs_f[0:4, c0 + HALF:c0 + SHARD],
                                  in_=cc_outB[:][s, 0:4, 0:SHARD - HALF])
            a16 = pd.tile([4, NCOL], dt.float16, tag="a16")
            nc.vector.tensor_copy(out=a16[:], in_=rhs_f[:])                  # a | sqhi
            b16 = pd.tile([4, NCOL], dt.float16, tag="b16")
            nc.vector.tensor_tensor(out=b16[:], in0=rhs_f[:], in1=a16[:],
                                    op=OP.subtract)                          # b | sqlo
            ones_n = pd.tile([2, NCOL], dt.float16, tag="ones_n")
            nc.vector.memset(ones_n[:], 1.0)
            rhs16 = top.tile([13, NCOL], dt.float16, tag="rhs16")
            nc.sync.dma_start(out=rhs16[0:3, :], in_=a16[0:3, :])
            nc.sync.dma_start(out=rhs16[3:6, :], in_=b16[0:3, :])
            nc.sync.dma_start(out=rhs16[6:9, :], in_=a16[0:3, :])
            nc.sync.dma_start(out=rhs16[9:11, :], in_=ones_n[:])
            nc.sync.dma_start(out=rhs16[11:12, :], in_=a16[3:4, :])
            nc.sync.dma_start(out=rhs16[12:13, :], in_=b16[3:4, :])

            if phases == "ABCD":
                nc.sync.dma_start(out=outD[0:13, 0:NCOL], in_=rhs16[:])
                raise _PhaseDone()

            # ---- phase E: cdist row-block x col-chunk (split-fp16 matmul, fp16 out) ----
            MRG = 4   # psum chunks merged into one output tile/DMA
            with tc.tile_pool(name="pe_d", bufs=3) as pe_d, \
                 tc.tile_pool(name="pe_d2", bufs=3) as pe_d2, \
                 tc.tile_pool(name="pe_ps", bufs=6, space="PSUM") as pe_ps:
                for rb in range(RB):
                    for mg in range(NCOL // (CCH * MRG)):
                        d2t = pe_d2.tile([128, CCH * MRG], dt.float16, tag="d2t")
                        for k in range(MRG):
                            ch = mg * MRG + k
                            dp = pe_ps.tile([128, CCH], dt.float32, tag="dp", space="PSUM")
                            nc.tensor.matmul(
                                dp[:], lhsT16[:, rb * 128:(rb + 1) * 128],
                                rhs16[:, ch * CCH:(ch + 1) * CCH],
                                start=True, stop=True)
                            nc.scalar.activation(out=d2t[:, k * CCH:(k + 1) * CCH],
                                                 in_=dp[:], func=AF.Sqrt,
                                                 bias=eps4_col[:, 0:1])
                        nc.sync.dma_start(
                            out=outD[rb * 128:(rb + 1) * 128,
                                     mg * CCH * MRG:(mg + 1) * CCH * MRG],
                            in_=d2t[:])

    except _PhaseDone:
        pass
    _tc_cm.__exit__(None, None, None)
    nc.compile()
    return nc


def _prep_host(x, edge_index):
    xp = np.zeros((NPAD, FIN), np.float32)
    xp[:N] = np.asarray(x, np.float32)
    xp16 = np.ascontiguousarray(xp.T.astype(np.float16))  # [256, NPAD]

    ei = np.asarray(edge_index)
    src = ei[0].astype(np.int64)
    dst = ei[1].astype(np.int64)

    core = dst // SHARD
    per_core = []
    max_tiles = 0
    for c in range(CORES):
        sel = core == c
        s_c = src[sel]
        d_c = dst[sel]
        loc = d_c - c * SHARD
        blk = loc // 128
        dl = loc - blk * 128
        blocks = []
        for b in range(RB):
            m = blk == b
            blocks.append((s_c[m], dl[m]))
            # tile 0 holds the block's self-loop edges; rest start at tile 1
            max_tiles = max(max_tiles, 1 + (len(blocks[-1][0]) + 127) // 128)
        per_core.append(blocks)

    TB = GC * ((max_tiles + GC - 1) // GC)
    S16 = 8 * TB
    NE = TB * 128

    hidx = np.zeros((CORES, RB, 16, S16), np.int16)
    ohtab = np.zeros((CORES, RB, 128, NE), np.float16)
    ohTtab = np.zeros((CORES, RB, 128, NE), np.float16)
    for c in range(CORES):
        for b in range(RB):
            # tile 0: self loops (edge at partition p has src=dst=block row p)
            rows = c * SHARD + b * 128 + np.arange(128)
            real = rows < N
            crows = np.minimum(rows, N - 1)
            jr = np.arange(128)
            hidx[c, b, jr % 16, jr // 16] = crows.astype(np.int16)
            pr = jr[real]
            ohtab[c, b, pr, pr] = 1.0
            ohTtab[c, b, pr, pr] = 1.0
            # remaining edges from tile 1 on
            s_b, dl_b = per_core[c][b]
            n = len(s_b)
            js = 128 + np.arange(n)
            hidx[c, b, js % 16, js // 16] = s_b.astype(np.int16)
            p = js % 128          # edge partition
            t = js // 128         # edge tile (>= 1)
            sl = dl_b.astype(np.int64)
            ohtab[c, b, p, t * 128 + sl] = 1.0
            ohTtab[c, b, sl, t * 128 + p] = 1.0
    hidx = np.tile(hidx, (1, 1, 8, 1))
    return xp16, hidx, ohtab, ohTtab, TB


def build_in_maps(inputs):
    xp16, hidx, ohtab, ohTtab, TB = _prep_host(inputs["x"], inputs["edge_index"])

    def bc(vec, n, f16=False):
        v = np.asarray(vec, np.float32).reshape(1, n)
        out = np.ascontiguousarray(np.broadcast_to(v, (128, n)).copy())
        return out.astype(np.float16) if f16 else out

    # rhsA: [256 (xfeat, 2 chunks of 128), 262] fp16
    # cols: 0:128 WgT head0 | 128 zero | 129:257 WgT head1 | 257 zero | 258:262 wtil
    Wg = np.asarray(inputs["W_gat"], np.float32)       # [256, 256] rows = H*F out
    att_src = np.asarray(inputs["att_src"], np.float32)  # [2, 128]
    att_dst = np.asarray(inputs["att_dst"], np.float32)
    rhsA = np.zeros((256, 262), np.float32)
    rhsA[:, 0:128] = Wg[0:128, :].T
    rhsA[:, 129:257] = Wg[128:256, :].T
    rhsA[:, 258] = Wg[0:128, :].T @ att_src[0]
    rhsA[:, 259] = Wg[128:256, :].T @ att_src[1]
    rhsA[:, 260] = Wg[0:128, :].T @ att_dst[0]
    rhsA[:, 261] = Wg[128:256, :].T @ att_dst[1]
    rhsA16 = rhsA.astype(np.float16).reshape(2, 128, 262)

    Wa = np.asarray(inputs["Wa"], np.float32)  # [128, 256]
    waT16 = np.ascontiguousarray(Wa.T.astype(np.float16)).reshape(2, 128, 128)
    w1T16 = np.ascontiguousarray(np.asarray(inputs["W1"], np.float32).T.astype(np.float16))
    w2T16 = np.ascontiguousarray(np.asarray(inputs["W2"], np.float32).T.astype(np.float16))
    w3T16 = np.ascontiguousarray(np.asarray(inputs["W3"], np.float32).T.astype(np.float16))

    shared = {
        "xt16": xp16,
        "rhsA": np.ascontiguousarray(rhsA16),
        "waT": waT16, "w1T": w1T16, "w2T": w2T16, "w3T": w3T16,
        "bgat_bc": bc(inputs["b_gat"], 256, f16=True),
        "ba_bc": bc(inputs["ba"], 128),
        "lnaw_bc": bc(inputs["lna_w"], 128), "lnab_bc": bc(inputs["lna_b"], 128),
        "b1_bc": bc(inputs["b1"], 64),
        "ln1w_bc": bc(inputs["ln1_w"], 64), "ln1b_bc": bc(inputs["ln1_b"], 64),
        "b2_bc": bc(inputs["b2"], 32),
        "ln2w_bc": bc(inputs["ln2_w"], 32), "ln2b_bc": bc(inputs["ln2_b"], 32),
        "b3_bc": bc(inputs["b3"], 3),
    }
    in_maps = [
        {**shared, "hidx": np.ascontiguousarray(hidx[c]),
         "ohtab": np.ascontiguousarray(ohtab[c]),
         "ohTtab": np.ascontiguousarray(ohTtab[c])}
        for c in range(CORES)
    ]
    return in_maps, TB


def kernel(**inputs):
    in_maps, TB = build_in_maps(inputs)

    import os
    phases = os.environ.get("K_PHASES", "ABCDE")
    key = (TB, phases)
    if key not in _BUILD_CACHE:
        _BUILD_CACHE[key] = _build(TB, phases)
    nc = _BUILD_CACHE[key]
    res = run_bass_kernel_spmd(nc, in_maps, core_ids=list(range(CORES)))
    global _LAST_RESULTS
    _LAST_RESULTS = res.results
    out = np.empty((N, N), np.float32)
    for c in range(CORES):
        out[c * SHARD:(c + 1) * SHARD, :] = \
            res.results[c]["outD"][:SHARD, :N].astype(np.float32)
    return out


# revision 20
# speedup vs baseline: 2.3725x; 1.0396x over previous
"""GAT (2-head) + 3x dense/LayerNorm + pairwise-distance kernel for 8 TRN2 NeuronCores.

Strategy: dst-sharded edge processing (one-hot matmul aggregation), replicated
small dense weights, row-block-sharded NxN cdist output.

v3: fp16 htable/gather rows, host-precomputed one-hot tables (no on-device
is_eq), 2-queue GC=12 gathers, whole-x preload, split-fp16 cdist matmuls,
fp16 output.
"""
import sys

import numpy as np

# Environment bootstrap (harness may run from a bare directory).
for _p in ("/root/.axon_site", "/root/.axon_site/_ro/trn_rl_repo",
           "/root/.axon_site/_ro/pypackages", "/opt/trn_rl_repo"):
    if _p not in sys.path:
        sys.path.append(_p)

import concourse.bass as bass
import concourse.bacc as bacc
import concourse.mybir as mybir
import concourse.tile as tile
from concourse.masks import make_identity
from concourse.bass_utils import run_bass_kernel_spmd

dt = mybir.dt
OP = mybir.AluOpType
AF = mybir.ActivationFunctionType

N = 10000
NPAD = 10112          # 79 * 128
NB = 79               # node blocks (phase A)
FIN = 256
F = 128               # per-head GAT dim
H = 2
HROW = 384            # htable row fp16 elems (768B, multiple of 256B)
CORES = 8
SHARD = 1250          # dst nodes per core
RB = 10               # dst blocks per core
RPAD = 1280
CCH = 512             # cdist column chunk
NCOL = 10240          # padded output columns
EPS = 1e-5

_BUILD_CACHE = {}
_LAST_RESULTS = None


GC = 6   # tile groups per dma_gather call (768 idxs = 48 descs/engine)
SUB = 3  # tiles per a_d/exp subchunk


def _build(TB, phases="ABCDE"):
    """Build the 8-core SPMD program. TB = gather tile groups per dst block (mult of GC)."""
    assert TB % GC == 0
    NCH = TB // GC
    S16 = 8 * TB          # idx columns ([128, S16] wrapped int16)

    nc = bacc.Bacc("TRN2", target_bir_lowering=False, debug=False,
                   num_devices=CORES, num_swdge_queues=2)

    def din(name, shape, d=dt.float32):
        return nc.dram_tensor(name, shape, d, kind="ExternalInput").ap()

    xt16 = din("xt16", [FIN, NPAD], dt.float16)
    rhsA = din("rhsA", [2, 128, 262], dt.float16)
    waT = din("waT", [2, 128, 128], dt.float16)
    w1T = din("w1T", [128, 64], dt.float16)
    w2T = din("w2T", [64, 32], dt.float16)
    w3T = din("w3T", [32, 3], dt.float16)
    bgat_bc = din("bgat_bc", [128, 256], dt.float16)
    ba_bc = din("ba_bc", [128, 128])
    lnaw_bc = din("lnaw_bc", [128, 128])
    lnab_bc = din("lnab_bc", [128, 128])
    b1_bc = din("b1_bc", [128, 64])
    ln1w_bc = din("ln1w_bc", [128, 64])
    ln1b_bc = din("ln1b_bc", [128, 64])
    b2_bc = din("b2_bc", [128, 32])
    ln2w_bc = din("ln2w_bc", [128, 32])
    ln2b_bc = din("ln2b_bc", [128, 32])
    b3_bc = din("b3_bc", [128, 3])
    hidxI = din("hidxI", [RB, 128, TB], dt.int32)
    ohtab = din("ohtab", [RB, 128, TB * 128], dt.float16)    # [edge_p, t*128+slot]
    ohTtab = din("ohTtab", [RB, 128, TB * 128], dt.float16)  # [slot_p, t*128+edge]
    outD = nc.dram_tensor("outD", [RPAD, NCOL], dt.float16, kind="ExternalOutput").ap()

    class _PhaseDone(Exception):
        pass

    import contextlib
    try:
        _tc_cm = tile.TileContext(nc)
        tc = _tc_cm.__enter__()
        est = contextlib.ExitStack()
        with est:
            top = est.enter_context(tc.tile_pool(name="top", bufs=1))
            dram = est.enter_context(tc.tile_pool(name="dram", bufs=1, space="DRAM"))

            htable = dram.tile([NPAD, HROW], dt.float16, tag="htable")
            HALF = RPAD // 2
            cc_inA = dram.tile([4, HALF], dt.float32, tag="cc_inA")
            cc_outA = dram.tile([CORES, 4, HALF], dt.float32, tag="cc_outA")
            cc_inB = dram.tile([4, HALF], dt.float32, tag="cc_inB")
            cc_outB = dram.tile([CORES, 4, HALF], dt.float32, tag="cc_outB")

            ident = top.tile([128, 128], dt.float32, tag="ident")
            make_identity(nc, ident[:])
            ident16 = top.tile([128, 128], dt.float16, tag="ident16")
            nc.vector.tensor_copy(out=ident16[:], in_=ident[:])
            eps_col = top.tile([128, 1], dt.float32, tag="eps_col")
            nc.vector.memset(eps_col[:], EPS)
            eps4_col = top.tile([128, 1], dt.float32, tag="eps4_col")
            nc.vector.memset(eps4_col[:], 1e-4)

            # ---- load replicated weights / biases into SBUF ----
            def ldw(name, ap, shape, d=dt.float32):
                t = top.tile(shape, d, tag=name)
                nc.sync.dma_start(out=t[:], in_=ap)
                return t

            rhsA0 = ldw("rhsA0", rhsA[0], [128, 262], dt.float16)
            rhsA1 = ldw("rhsA1", rhsA[1], [128, 262], dt.float16)
            waT0 = ldw("waT0", waT[0], [128, 128], dt.float16)
            waT1 = ldw("waT1", waT[1], [128, 128], dt.float16)
            w1T_sb = ldw("w1T_sb", w1T[:], [128, 64], dt.float16)
            w2T_sb = ldw("w2T_sb", w2T[:], [64, 32], dt.float16)
            w3T_sb = ldw("w3T_sb", w3T[:], [32, 3], dt.float16)
            bgat_sb = ldw("bgat_sb", bgat_bc[:], [128, 256], dt.float16)
            ba_sb = ldw("ba_sb", ba_bc[:], [128, 128])
            lnaw_sb = ldw("lnaw_sb", lnaw_bc[:], [128, 128])
            lnab_sb = ldw("lnab_sb", lnab_bc[:], [128, 128])
            b1_sb = ldw("b1_sb", b1_bc[:], [128, 64])
            ln1w_sb = ldw("ln1w_sb", ln1w_bc[:], [128, 64])
            ln1b_sb = ldw("ln1b_sb", ln1b_bc[:], [128, 64])
            b2_sb = ldw("b2_sb", b2_bc[:], [128, 32])
            ln2w_sb = ldw("ln2w_sb", ln2w_bc[:], [128, 32])
            ln2b_sb = ldw("ln2b_sb", ln2b_bc[:], [128, 32])
            b3_sb = ldw("b3_sb", b3_bc[:], [128, 3])

            # ---- phase A: htable rows [h0 | 1 | h1 | 1 | a_s(2) a_d(2)] fp16 ----
            with tc.tile_pool(name="pa", bufs=1) as pa, \
                 tc.tile_pool(name="pa_ht", bufs=4) as pa_ht, \
                 tc.tile_pool(name="pa_h", bufs=4, space="PSUM") as pa_h:
                xta = pa.tile([128, NPAD], dt.float16, tag="xta")
                xtb = pa.tile([128, NPAD], dt.float16, tag="xtb")
                nc.sync.dma_start(out=xta[:], in_=xt16[0:128, :])
                nc.sync.dma_start(out=xtb[:], in_=xt16[128:256, :])
                for i in range(NB):
                    r0 = i * 128
                    hp = pa_h.tile([128, 262], dt.float32, tag="hp", space="PSUM")
                    nc.tensor.matmul(hp[:], xta[:, r0:r0 + 128], rhsA0[:],
                                     start=True, stop=False)
                    nc.tensor.matmul(hp[:], xtb[:, r0:r0 + 128], rhsA1[:],
                                     start=False, stop=True)
                    ht = pa_ht.tile([128, 262], dt.float16, tag="ht")
                    if i % 2 == 0:
                        nc.scalar.copy(out=ht[:], in_=hp[:])
                    else:
                        nc.vector.tensor_copy(out=ht[:], in_=hp[:])
                    nc.gpsimd.memset(ht[:, 128:129], 1.0)
                    nc.gpsimd.memset(ht[:, 257:258], 1.0)
                    nc.sync.dma_start(out=htable[r0:r0 + 128, 0:262], in_=ht[:])

            if phases == "A":
                dbg = top.tile([128, 262], dt.float16, tag="dbgA")
                for i in range(RB):
                    nc.sync.dma_start(out=dbg[:], in_=htable[i * 128:(i + 1) * 128, 0:262])
                    nc.sync.dma_start(out=outD[i * 128:(i + 1) * 128, 0:262], in_=dbg[:])
                raise _PhaseDone()

            # ---- phase B: GAT aggregation per dst block ----
            xg_pool = est.enter_context(tc.tile_pool(name="xg", bufs=1))
            xgs = []
            with tc.tile_pool(name="pb_idx", bufs=2) as pb_idx, \
                 tc.tile_pool(name="pb_oht", bufs=3) as pb_oht, \
                 tc.tile_pool(name="pb_g", bufs=4) as pb_g, \
                 tc.tile_pool(name="pb_ad", bufs=2) as pb_ad, \
                 tc.tile_pool(name="pb_ex", bufs=4) as pb_ex, \
                 tc.tile_pool(name="pb_rhs", bufs=6) as pb_rhs, \
                 tc.tile_pool(name="pb_ep", bufs=2) as pb_ep, \
                 tc.tile_pool(name="pb_ps", bufs=2, space="PSUM") as pb_ps, \
                 tc.tile_pool(name="pb_adp", bufs=4, space="PSUM") as pb_adp:
                for _slot in range(4):
                    gz = pb_g.tile([128, GC, HROW], dt.float16, tag="g")
                    nc.vector.memset(gz[:], 0.0)
                for b in range(RB):
                    hix = pb_idx.tile([128, TB], dt.int32, tag="hix")
                    nc.sync.dma_start(out=hix[:], in_=hidxI[b])
                    oh_sb = pb_oht.tile([128, TB * 128], dt.float16, tag="oh_sb")
                    nc.sync.dma_start(out=oh_sb[:], in_=ohtab[b])
                    ohT_sb = pb_oht.tile([128, TB * 128], dt.float16, tag="ohT_sb")
                    nc.sync.dma_start(out=ohT_sb[:], in_=ohTtab[b])

                    adblk = pb_ad.tile([128, 2], dt.float16, tag="adblk")
                    ps = pb_ps.tile([128, 258], dt.float32, tag="ps", space="PSUM")

                    for c in range(NCH):
                        g = pb_g.tile([128, GC, HROW], dt.float16, tag="g")
                        for tl in range(GC):
                            nc.gpsimd.indirect_dma_start(
                                out=g[:, tl, :], out_offset=None,
                                in_=htable[:],
                                in_offset=bass.IndirectOffsetOnAxis(
                                    ap=hix[:, c * GC + tl:c * GC + tl + 1], axis=0))
                        if c == 0:
                            # tile 0 rows are this block's own dst rows (self loops)
                            nc.vector.tensor_copy(out=adblk[:], in_=g[:, 0, 260:262])
                        for hs in range(GC // SUB):
                            t0 = c * GC + hs * SUB
                            adps = pb_adp.tile([128, SUB, 2], dt.float32, tag="adps",
                                               space="PSUM")
                            for k in range(SUB):
                                t = t0 + k
                                nc.tensor.matmul(adps[:, k, :],
                                                 ohT_sb[:, t * 128:(t + 1) * 128],
                                                 adblk[:], start=True, stop=True)
                            # e = a_s[src] + a_d[dst]; exv = max(exp(e), exp(0.2e))
                            exr = pb_ex.tile([128, SUB, 2], dt.float32, tag="exr")
                            nc.vector.tensor_tensor(
                                out=exr[:], in0=adps[:],
                                in1=g[:, hs * SUB:(hs + 1) * SUB, 258:260], op=OP.add)
                            exn = pb_ex.tile([128, SUB, 2], dt.float32, tag="exn")
                            nc.scalar.activation(out=exn[:], in_=exr[:], func=AF.Exp,
                                                 scale=0.2)
                            exp_ = pb_ex.tile([128, SUB, 2], dt.float32, tag="exp_")
                            nc.scalar.activation(out=exp_[:], in_=exr[:], func=AF.Exp)
                            exv = pb_ex.tile([128, SUB, 2], dt.float32, tag="exv")
                            nc.vector.tensor_tensor(out=exv[:], in0=exp_[:], in1=exn[:],
                                                    op=OP.max)
                            for k in range(SUB):
                                t = t0 + k
                                tl = hs * SUB + k
                                rhs = pb_rhs.tile([128, 258], dt.float16, tag="rhs")
                                if t % 2 == 0:
                                    nc.scalar.activation(out=rhs[:, 0:129],
                                                         in_=g[:, tl, 0:129],
                                                         func=AF.Copy,
                                                         scale=exv[:, k, 0:1])
                                    nc.vector.tensor_scalar(
                                        out=rhs[:, 129:258], in0=g[:, tl, 129:258],
                                        scalar1=exv[:, k, 1:2], scalar2=None, op0=OP.mult)
                                else:
                                    nc.vector.tensor_scalar(
                                        out=rhs[:, 0:129], in0=g[:, tl, 0:129],
                                        scalar1=exv[:, k, 0:1], scalar2=None, op0=OP.mult)
                                    nc.scalar.activation(out=rhs[:, 129:258],
                                                         in_=g[:, tl, 129:258],
                                                         func=AF.Copy,
                                                         scale=exv[:, k, 1:2])
                                nc.tensor.matmul(ps[:],
                                                 oh_sb[:, t * 128:(t + 1) * 128],
                                                 rhs[:],
                                                 start=(t == 0), stop=(t == TB - 1))

                    # epilogue: normalize, +b_gat, lrelu(0.01) -> fp16 xg
                    rec0 = pb_ep.tile([128, 1], dt.float32, tag="rec0")
                    nc.vector.reciprocal(rec0[:], ps[:, 128:129])
                    rec1 = pb_ep.tile([128, 1], dt.float32, tag="rec1")
                    nc.vector.reciprocal(rec1[:], ps[:, 257:258])
                    xg = xg_pool.tile([128, 256], dt.float16, tag=f"xg{b}")
                    nc.scalar.activation(out=xg[:, 0:128], in_=ps[:, 0:128],
                                         func=AF.Copy, scale=rec0[:])
                    nc.scalar.activation(out=xg[:, 128:256], in_=ps[:, 129:257],
                                         func=AF.Copy, scale=rec1[:])
                    nc.vector.tensor_tensor(out=xg[:], in0=xg[:], in1=bgat_sb[:], op=OP.add)
                    ng = pb_ep.tile([128, 256], dt.float16, tag="ng")
                    nc.vector.tensor_scalar(out=ng[:], in0=xg[:], scalar1=0.0,
                                            scalar2=0.01, op0=OP.min, op1=OP.mult)
                    nc.vector.scalar_tensor_tensor(out=xg[:], in0=xg[:], scalar=0.0,
                                                   in1=ng[:], op0=OP.max, op1=OP.add)
                    xgs.append(xg)

            if phases == "AB":
                for b in range(RB):
                    nc.sync.dma_start(out=outD[b * 128:(b + 1) * 128, 0:256], in_=xgs[b][:])
                raise _PhaseDone()

            # ---- phase C: dense + LN on own shard (stage-parallel across blocks) ----
            cc_sb = top.tile([4, RPAD], dt.float32, tag="cc_sb")
            with tc.tile_pool(name="pc", bufs=12) as pc, \
                 tc.tile_pool(name="pc_ps", bufs=2, space="PSUM") as pc_ps, \
                 tc.tile_pool(name="pc_mm", bufs=4, space="PSUM") as pc_mm:

                def transpose16(xin, pdim, fdim):
                    # xin fp16 [pdim, fdim] -> sbuf fp16 [fdim, pdim]
                    p = pc_ps.tile([128, 128], dt.float16, tag="tpp16", space="PSUM")
                    nc.tensor.transpose(out=p[:fdim, 0:pdim], in_=xin,
                                        identity=ident16[:pdim, :pdim])
                    s = pc.tile([fdim, pdim], dt.float16, tag=f"tt{fdim}_{pdim}")
                    nc.scalar.copy(out=s[:], in_=p[:fdim, 0:pdim])
                    return s

                def c_chain(b):
                    x0 = xgs[b]
                    xt0 = transpose16(x0[:, 0:128], 128, 128)
                    yield
                    xt1 = transpose16(x0[:, 128:256], 128, 128)
                    yield
                    pA = pc_mm.tile([128, 128], dt.float32, tag="mm", space="PSUM")
                    nc.tensor.matmul(pA[:], xt0[:], waT0[:], start=True, stop=False)
                    nc.tensor.matmul(pA[:], xt1[:], waT1[:], start=False, stop=True)
                    yield
                    x1 = yield from ln_lrelu(b, pA[:], 128, ba_sb, lnaw_sb, lnab_sb)
                    x1t = transpose16(x1[:], 128, 128)
                    yield
                    p1 = pc_mm.tile([128, 64], dt.float32, tag="mm", space="PSUM")
                    nc.tensor.matmul(p1[:], x1t[:], w1T_sb[:], start=True, stop=True)
                    yield
                    x2 = yield from ln_lrelu(b, p1[:], 64, b1_sb, ln1w_sb, ln1b_sb)
                    x2t = transpose16(x2[:], 128, 64)
                    yield
                    p2 = pc_mm.tile([128, 32], dt.float32, tag="mm", space="PSUM")
                    nc.tensor.matmul(p2[:], x2t[:], w2T_sb[:], start=True, stop=True)
                    yield
                    x3 = yield from ln_lrelu(b, p2[:], 32, b2_sb, ln2w_sb, ln2b_sb)
                    x3t = transpose16(x3[:], 128, 32)
                    yield
                    p3 = pc_mm.tile([128, 3], dt.float32, tag="mm", space="PSUM")
                    nc.tensor.matmul(p3[:], x3t[:], w3T_sb[:], start=True, stop=True)
                    yield
                    y3 = pc.tile([128, 4], dt.float32, tag="y3")
                    nc.vector.tensor_tensor(out=y3[:, 0:3], in0=p3[:], in1=b3_sb[:], op=OP.add)
                    scr3 = pc.tile([128, 3], dt.float32, tag="scr3")
                    nc.scalar.activation(out=scr3[:], in_=y3[:, 0:3], func=AF.Square,
                                         accum_out=y3[:, 3:4])
                    yield
                    h3p = pc_ps.tile([128, 128], dt.float32, tag="tpp", space="PSUM")
                    nc.tensor.transpose(out=h3p[:4, 0:128], in_=y3[:], identity=ident[:])
                    nc.scalar.copy(out=cc_sb[:, b * 128:(b + 1) * 128], in_=h3p[:4, 0:128])

                def ln_lrelu(b, xin, fdim, bias_bc, w_bc, b_bc):
                    # y = xin + bias; u = LN(y)*w + b; return lrelu001(u) fp16
                    y = pc.tile([128, fdim], dt.float32, tag=f"y{fdim}")
                    nc.vector.tensor_tensor(out=y[:], in0=xin, in1=bias_bc[:], op=OP.add)
                    yield
                    scr = pc.tile([128, fdim], dt.float32, tag=f"scr{fdim}")
                    msum = pc.tile([128, 1], dt.float32, tag="msum")
                    nc.scalar.activation(out=scr[:], in_=y[:], func=AF.Copy,
                                         accum_out=msum[:])
                    sqs = pc.tile([128, 1], dt.float32, tag="sqs")
                    nc.scalar.activation(out=scr[:], in_=y[:], func=AF.Square,
                                         accum_out=sqs[:])
                    yield
                    mean = pc.tile([128, 1], dt.float32, tag="mean")
                    nc.vector.tensor_scalar(out=mean[:], in0=msum[:], scalar1=1.0 / fdim,
                                            scalar2=None, op0=OP.mult)
                    var = pc.tile([128, 1], dt.float32, tag="var")
                    nc.vector.tensor_scalar(out=var[:], in0=sqs[:], scalar1=1.0 / fdim,
                                            scalar2=None, op0=OP.mult)
                    m2 = pc.tile([128, 1], dt.float32, tag="m2")
                    nc.vector.tensor_scalar(out=m2[:], in0=mean[:], scalar1=mean[:, 0:1],
                                            scalar2=None, op0=OP.mult)
                    nc.vector.tensor_tensor(out=var[:], in0=var[:], in1=m2[:], op=OP.subtract)
                    sd = pc.tile([128, 1], dt.float32, tag="sd")
                    nc.scalar.activation(out=sd[:], in_=var[:], func=AF.Sqrt,
                                         bias=eps_col[:, 0:1])
                    rstd = pc.tile([128, 1], dt.float32, tag="rstd")
                    nc.vector.reciprocal(rstd[:], sd[:])
                    yield
                    u = pc.tile([128, fdim], dt.float32, tag=f"u{fdim}")
                    nc.vector.scalar_tensor_tensor(out=u[:], in0=y[:], scalar=mean[:, 0:1],
                                                   in1=w_bc[:], op0=OP.subtract, op1=OP.mult)
                    nc.vector.scalar_tensor_tensor(out=u[:], in0=u[:], scalar=rstd[:, 0:1],
                                                   in1=b_bc[:], op0=OP.mult, op1=OP.add)
                    yield
                    ngt = pc.tile([128, fdim], dt.float32, tag=f"ng{fdim}")
                    nc.vector.tensor_scalar(out=ngt[:], in0=u[:], scalar1=0.0,
                                            scalar2=0.01, op0=OP.min, op1=OP.mult)
                    u16 = pc.tile([128, fdim], dt.float16, tag=f"u16_{fdim}")
                    nc.vector.scalar_tensor_tensor(out=u16[:], in0=u[:], scalar=0.0,
                                                   in1=ngt[:], op0=OP.max, op1=OP.add)
                    yield
                    return u16

                gens = [c_chain(b) for b in range(RB)]

                def run_gens(idxs):
                    done = {b: False for b in idxs}
                    while not all(done.values()):
                        for b in idxs:
                            if not done[b]:
                                try:
                                    next(gens[b])
                                except StopIteration:
                                    done[b] = True

                run_gens(range(RB // 2))
                nc.sync.dma_start(out=cc_inA[:], in_=cc_sb[:, 0:HALF])
                nc.gpsimd.collective_compute(
                    "AllGather", OP.bypass, replica_groups=[list(range(CORES))],
                    ins=[cc_inA[:].opt()], outs=[cc_outA[:].opt()])
                run_gens(range(RB // 2, RB))
                nc.sync.dma_start(out=cc_inB[:], in_=cc_sb[:, HALF:RPAD])
                nc.gpsimd.collective_compute(
                    "AllGather", OP.bypass, replica_groups=[list(range(CORES))],
                    ins=[cc_inB[:].opt()], outs=[cc_outB[:].opt()])

            if phases == "ABC":
                dbg16 = top.tile([4, RPAD], dt.float16, tag="dbgc")
                nc.vector.tensor_copy(out=dbg16[:], in_=cc_sb[:])
                nc.sync.dma_start(out=outD[0:4, 0:RPAD], in_=dbg16[:])
                raise _PhaseDone()

            # ---- phase D: build split-fp16 cdist operands ----
            # lhsT16 rows: [-2a(3) | -2a(3) | -2b(3) | sqhi | sqlo | 1 | 1]
            # (compute in partition-0 tiles, assemble via sbuf-to-sbuf DMA)
            lhsT16 = top.tile([13, RPAD], dt.float16, tag="lhsT16")
            pd = est.enter_context(tc.tile_pool(name="pd", bufs=1))
            a_own = pd.tile([4, RPAD], dt.float16, tag="a_own")
            nc.vector.tensor_copy(out=a_own[:], in_=cc_sb[:])
            b_own = pd.tile([4, RPAD], dt.float16, tag="b_own")
            nc.vector.tensor_tensor(out=b_own[:], in0=cc_sb[:], in1=a_own[:],
                                    op=OP.subtract)
            na4 = pd.tile([4, RPAD], dt.float16, tag="na4")
            nc.scalar.activation(out=na4[:], in_=a_own[:], func=AF.Copy, scale=-2.0)
            nb4 = pd.tile([4, RPAD], dt.float16, tag="nb4")
            nc.scalar.activation(out=nb4[:], in_=b_own[:], func=AF.Copy, scale=-2.0)
            ones_r = pd.tile([2, RPAD], dt.float16, tag="ones_r")
            nc.vector.memset(ones_r[:], 1.0)
            nc.sync.dma_start(out=lhsT16[0:3, :], in_=na4[0:3, :])
            nc.sync.dma_start(out=lhsT16[3:6, :], in_=na4[0:3, :])
            nc.sync.dma_start(out=lhsT16[6:9, :], in_=nb4[0:3, :])
            nc.sync.dma_start(out=lhsT16[9:10, :], in_=a_own[3:4, :])
            nc.sync.dma_start(out=lhsT16[10:11, :], in_=b_own[3:4, :])
            nc.sync.dma_start(out=lhsT16[11:13, :], in_=ones_r[:])

            # rhs16 rows: [a(3) | b(3) | a(3) | 1 | 1 | sqhi | sqlo]
            rhs_f = pd.tile([4, NCOL], dt.float32, tag="rhs_f")
            nc.vector.memset(rhs_f[:, N:NCOL], 0.0)
            for s in range(CORES):
                c0 = s * SHARD
                nc.sync.dma_start(out=rhs_f[0:4, c0:c0 + HALF],
                                  in_=cc_outA[:][s])
                nc.sync.dma_start(out=rhs_f[0:4, c0 + HALF:c0 + SHARD],
                                  in_=cc_outB[:][s, 0:4, 0:SHARD - HALF])
            a16 = pd.tile([4, NCOL], dt.float16, tag="a16")
            nc.vector.tensor_copy(out=a16[:], in_=rhs_f[:])                  # a | sqhi
            b16 = pd.tile([4, NCOL], dt.float16, tag="b16")
            nc.vector.tensor_tensor(out=b16[:], in0=rhs_f[:], in1=a16[:],
                                    op=OP.subtract)                          # b | sqlo
            ones_n = pd.tile([2, NCOL], dt.float16, tag="ones_n")
            nc.vector.memset(ones_n[:], 1.0)
            rhs16 = top.tile([13, NCOL], dt.float16, tag="rhs16")
            nc.sync.dma_start(out=rhs16[0:3, :], in_=a16[0:3, :])
            nc.sync.dma_start(out=rhs16[3:6, :], in_=b16[0:3, :])
            nc.sync.dma_start(out=rhs16[6:9, :], in_=a16[0:3, :])
            nc.sync.dma_start(out=rhs16[9:11, :], in_=ones_n[:])
            nc.sync.dma_start(out=rhs16[11:12, :], in_=a16[3:4, :])
            nc.sync.dma_start(out=rhs16[12:13, :], in_=b16[3:4, :])

            if phases == "ABCD":
                nc.sync.dma_start(out=outD[0:13, 0:NCOL], in_=rhs16[:])
                raise _PhaseDone()

            # ---- phase E: cdist row-block x col-chunk (split-fp16 matmul, fp16 out) ----
            MRG = 4   # psum chunks merged into one output tile/DMA
            with tc.tile_pool(name="pe_d", bufs=3) as pe_d, \
                 tc.tile_pool(name="pe_d2", bufs=3) as pe_d2, \
                 tc.tile_pool(name="pe_ps", bufs=6, space="PSUM") as pe_ps:
                for rb in range(RB):
                    for mg in range(NCOL // (CCH * MRG)):
                        d2t = pe_d2.tile([128, CCH * MRG], dt.float16, tag="d2t")
                        for k in range(MRG):
                            ch = mg * MRG + k
                            dp = pe_ps.tile([128, CCH], dt.float32, tag="dp", space="PSUM")
                            nc.tensor.matmul(
                                dp[:], lhsT16[:, rb * 128:(rb + 1) * 128],
                                rhs16[:, ch * CCH:(ch + 1) * CCH],
                                start=True, stop=True)
                            nc.scalar.activation(out=d2t[:, k * CCH:(k + 1) * CCH],
                                                 in_=dp[:], func=AF.Sqrt,
                                                 bias=eps4_col[:, 0:1])
                        nc.sync.dma_start(
                            out=outD[rb * 128:(rb + 1) * 128,
                                     mg * CCH * MRG:(mg + 1) * CCH * MRG],
                            in_=d2t[:])

    except _PhaseDone:
        pass
    _tc_cm.__exit__(None, None, None)
    nc.compile()
    return nc


def _prep_host(x, edge_index):
    xp = np.zeros((NPAD, FIN), np.float32)
    xp[:N] = np.asarray(x, np.float32)
    xp16 = np.ascontiguousarray(xp.T.astype(np.float16))  # [256, NPAD]

    ei = np.asarray(edge_index)
    src = ei[0].astype(np.int64)
    dst = ei[1].astype(np.int64)

    core = dst // SHARD
    per_core = []
    max_tiles = 0
    for c in range(CORES):
        sel = core == c
        s_c = src[sel]
        d_c = dst[sel]
        loc = d_c - c * SHARD
        blk = loc // 128
        dl = loc - blk * 128
        blocks = []
        for b in range(RB):
            m = blk == b
            blocks.append((s_c[m], dl[m]))
            # tile 0 holds the block's self-loop edges; rest start at tile 1
            max_tiles = max(max_tiles, 1 + (len(blocks[-1][0]) + 127) // 128)
        per_core.append(blocks)

    TB = GC * ((max_tiles + GC - 1) // GC)
    S16 = 8 * TB
    NE = TB * 128

    hidx = np.zeros((CORES, RB, 128, TB), np.int32)
    ohtab = np.zeros((CORES, RB, 128, NE), np.float16)
    ohTtab = np.zeros((CORES, RB, 128, NE), np.float16)
    for c in range(CORES):
        for b in range(RB):
            # tile 0: self loops (edge at partition p has src=dst=block row p)
            rows = c * SHARD + b * 128 + np.arange(128)
            real = rows < N
            crows = np.minimum(rows, N - 1)
            jr = np.arange(128)
            hidx[c, b, jr, 0] = crows.astype(np.int32)
            pr = jr[real]
            ohtab[c, b, pr, pr] = 1.0
            ohTtab[c, b, pr, pr] = 1.0
            # remaining edges from tile 1 on
            s_b, dl_b = per_core[c][b]
            n = len(s_b)
            js = 128 + np.arange(n)
            p = js % 128          # edge partition
            t = js // 128         # edge tile (>= 1)
            hidx[c, b, p, t] = s_b.astype(np.int32)
            sl = dl_b.astype(np.int64)
            ohtab[c, b, p, t * 128 + sl] = 1.0
            ohTtab[c, b, sl, t * 128 + p] = 1.0
    return xp16, hidx, ohtab, ohTtab, TB


def build_in_maps(inputs):
    xp16, hidx, ohtab, ohTtab, TB = _prep_host(inputs["x"], inputs["edge_index"])

    def bc(vec, n, f16=False):
        v = np.asarray(vec, np.float32).reshape(1, n)
        out = np.ascontiguousarray(np.broadcast_to(v, (128, n)).copy())
        return out.astype(np.float16) if f16 else out

    # rhsA: [256 (xfeat, 2 chunks of 128), 262] fp16
    # cols: 0:128 WgT head0 | 128 zero | 129:257 WgT head1 | 257 zero | 258:262 wtil
    Wg = np.asarray(inputs["W_gat"], np.float32)       # [256, 256] rows = H*F out
    att_src = np.asarray(inputs["att_src"], np.float32)  # [2, 128]
    att_dst = np.asarray(inputs["att_dst"], np.float32)
    rhsA = np.zeros((256, 262), np.float32)
    rhsA[:, 0:128] = Wg[0:128, :].T
    rhsA[:, 129:257] = Wg[128:256, :].T
    rhsA[:, 258] = Wg[0:128, :].T @ att_src[0]
    rhsA[:, 259] = Wg[128:256, :].T @ att_src[1]
    rhsA[:, 260] = Wg[0:128, :].T @ att_dst[0]
    rhsA[:, 261] = Wg[128:256, :].T @ att_dst[1]
    rhsA16 = rhsA.astype(np.float16).reshape(2, 128, 262)

    Wa = np.asarray(inputs["Wa"], np.float32)  # [128, 256]
    waT16 = np.ascontiguousarray(Wa.T.astype(np.float16)).reshape(2, 128, 128)
    w1T16 = np.ascontiguousarray(np.asarray(inputs["W1"], np.float32).T.astype(np.float16))
    w2T16 = np.ascontiguousarray(np.asarray(inputs["W2"], np.float32).T.astype(np.float16))
    w3T16 = np.ascontiguousarray(np.asarray(inputs["W3"], np.float32).T.astype(np.float16))

    shared = {
        "xt16": xp16,
        "rhsA": np.ascontiguousarray(rhsA16),
        "waT": waT16, "w1T": w1T16, "w2T": w2T16, "w3T": w3T16,
        "bgat_bc": bc(inputs["b_gat"], 256, f16=True),
        "ba_bc": bc(inputs["ba"], 128),
        "lnaw_bc": bc(inputs["lna_w"], 128), "lnab_bc": bc(inputs["lna_b"], 128),
        "b1_bc": bc(inputs["b1"], 64),
        "ln1w_bc": bc(inputs["ln1_w"], 64), "ln1b_bc": bc(inputs["ln1_b"], 64),
        "b2_bc": bc(inputs["b2"], 32),
        "ln2w_bc": bc(inputs["ln2_w"], 32), "ln2b_bc": bc(inputs["ln2_b"], 32),
        "b3_bc": bc(inputs["b3"], 3),
    }
    in_maps = [
        {**shared, "hidxI": np.ascontiguousarray(hidx[c]),
         "ohtab": np.ascontiguousarray(ohtab[c]),
         "ohTtab": np.ascontiguousarray(ohTtab[c])}
        for c in range(CORES)
    ]
    return in_maps, TB


def kernel(**inputs):
    in_maps, TB = build_in_maps(inputs)

    import os
    phases = os.environ.get("K_PHASES", "ABCDE")
    key = (TB, phases)
    if key not in _BUILD_CACHE:
        _BUILD_CACHE[key] = _build(TB, phases)
    nc = _BUILD_CACHE[key]
    res = run_bass_kernel_spmd(nc, in_maps, core_ids=list(range(CORES)))
    global _LAST_RESULTS
    _LAST_RESULTS = res.results
    out = np.empty((N, N), np.float32)
    for c in range(CORES):
        out[c * SHARD:(c + 1) * SHARD, :] = \
            res.results[c]["outD"][:SHARD, :N].astype(np.float32)
    return out


# revision 21
# speedup vs baseline: 2.3995x; 1.0114x over previous
"""GAT (2-head) + 3x dense/LayerNorm + pairwise-distance kernel for 8 TRN2 NeuronCores.

Strategy: dst-sharded edge processing (one-hot matmul aggregation), replicated
small dense weights, row-block-sharded NxN cdist output.

v3: fp16 htable/gather rows, host-precomputed one-hot tables (no on-device
is_eq), 2-queue GC=12 gathers, whole-x preload, split-fp16 cdist matmuls,
fp16 output.
"""
import sys

import numpy as np

# Environment bootstrap (harness may run from a bare directory).
for _p in ("/root/.axon_site", "/root/.axon_site/_ro/trn_rl_repo",
           "/root/.axon_site/_ro/pypackages", "/opt/trn_rl_repo"):
    if _p not in sys.path:
        sys.path.append(_p)

import concourse.bass as bass
import concourse.bacc as bacc
import concourse.mybir as mybir
import concourse.tile as tile
from concourse.masks import make_identity
from concourse.bass_utils import run_bass_kernel_spmd

dt = mybir.dt
OP = mybir.AluOpType
AF = mybir.ActivationFunctionType

N = 10000
NPAD = 10112          # 79 * 128
NB = 79               # node blocks (phase A)
FIN = 256
F = 128               # per-head GAT dim
H = 2
HROW = 384            # htable row fp16 elems (768B, multiple of 256B)
CORES = 8
SHARD = 1250          # dst nodes per core
RB = 10               # dst blocks per core
RPAD = 1280
CCH = 512             # cdist column chunk
NCOL = 10240          # padded output columns
EPS = 1e-5

_BUILD_CACHE = {}
_LAST_RESULTS = None


GC = 6   # tile groups per dma_gather call (768 idxs = 48 descs/engine)
SUB = 3  # tiles per a_d/exp subchunk


def _build(TB, phases="ABCDE"):
    """Build the 8-core SPMD program. TB = gather tile groups per dst block (mult of GC)."""
    assert TB % GC == 0
    NCH = TB // GC
    S16 = 8 * TB          # idx columns ([128, S16] wrapped int16)

    nc = bacc.Bacc("TRN2", target_bir_lowering=False, debug=False,
                   num_devices=CORES, num_swdge_queues=2)

    def din(name, shape, d=dt.float32):
        return nc.dram_tensor(name, shape, d, kind="ExternalInput").ap()

    xt16 = din("xt16", [FIN, NPAD], dt.float16)
    rhsA = din("rhsA", [2, 128, 262], dt.float16)
    waT = din("waT", [2, 128, 128], dt.float16)
    w1T = din("w1T", [128, 64], dt.float16)
    w2T = din("w2T", [64, 32], dt.float16)
    w3T = din("w3T", [32, 3], dt.float16)
    bgat_bc = din("bgat_bc", [128, 256], dt.float16)
    ba_bc = din("ba_bc", [128, 128])
    lnaw_bc = din("lnaw_bc", [128, 128])
    lnab_bc = din("lnab_bc", [128, 128])
    b1_bc = din("b1_bc", [128, 64])
    ln1w_bc = din("ln1w_bc", [128, 64])
    ln1b_bc = din("ln1b_bc", [128, 64])
    b2_bc = din("b2_bc", [128, 32])
    ln2w_bc = din("ln2w_bc", [128, 32])
    ln2b_bc = din("ln2b_bc", [128, 32])
    b3_bc = din("b3_bc", [128, 3])
    hidxI = din("hidxI", [RB, 128, TB], dt.int32)
    ohtab = din("ohtab", [RB, 128, TB * 128], dt.float16)    # [edge_p, t*128+slot]
    ohTtab = din("ohTtab", [RB, 128, TB * 128], dt.float16)  # [slot_p, t*128+edge]
    outD = nc.dram_tensor("outD", [RPAD, NCOL], dt.float16, kind="ExternalOutput").ap()

    class _PhaseDone(Exception):
        pass

    import contextlib
    try:
        _tc_cm = tile.TileContext(nc)
        tc = _tc_cm.__enter__()
        est = contextlib.ExitStack()
        with est:
            top = est.enter_context(tc.tile_pool(name="top", bufs=1))
            dram = est.enter_context(tc.tile_pool(name="dram", bufs=1, space="DRAM"))

            htable = dram.tile([NPAD, HROW], dt.float16, tag="htable")
            HALF = RPAD // 2
            cc_inA = dram.tile([4, HALF], dt.float32, tag="cc_inA")
            cc_outA = dram.tile([CORES, 4, HALF], dt.float32, tag="cc_outA")
            cc_inB = dram.tile([4, HALF], dt.float32, tag="cc_inB")
            cc_outB = dram.tile([CORES, 4, HALF], dt.float32, tag="cc_outB")

            ident = top.tile([128, 128], dt.float32, tag="ident")
            make_identity(nc, ident[:])
            ident16 = top.tile([128, 128], dt.float16, tag="ident16")
            nc.vector.tensor_copy(out=ident16[:], in_=ident[:])
            eps_col = top.tile([128, 1], dt.float32, tag="eps_col")
            nc.vector.memset(eps_col[:], EPS)
            eps4_col = top.tile([128, 1], dt.float32, tag="eps4_col")
            nc.vector.memset(eps4_col[:], 1e-4)

            # ---- load replicated weights / biases into SBUF ----
            def ldw(name, ap, shape, d=dt.float32):
                t = top.tile(shape, d, tag=name)
                nc.sync.dma_start(out=t[:], in_=ap)
                return t

            rhsA0 = ldw("rhsA0", rhsA[0], [128, 262], dt.float16)
            rhsA1 = ldw("rhsA1", rhsA[1], [128, 262], dt.float16)
            waT0 = ldw("waT0", waT[0], [128, 128], dt.float16)
            waT1 = ldw("waT1", waT[1], [128, 128], dt.float16)
            w1T_sb = ldw("w1T_sb", w1T[:], [128, 64], dt.float16)
            w2T_sb = ldw("w2T_sb", w2T[:], [64, 32], dt.float16)
            w3T_sb = ldw("w3T_sb", w3T[:], [32, 3], dt.float16)
            bgat_sb = ldw("bgat_sb", bgat_bc[:], [128, 256], dt.float16)
            ba_sb = ldw("ba_sb", ba_bc[:], [128, 128])
            lnaw_sb = ldw("lnaw_sb", lnaw_bc[:], [128, 128])
            lnab_sb = ldw("lnab_sb", lnab_bc[:], [128, 128])
            b1_sb = ldw("b1_sb", b1_bc[:], [128, 64])
            ln1w_sb = ldw("ln1w_sb", ln1w_bc[:], [128, 64])
            ln1b_sb = ldw("ln1b_sb", ln1b_bc[:], [128, 64])
            b2_sb = ldw("b2_sb", b2_bc[:], [128, 32])
            ln2w_sb = ldw("ln2w_sb", ln2w_bc[:], [128, 32])
            ln2b_sb = ldw("ln2b_sb", ln2b_bc[:], [128, 32])
            b3_sb = ldw("b3_sb", b3_bc[:], [128, 3])

            # ---- phase A: htable rows [h0 | 1 | h1 | 1 | a_s(2) a_d(2)] fp16 ----
            with tc.tile_pool(name="pa", bufs=1) as pa, \
                 tc.tile_pool(name="pa_ht", bufs=4) as pa_ht, \
                 tc.tile_pool(name="pa_h", bufs=4, space="PSUM") as pa_h:
                xta = pa.tile([128, NPAD], dt.float16, tag="xta")
                xtb = pa.tile([128, NPAD], dt.float16, tag="xtb")
                NH = 2560
                nc.sync.dma_start(out=xta[:, 0:NH], in_=xt16[0:128, 0:NH])
                nc.sync.dma_start(out=xtb[:, 0:NH], in_=xt16[128:256, 0:NH])
                nc.sync.dma_start(out=xta[:, NH:NPAD], in_=xt16[0:128, NH:NPAD])
                nc.sync.dma_start(out=xtb[:, NH:NPAD], in_=xt16[128:256, NH:NPAD])
                for i in range(NB):
                    r0 = i * 128
                    hp = pa_h.tile([128, 262], dt.float32, tag="hp", space="PSUM")
                    nc.tensor.matmul(hp[:], xta[:, r0:r0 + 128], rhsA0[:],
                                     start=True, stop=False)
                    nc.tensor.matmul(hp[:], xtb[:, r0:r0 + 128], rhsA1[:],
                                     start=False, stop=True)
                    ht = pa_ht.tile([128, 262], dt.float16, tag="ht")
                    if i % 2 == 0:
                        nc.scalar.copy(out=ht[:], in_=hp[:])
                    else:
                        nc.vector.tensor_copy(out=ht[:], in_=hp[:])
                    nc.gpsimd.memset(ht[:, 128:129], 1.0)
                    nc.gpsimd.memset(ht[:, 257:258], 1.0)
                    nc.sync.dma_start(out=htable[r0:r0 + 128, 0:262], in_=ht[:])

            if phases == "A":
                dbg = top.tile([128, 262], dt.float16, tag="dbgA")
                for i in range(RB):
                    nc.sync.dma_start(out=dbg[:], in_=htable[i * 128:(i + 1) * 128, 0:262])
                    nc.sync.dma_start(out=outD[i * 128:(i + 1) * 128, 0:262], in_=dbg[:])
                raise _PhaseDone()

            # ---- phase B: GAT aggregation per dst block ----
            xg_pool = est.enter_context(tc.tile_pool(name="xg", bufs=1))
            xgs = []
            with tc.tile_pool(name="pb_idx", bufs=2) as pb_idx, \
                 tc.tile_pool(name="pb_oht", bufs=3) as pb_oht, \
                 tc.tile_pool(name="pb_g", bufs=6) as pb_g, \
                 tc.tile_pool(name="pb_ad", bufs=2) as pb_ad, \
                 tc.tile_pool(name="pb_ex", bufs=4) as pb_ex, \
                 tc.tile_pool(name="pb_rhs", bufs=6) as pb_rhs, \
                 tc.tile_pool(name="pb_ep", bufs=2) as pb_ep, \
                 tc.tile_pool(name="pb_ps", bufs=2, space="PSUM") as pb_ps, \
                 tc.tile_pool(name="pb_adp", bufs=4, space="PSUM") as pb_adp:
                for _slot in range(6):
                    gz = pb_g.tile([128, GC, HROW], dt.float16, tag="g")
                    nc.vector.memset(gz[:], 0.0)
                for b in range(RB):
                    hix = pb_idx.tile([128, TB], dt.int32, tag="hix")
                    nc.sync.dma_start(out=hix[:], in_=hidxI[b])
                    oh_sb = pb_oht.tile([128, TB * 128], dt.float16, tag="oh_sb")
                    nc.sync.dma_start(out=oh_sb[:], in_=ohtab[b])
                    ohT_sb = pb_oht.tile([128, TB * 128], dt.float16, tag="ohT_sb")
                    nc.sync.dma_start(out=ohT_sb[:], in_=ohTtab[b])

                    adblk = pb_ad.tile([128, 2], dt.float16, tag="adblk")
                    ps = pb_ps.tile([128, 258], dt.float32, tag="ps", space="PSUM")

                    for c in range(NCH):
                        g = pb_g.tile([128, GC, HROW], dt.float16, tag="g")
                        for tl in range(GC):
                            nc.gpsimd.indirect_dma_start(
                                out=g[:, tl, :], out_offset=None,
                                in_=htable[:],
                                in_offset=bass.IndirectOffsetOnAxis(
                                    ap=hix[:, c * GC + tl:c * GC + tl + 1], axis=0))
                        if c == 0:
                            # tile 0 rows are this block's own dst rows (self loops)
                            nc.vector.tensor_copy(out=adblk[:], in_=g[:, 0, 260:262])
                        for hs in range(GC // SUB):
                            t0 = c * GC + hs * SUB
                            adps = pb_adp.tile([128, SUB, 2], dt.float32, tag="adps",
                                               space="PSUM")
                            for k in range(SUB):
                                t = t0 + k
                                nc.tensor.matmul(adps[:, k, :],
                                                 ohT_sb[:, t * 128:(t + 1) * 128],
                                                 adblk[:], start=True, stop=True)
                            # e = a_s[src] + a_d[dst]; exv = max(exp(e), exp(0.2e))
                            exr = pb_ex.tile([128, SUB, 2], dt.float32, tag="exr")
                            nc.vector.tensor_tensor(
                                out=exr[:], in0=adps[:],
                                in1=g[:, hs * SUB:(hs + 1) * SUB, 258:260], op=OP.add)
                            exn = pb_ex.tile([128, SUB, 2], dt.float32, tag="exn")
                            nc.scalar.activation(out=exn[:], in_=exr[:], func=AF.Exp,
                                                 scale=0.2)
                            exp_ = pb_ex.tile([128, SUB, 2], dt.float32, tag="exp_")
                            nc.scalar.activation(out=exp_[:], in_=exr[:], func=AF.Exp)
                            exv = pb_ex.tile([128, SUB, 2], dt.float32, tag="exv")
                            nc.vector.tensor_tensor(out=exv[:], in0=exp_[:], in1=exn[:],
                                                    op=OP.max)
                            for k in range(SUB):
                                t = t0 + k
                                tl = hs * SUB + k
                                rhs = pb_rhs.tile([128, 258], dt.float16, tag="rhs")
                                if t % 2 == 0:
                                    nc.scalar.activation(out=rhs[:, 0:129],
                                                         in_=g[:, tl, 0:129],
                                                         func=AF.Copy,
                                                         scale=exv[:, k, 0:1])
                                    nc.vector.tensor_scalar(
                                        out=rhs[:, 129:258], in0=g[:, tl, 129:258],
                                        scalar1=exv[:, k, 1:2], scalar2=None, op0=OP.mult)
                                else:
                                    nc.vector.tensor_scalar(
                                        out=rhs[:, 0:129], in0=g[:, tl, 0:129],
                                        scalar1=exv[:, k, 0:1], scalar2=None, op0=OP.mult)
                                    nc.scalar.activation(out=rhs[:, 129:258],
                                                         in_=g[:, tl, 129:258],
                                                         func=AF.Copy,
                                                         scale=exv[:, k, 1:2])
                                nc.tensor.matmul(ps[:],
                                                 oh_sb[:, t * 128:(t + 1) * 128],
                                                 rhs[:],
                                                 start=(t == 0), stop=(t == TB - 1))

                    # epilogue: normalize, +b_gat, lrelu(0.01) -> fp16 xg
                    rec0 = pb_ep.tile([128, 1], dt.float32, tag="rec0")
                    nc.vector.reciprocal(rec0[:], ps[:, 128:129])
                    rec1 = pb_ep.tile([128, 1], dt.float32, tag="rec1")
                    nc.vector.reciprocal(rec1[:], ps[:, 257:258])
                    xg = xg_pool.tile([128, 256], dt.float16, tag=f"xg{b}")
                    nc.scalar.activation(out=xg[:, 0:128], in_=ps[:, 0:128],
                                         func=AF.Copy, scale=rec0[:])
                    nc.scalar.activation(out=xg[:, 128:256], in_=ps[:, 129:257],
                                         func=AF.Copy, scale=rec1[:])
                    nc.vector.tensor_tensor(out=xg[:], in0=xg[:], in1=bgat_sb[:], op=OP.add)
                    ng = pb_ep.tile([128, 256], dt.float16, tag="ng")
                    nc.vector.tensor_scalar(out=ng[:], in0=xg[:], scalar1=0.0,
                                            scalar2=0.01, op0=OP.min, op1=OP.mult)
                    nc.vector.scalar_tensor_tensor(out=xg[:], in0=xg[:], scalar=0.0,
                                                   in1=ng[:], op0=OP.max, op1=OP.add)
                    xgs.append(xg)

            if phases == "AB":
                for b in range(RB):
                    nc.sync.dma_start(out=outD[b * 128:(b + 1) * 128, 0:256], in_=xgs[b][:])
                raise _PhaseDone()

            # ---- phase C: dense + LN on own shard (stage-parallel across blocks) ----
            cc_sb = top.tile([4, RPAD], dt.float32, tag="cc_sb")
            with tc.tile_pool(name="pc", bufs=12) as pc, \
                 tc.tile_pool(name="pc_ps", bufs=2, space="PSUM") as pc_ps, \
                 tc.tile_pool(name="pc_mm", bufs=4, space="PSUM") as pc_mm:

                def transpose16(xin, pdim, fdim):
                    # xin fp16 [pdim, fdim] -> sbuf fp16 [fdim, pdim]
                    p = pc_ps.tile([128, 128], dt.float16, tag="tpp16", space="PSUM")
                    nc.tensor.transpose(out=p[:fdim, 0:pdim], in_=xin,
                                        identity=ident16[:pdim, :pdim])
                    s = pc.tile([fdim, pdim], dt.float16, tag=f"tt{fdim}_{pdim}")
                    nc.scalar.copy(out=s[:], in_=p[:fdim, 0:pdim])
                    return s

                def c_chain(b):
                    x0 = xgs[b]
                    xt0 = transpose16(x0[:, 0:128], 128, 128)
                    yield
                    xt1 = transpose16(x0[:, 128:256], 128, 128)
                    yield
                    pA = pc_mm.tile([128, 128], dt.float32, tag="mm", space="PSUM")
                    nc.tensor.matmul(pA[:], xt0[:], waT0[:], start=True, stop=False)
                    nc.tensor.matmul(pA[:], xt1[:], waT1[:], start=False, stop=True)
                    yield
                    x1 = yield from ln_lrelu(b, pA[:], 128, ba_sb, lnaw_sb, lnab_sb)
                    x1t = transpose16(x1[:], 128, 128)
                    yield
                    p1 = pc_mm.tile([128, 64], dt.float32, tag="mm", space="PSUM")
                    nc.tensor.matmul(p1[:], x1t[:], w1T_sb[:], start=True, stop=True)
                    yield
                    x2 = yield from ln_lrelu(b, p1[:], 64, b1_sb, ln1w_sb, ln1b_sb)
                    x2t = transpose16(x2[:], 128, 64)
                    yield
                    p2 = pc_mm.tile([128, 32], dt.float32, tag="mm", space="PSUM")
                    nc.tensor.matmul(p2[:], x2t[:], w2T_sb[:], start=True, stop=True)
                    yield
                    x3 = yield from ln_lrelu(b, p2[:], 32, b2_sb, ln2w_sb, ln2b_sb)
                    x3t = transpose16(x3[:], 128, 32)
                    yield
                    p3 = pc_mm.tile([128, 3], dt.float32, tag="mm", space="PSUM")
                    nc.tensor.matmul(p3[:], x3t[:], w3T_sb[:], start=True, stop=True)
                    yield
                    y3 = pc.tile([128, 4], dt.float32, tag="y3")
                    nc.vector.tensor_tensor(out=y3[:, 0:3], in0=p3[:], in1=b3_sb[:], op=OP.add)
                    scr3 = pc.tile([128, 3], dt.float32, tag="scr3")
                    nc.scalar.activation(out=scr3[:], in_=y3[:, 0:3], func=AF.Square,
                                         accum_out=y3[:, 3:4])
                    yield
                    h3p = pc_ps.tile([128, 128], dt.float32, tag="tpp", space="PSUM")
                    nc.tensor.transpose(out=h3p[:4, 0:128], in_=y3[:], identity=ident[:])
                    nc.scalar.copy(out=cc_sb[:, b * 128:(b + 1) * 128], in_=h3p[:4, 0:128])

                def ln_lrelu(b, xin, fdim, bias_bc, w_bc, b_bc):
                    # y = xin + bias; u = LN(y)*w + b; return lrelu001(u) fp16
                    y = pc.tile([128, fdim], dt.float32, tag=f"y{fdim}")
                    nc.vector.tensor_tensor(out=y[:], in0=xin, in1=bias_bc[:], op=OP.add)
                    yield
                    scr = pc.tile([128, fdim], dt.float32, tag=f"scr{fdim}")
                    msum = pc.tile([128, 1], dt.float32, tag="msum")
                    nc.scalar.activation(out=scr[:], in_=y[:], func=AF.Copy,
                                         accum_out=msum[:])
                    sqs = pc.tile([128, 1], dt.float32, tag="sqs")
                    nc.scalar.activation(out=scr[:], in_=y[:], func=AF.Square,
                                         accum_out=sqs[:])
                    yield
                    mean = pc.tile([128, 1], dt.float32, tag="mean")
                    nc.vector.tensor_scalar(out=mean[:], in0=msum[:], scalar1=1.0 / fdim,
                                            scalar2=None, op0=OP.mult)
                    var = pc.tile([128, 1], dt.float32, tag="var")
                    nc.vector.tensor_scalar(out=var[:], in0=sqs[:], scalar1=1.0 / fdim,
                                            scalar2=None, op0=OP.mult)
                    m2 = pc.tile([128, 1], dt.float32, tag="m2")
                    nc.vector.tensor_scalar(out=m2[:], in0=mean[:], scalar1=mean[:, 0:1],
                                            scalar2=None, op0=OP.mult)
                    nc.vector.tensor_tensor(out=var[:], in0=var[:], in1=m2[:], op=OP.subtract)
                    sd = pc.tile([128, 1], dt.float32, tag="sd")
                    nc.scalar.activation(out=sd[:], in_=var[:], func=AF.Sqrt,
                                         bias=eps_col[:, 0:1])
                    rstd = pc.tile([128, 1], dt.float32, tag="rstd")
                    nc.vector.reciprocal(rstd[:], sd[:])
                    yield
                    u = pc.tile([128, fdim], dt.float32, tag=f"u{fdim}")
                    nc.vector.scalar_tensor_tensor(out=u[:], in0=y[:], scalar=mean[:, 0:1],
                                                   in1=w_bc[:], op0=OP.subtract, op1=OP.mult)
                    nc.vector.scalar_tensor_tensor(out=u[:], in0=u[:], scalar=rstd[:, 0:1],
                                                   in1=b_bc[:], op0=OP.mult, op1=OP.add)
                    yield
                    ngt = pc.tile([128, fdim], dt.float32, tag=f"ng{fdim}")
                    nc.vector.tensor_scalar(out=ngt[:], in0=u[:], scalar1=0.0,
                                            scalar2=0.01, op0=OP.min, op1=OP.mult)
                    u16 = pc.tile([128, fdim], dt.float16, tag=f"u16_{fdim}")
                    nc.vector.scalar_tensor_tensor(out=u16[:], in0=u[:], scalar=0.0,
                                                   in1=ngt[:], op0=OP.max, op1=OP.add)
                    yield
                    return u16

                gens = [c_chain(b) for b in range(RB)]

                def run_gens(idxs):
                    done = {b: False for b in idxs}
                    while not all(done.values()):
                        for b in idxs:
                            if not done[b]:
                                try:
                                    next(gens[b])
                                except StopIteration:
                                    done[b] = True

                run_gens(range(RB // 2))
                nc.sync.dma_start(out=cc_inA[:], in_=cc_sb[:, 0:HALF])
                nc.gpsimd.collective_compute(
                    "AllGather", OP.bypass, replica_groups=[list(range(CORES))],
                    ins=[cc_inA[:].opt()], outs=[cc_outA[:].opt()])
                run_gens(range(RB // 2, RB))
                nc.sync.dma_start(out=cc_inB[:], in_=cc_sb[:, HALF:RPAD])
                nc.gpsimd.collective_compute(
                    "AllGather", OP.bypass, replica_groups=[list(range(CORES))],
                    ins=[cc_inB[:].opt()], outs=[cc_outB[:].opt()])

            if phases == "ABC":
                dbg16 = top.tile([4, RPAD], dt.float16, tag="dbgc")
                nc.vector.tensor_copy(out=dbg16[:], in_=cc_sb[:])
                nc.sync.dma_start(out=outD[0:4, 0:RPAD], in_=dbg16[:])
                raise _PhaseDone()

            # ---- phase D: build split-fp16 cdist operands ----
            # lhsT16 rows: [-2a(3) | -2a(3) | -2b(3) | sqhi | sqlo | 1 | 1]
            # (compute in partition-0 tiles, assemble via sbuf-to-sbuf DMA)
            lhsT16 = top.tile([13, RPAD], dt.float16, tag="lhsT16")
            pd = est.enter_context(tc.tile_pool(name="pd", bufs=1))
            a_own = pd.tile([4, RPAD], dt.float16, tag="a_own")
            nc.vector.tensor_copy(out=a_own[:], in_=cc_sb[:])
            b_own = pd.tile([4, RPAD], dt.float16, tag="b_own")
            nc.vector.tensor_tensor(out=b_own[:], in0=cc_sb[:], in1=a_own[:],
                                    op=OP.subtract)
            na4 = pd.tile([4, RPAD], dt.float16, tag="na4")
            nc.scalar.activation(out=na4[:], in_=a_own[:], func=AF.Copy, scale=-2.0)
            nb4 = pd.tile([4, RPAD], dt.float16, tag="nb4")
            nc.scalar.activation(out=nb4[:], in_=b_own[:], func=AF.Copy, scale=-2.0)
            ones_r = pd.tile([2, RPAD], dt.float16, tag="ones_r")
            nc.vector.memset(ones_r[:], 1.0)
            nc.sync.dma_start(out=lhsT16[0:3, :], in_=na4[0:3, :])
            nc.sync.dma_start(out=lhsT16[3:6, :], in_=na4[0:3, :])
            nc.sync.dma_start(out=lhsT16[6:9, :], in_=nb4[0:3, :])
            nc.sync.dma_start(out=lhsT16[9:10, :], in_=a_own[3:4, :])
            nc.sync.dma_start(out=lhsT16[10:11, :], in_=b_own[3:4, :])
            nc.sync.dma_start(out=lhsT16[11:13, :], in_=ones_r[:])

            # rhs16 rows: [a(3) | b(3) | a(3) | 1 | 1 | sqhi | sqlo]
            rhs_f = pd.tile([4, NCOL], dt.float32, tag="rhs_f")
            nc.vector.memset(rhs_f[:, N:NCOL], 0.0)
            for s in range(CORES):
                c0 = s * SHARD
                nc.sync.dma_start(out=rhs_f[0:4, c0:c0 + HALF],
                                  in_=cc_outA[:][s])
                nc.sync.dma_start(out=rhs_f[0:4, c0 + HALF:c0 + SHARD],
                                  in_=cc_outB[:][s, 0:4, 0:SHARD - HALF])
            a16 = pd.tile([4, NCOL], dt.float16, tag="a16")
            nc.vector.tensor_copy(out=a16[:], in_=rhs_f[:])                  # a | sqhi
            b16 = pd.tile([4, NCOL], dt.float16, tag="b16")
            nc.vector.tensor_tensor(out=b16[:], in0=rhs_f[:], in1=a16[:],
                                    op=OP.subtract)                          # b | sqlo
            ones_n = pd.tile([2, NCOL], dt.float16, tag="ones_n")
            nc.vector.memset(ones_n[:], 1.0)
            rhs16 = top.tile([13, NCOL], dt.float16, tag="rhs16")
            nc.sync.dma_start(out=rhs16[0:3, :], in_=a16[0:3, :])
            nc.sync.dma_start(out=rhs16[3:6, :], in_=b16[0:3, :])
            nc.sync.dma_start(out=rhs16[6:9, :], in_=a16[0:3, :])
            nc.sync.dma_start(out=rhs16[9:11, :], in_=ones_n[:])
            nc.sync.dma_start(out=rhs16[11:12, :], in_=a16[3:4, :])
            nc.sync.dma_start(out=rhs16[12:13, :], in_=b16[3:4, :])

            if phases == "ABCD":
                nc.sync.dma_start(out=outD[0:13, 0:NCOL], in_=rhs16[:])
                raise _PhaseDone()

            # ---- phase E: cdist row-block x col-chunk (split-fp16 matmul, fp16 out) ----
            MRG = 4   # psum chunks merged into one output tile/DMA
            with tc.tile_pool(name="pe_d", bufs=3) as pe_d, \
                 tc.tile_pool(name="pe_d2", bufs=3) as pe_d2, \
                 tc.tile_pool(name="pe_ps", bufs=6, space="PSUM") as pe_ps:
                for rb in range(RB):
                    for mg in range(NCOL // (CCH * MRG)):
                        d2t = pe_d2.tile([128, CCH * MRG], dt.float16, tag="d2t")
                        for k in range(MRG):
                            ch = mg * MRG + k
                            dp = pe_ps.tile([128, CCH], dt.float32, tag="dp", space="PSUM")
                            nc.tensor.matmul(
                                dp[:], lhsT16[:, rb * 128:(rb + 1) * 128],
                                rhs16[:, ch * CCH:(ch + 1) * CCH],
                                start=True, stop=True)
                            nc.scalar.activation(out=d2t[:, k * CCH:(k + 1) * CCH],
                                                 in_=dp[:], func=AF.Sqrt,
                                                 bias=eps4_col[:, 0:1])
                        nc.sync.dma_start(
                            out=outD[rb * 128:(rb + 1) * 128,
                                     mg * CCH * MRG:(mg + 1) * CCH * MRG],
                            in_=d2t[:])

    except _PhaseDone:
        pass
    _tc_cm.__exit__(None, None, None)
    nc.compile()
    return nc


def _prep_host(x, edge_index):
    xp = np.zeros((NPAD, FIN), np.float32)
    xp[:N] = np.asarray(x, np.float32)
    xp16 = np.ascontiguousarray(xp.T.astype(np.float16))  # [256, NPAD]

    ei = np.asarray(edge_index)
    src = ei[0].astype(np.int64)
    dst = ei[1].astype(np.int64)

    core = dst // SHARD
    per_core = []
    max_tiles = 0
    for c in range(CORES):
        sel = core == c
        s_c = src[sel]
        d_c = dst[sel]
        loc = d_c - c * SHARD
        blk = loc // 128
        dl = loc - blk * 128
        blocks = []
        for b in range(RB):
            m = blk == b
            blocks.append((s_c[m], dl[m]))
            # tile 0 holds the block's self-loop edges; rest start at tile 1
            max_tiles = max(max_tiles, 1 + (len(blocks[-1][0]) + 127) // 128)
        per_core.append(blocks)

    TB = GC * ((max_tiles + GC - 1) // GC)
    S16 = 8 * TB
    NE = TB * 128

    hidx = np.zeros((CORES, RB, 128, TB), np.int32)
    ohtab = np.zeros((CORES, RB, 128, NE), np.float16)
    ohTtab = np.zeros((CORES, RB, 128, NE), np.float16)
    for c in range(CORES):
        for b in range(RB):
            # tile 0: self loops (edge at partition p has src=dst=block row p)
            rows = c * SHARD + b * 128 + np.arange(128)
            real = rows < N
            crows = np.minimum(rows, N - 1)
            jr = np.arange(128)
            hidx[c, b, jr, 0] = crows.astype(np.int32)
            pr = jr[real]
            ohtab[c, b, pr, pr] = 1.0
            ohTtab[c, b, pr, pr] = 1.0
            # remaining edges from tile 1 on
            s_b, dl_b = per_core[c][b]
            n = len(s_b)
            js = 128 + np.arange(n)
            p = js % 128          # edge partition
            t = js // 128         # edge tile (>= 1)
            hidx[c, b, p, t] = s_b.astype(np.int32)
            sl = dl_b.astype(np.int64)
            ohtab[c, b, p, t * 128 + sl] = 1.0
            ohTtab[c, b, sl, t * 128 + p] = 1.0
    return xp16, hidx, ohtab, ohTtab, TB


def build_in_maps(inputs):
    xp16, hidx, ohtab, ohTtab, TB = _prep_host(inputs["x"], inputs["edge_index"])

    def bc(vec, n, f16=False):
        v = np.asarray(vec, np.float32).reshape(1, n)
        out = np.ascontiguousarray(np.broadcast_to(v, (128, n)).copy())
        return out.astype(np.float16) if f16 else out

    # rhsA: [256 (xfeat, 2 chunks of 128), 262] fp16
    # cols: 0:128 WgT head0 | 128 zero | 129:257 WgT head1 | 257 zero | 258:262 wtil
    Wg = np.asarray(inputs["W_gat"], np.float32)       # [256, 256] rows = H*F out
    att_src = np.asarray(inputs["att_src"], np.float32)  # [2, 128]
    att_dst = np.asarray(inputs["att_dst"], np.float32)
    rhsA = np.zeros((256, 262), np.float32)
    rhsA[:, 0:128] = Wg[0:128, :].T
    rhsA[:, 129:257] = Wg[128:256, :].T
    rhsA[:, 258] = Wg[0:128, :].T @ att_src[0]
    rhsA[:, 259] = Wg[128:256, :].T @ att_src[1]
    rhsA[:, 260] = Wg[0:128, :].T @ att_dst[0]
    rhsA[:, 261] = Wg[128:256, :].T @ att_dst[1]
    rhsA16 = rhsA.astype(np.float16).reshape(2, 128, 262)

    Wa = np.asarray(inputs["Wa"], np.float32)  # [128, 256]
    waT16 = np.ascontiguousarray(Wa.T.astype(np.float16)).reshape(2, 128, 128)
    w1T16 = np.ascontiguousarray(np.asarray(inputs["W1"], np.float32).T.astype(np.float16))
    w2T16 = np.ascontiguousarray(np.asarray(inputs["W2"], np.float32).T.astype(np.float16))
    w3T16 = np.ascontiguousarray(np.asarray(inputs["W3"], np.float32).T.astype(np.float16))

    shared = {
        "xt16": xp16,
        "rhsA": np.ascontiguousarray(rhsA16),
        "waT": waT16, "w1T": w1T16, "w2T": w2T16, "w3T": w3T16,
        "bgat_bc": bc(inputs["b_gat"], 256, f16=True),
        "ba_bc": bc(inputs["ba"], 128),
        "lnaw_bc": bc(inputs["lna_w"], 128), "lnab_bc": bc(inputs["lna_b"], 128),
        "b1_bc": bc(inputs["b1"], 64),
        "ln1w_bc": bc(inputs["ln1_w"], 64), "ln1b_bc": bc(inputs["ln1_b"], 64),
        "b2_bc": bc(inputs["b2"], 32),
        "ln2w_bc": bc(inputs["ln2_w"], 32), "ln2b_bc": bc(inputs["ln2_b"], 32),
        "b3_bc": bc(inputs["b3"], 3),
    }
    in_maps = [
        {**shared, "hidxI": np.ascontiguousarray(hidx[c]),
         "ohtab": np.ascontiguousarray(ohtab[c]),
         "ohTtab": np.ascontiguousarray(ohTtab[c])}
        for c in range(CORES)
    ]
    return in_maps, TB


def kernel(**inputs):
    in_maps, TB = build_in_maps(inputs)

    import os
    phases = os.environ.get("K_PHASES", "ABCDE")
    key = (TB, phases)
    if key not in _BUILD_CACHE:
        _BUILD_CACHE[key] = _build(TB, phases)
    nc = _BUILD_CACHE[key]
    res = run_bass_kernel_spmd(nc, in_maps, core_ids=list(range(CORES)))
    global _LAST_RESULTS
    _LAST_RESULTS = res.results
    out = np.empty((N, N), np.float32)
    for c in range(CORES):
        out[c * SHARD:(c + 1) * SHARD, :] = \
            res.results[c]["outD"][:SHARD, :N].astype(np.float32)
    return out
